# revision 1
# baseline (speedup 1.0000x reference)
"""Trainium2 Bass kernel for the YOLO/FCOS-layer loss (nn_FCOSLayer_22840636080477).

Sharding: data-parallel over batch, 2 images per NeuronCore x 8 cores, one
SPMD program. Host does label-side preprocessing (anchor matching, scatter
dedup, constant packing); device does everything that touches `raw`:

  loss = sum_cells softplus(conf) * (conf_mask & ~gt)          (dense, DVE+ACT)
       + sum_gtcells [ softplus(conf)-conf                      (sparse, gather)
                      + sum_c (softplus(cls_c) - onehot_c*cls_c)
                      + sum_4 (ltrb_raw - tgt)^2 ]

The ignore mask (`conf_mask`) needs a max-IoU scan of all 12288 pred boxes
against each image's gt boxes - that is the compute-heavy part, done with
broadcast-AP batched vector ops.
"""
import sys
import numpy as np

sys.path.insert(0, "/opt/trn_rl_repo")

N_CLS = 80
nA = 3
STRIDE = 8
IGNORE_THRE = 0.6
EPS = 1e-16
B = 16
K = 50
nG = 64
N_CORES = 8
P = 128
NCELL = nG * nG
f32 = np.float32

# tunables
KB = 4              # gt slots per batched op group
DTYPE_BF16 = False  # compute iou scan in bf16
DUP = 1             # duplicate gt scalars pairwise (bf16 2x packing aid)


# ---------------------------------------------------------------------------
# host-side label math (replicates reference.py semantics in f32 numpy)
# ---------------------------------------------------------------------------

def _host_precompute(labels, anchors_all, img_size):
    labels = np.asarray(labels, f32)
    anchors_all = np.asarray(anchors_all, f32)
    img_size = f32(img_size)
    anchors = anchors_all[:nA]
    norm_anch = anchors_all / img_size
    anch_w_n = anchors[:, 0] / img_size

    per_img = []
    for bb in range(B):
        lab = labels[bb]
        valid_row = lab.sum(-1) > 0
        tw, th = lab[:, 3], lab[:, 4]
        inter = np.minimum(tw[:, None], norm_anch[:, 0]) * np.minimum(
            th[:, None], norm_anch[:, 1]
        )
        union = tw[:, None] * th[:, None] + norm_anch[:, 0] * norm_anch[:, 1] - inter
        an_iou = inter / (union + f32(EPS))
        best_n_all = np.argmax(an_iou, axis=-1)
        best_n = best_n_all % nA
        valid = valid_row & (best_n_all < nA)

        ks = np.where(valid_row)[0]
        gcx, gcy, gw, gh = lab[ks, 1], lab[ks, 2], lab[ks, 3], lab[ks, 4]
        gt = dict(
            tlx=(gcx - gw / 2).astype(f32),
            tly=(gcy - gh / 2).astype(f32),
            brx=(gcx + gw / 2).astype(f32),
            bry=(gcy + gh / 2).astype(f32),
            area=(gw * gh).astype(f32),
        )

        tx = lab[:, 1] * nG
        ty = lab[:, 2] * nG
        ti = tx.astype(np.int32)
        tj = ty.astype(np.int32)
        tcls = lab[:, 0].astype(np.int32)
        lw, lh = lab[:, 3] * nG, lab[:, 4] * nG
        xc = np.floor(tx) + f32(0.5)
        yc = np.floor(ty) + f32(0.5)
        lab_ltrb = (
            np.maximum(
                np.stack(
                    [xc - (tx - lw / 2), yc - (ty - lh / 2),
                     (tx + lw / 2) - xc, (ty + lh / 2) - yc], -1),
                0.0,
            ) / f32(nG)
        ).astype(f32)
        cellmap = {}
        for k in range(K):
            if not valid[k]:
                continue
            key = (int(best_n[k]), int(tj[k]), int(ti[k]))
            tgt = np.log(lab_ltrb[k] / anch_w_n[best_n[k]] + f32(EPS)).astype(f32)
            if key not in cellmap:
                cellmap[key] = dict(tgt=tgt, cls=set([int(tcls[k])]))
            else:
                cellmap[key]["tgt"] = tgt  # scatter last-wins
                cellmap[key]["cls"].add(int(tcls[k]))
        per_img.append(dict(K=len(ks), gt=gt, cellmap=cellmap,
                            has_valid=bool(valid.any())))
    return per_img


def _pack_core_inputs(core, per_img, raw, anchors_all, img_size, S, NGmax):
    img_size = f32(img_size)
    thre = f32(IGNORE_THRE)
    cthre = (thre / (f32(1.0) + thre)).astype(f32)
    b0 = core * 2
    D4 = 4 * DUP

    rawsh = np.ascontiguousarray(raw[b0:b0 + 2]).reshape(2, 255, NCELL)

    # G5 [128, 5*S*D4]: comp-major {tlx,tly,brx,bry,athc}; within comp:
    # col = slot*4*DUP + strm*2*DUP + img*DUP + dup
    g5 = np.zeros((5, S, 2, 2, DUP), f32)
    g5[4] = cthre * f32(EPS)
    for im in range(2):
        info = per_img[b0 + im]
        gt = info["gt"]
        for k in range(info["K"]):
            strm, slot = k % 2, k // 2
            g5[0, slot, strm, im, :] = gt["tlx"][k]
            g5[1, slot, strm, im, :] = gt["tly"][k]
            g5[2, slot, strm, im, :] = gt["brx"][k]
            g5[3, slot, strm, im, :] = gt["bry"][k]
            g5[4, slot, strm, im, :] = cthre * (gt["area"][k] + f32(EPS))
    g5 = np.ascontiguousarray(
        np.broadcast_to(g5.reshape(1, 5 * S * D4), (P, 5 * S * D4))
    )

    # XYC [128, 192]: col = comp*96 + aq, cell q = 32p + (aq % 32)
    pidx = np.arange(P)[:, None]
    aqidx = np.arange(96)[None, :]
    q = 32 * pidx + (aqidx % 32)
    gx = (q % nG).astype(f32)
    gy = (q // nG).astype(f32)
    xyc = np.concatenate([(gx + f32(0.5)) / f32(nG), (gy + f32(0.5)) / f32(nG)],
                         axis=1).astype(f32)

    gtmask = np.zeros((P, 192), f32)
    gtplane = np.zeros((P, 192), f32)
    cells = []
    for im in range(2):
        info = per_img[b0 + im]
        for (a, j, i), d in info["cellmap"].items():
            cq = j * nG + i
            gtmask[cq // 32, im * 96 + a * 32 + cq % 32] = 1.0
            cells.append((im, a, cq, d["tgt"], d["cls"]))
        if info["has_valid"]:
            gtplane[:, im * 96:(im + 1) * 96] = gtmask[:, im * 96:(im + 1) * 96]
        else:
            gtplane[:, im * 96:(im + 1) * 96] = 1.0

    tgt85 = np.zeros((P, 85), f32)
    onehot = np.zeros((P, 85), f32)
    validng = np.zeros((P, 85), f32)
    gidx = np.zeros((P, NGmax), np.int32)
    for g, (im, a, cq, tgt, clsset) in enumerate(cells):
        tgt85[g, 0:4] = tgt
        onehot[g, 4] = 1.0
        for c in clsset:
            onehot[g, 5 + c] = 1.0
        validng[g, :] = 1.0
        gidx[:, g] = (im * 255 + a * 85) * NCELL + cq
    consts = np.concatenate(
        [g5, xyc, gtplane, gtmask, tgt85, onehot, validng,
         gidx.view(f32)], axis=1).astype(f32)
    return dict(rawsh=rawsh, consts=consts)


# ---------------------------------------------------------------------------
# device program
# ---------------------------------------------------------------------------

def _build_program(S, NGmax, anchors_all, img_size):
    import concourse.bass as bass
    import concourse.mybir as mybir
    from concourse.tile import TileContext
    from concourse.tile_rust import add_dep_helper

    dt = mybir.dt.bfloat16 if DTYPE_BF16 else mybir.dt.float32
    D4 = 4 * DUP
    AF = mybir.ActivationFunctionType
    OP = mybir.AluOpType
    cthre = float(IGNORE_THRE / (1.0 + IGNORE_THRE))
    nb = S // KB
    W = KB * 384  # flat width of one slot batch

    nc = bass.Bass()

    def _reg_const(value):
        key = (mybir.dt.float32, value)
        if key not in nc.const_aps.aps:
            t = nc.alloc_sbuf_tensor(f"const-f32-{value}", [P, 1],
                                     mybir.dt.float32)
            nc.gpsimd.memset(t.ap(), value)
            nc.const_aps.aps[key] = t.ap()

    import math
    lnaw_vals = [float(math.log(anchors_all[a][0] / img_size)) for a in range(nA)]
    for v in lnaw_vals:
        _reg_const(v)
    nc.all_engine_barrier()

    rawsh = nc.declare_dram_parameter("rawsh", [2, 255, NCELL], mybir.dt.float32, False)
    CW = 5 * S * D4 + 192 * 3 + 85 * 3 + NGmax
    off_g5 = 0
    off_xyc = 5 * S * D4
    off_gtp = off_xyc + 192
    off_gtm = off_gtp + 192
    off_tgt = off_gtm + 192
    off_oh = off_tgt + 85
    off_vn = off_oh + 85
    off_gidx = off_vn + 85
    consts = nc.declare_dram_parameter("consts", [P, CW], mybir.dt.float32, False)
    out = nc.declare_dram_parameter("out", [P, 4], mybir.dt.float32, True)

    def A(t, offset, dims):
        h = t.tensor if hasattr(t, "tensor") else t
        return bass.AP(h, offset, dims)

    with TileContext(nc) as tc, \
            tc.tile_pool(name="main", bufs=1) as pool, \
            tc.tile_pool(name="psum", bufs=1, space="PSUM") as ppool:
        RAW = pool.tile([P, 960], mybir.dt.float32, name="RAW")
        E = pool.tile([P, 768], dt, name="E")
        TL = pool.tile([P, 384], dt, name="TL")
        BR = pool.tile([P, 384], dt, name="BR")
        WH = pool.tile([P, 384], dt, name="WH")
        AREA = pool.tile([P, 192], dt, name="AREA")
        ATH = pool.tile([P, 192], dt, name="ATH")
        CONSTS = pool.tile([P, CW], mybir.dt.float32, name="CONSTS")
        CONSTB = pool.tile([P, CW], mybir.dt.float32, name="CONSTB")
        ACC = pool.tile([P, 192], dt, name="ACC")
        MASK = pool.tile([P, 192], mybir.dt.float32, name="MASK")
        MASKF = pool.tile([P, 192], mybir.dt.float32, name="MASKF")
        MEXCL = pool.tile([P, 192], mybir.dt.float32, name="MEXCL")
        SP = pool.tile([P, 192], mybir.dt.float32, name="SP")
        SPA = pool.tile([P, 192], mybir.dt.float32, name="SPA")
        SPB = pool.tile([P, 192], mybir.dt.float32, name="SPB")
        SPM = pool.tile([P, 192], mybir.dt.float32, name="SPM")
        GT85 = pool.tile([P, 85], mybir.dt.float32, name="GT85")
        U = pool.tile([P, 85], mybir.dt.float32, name="U")
        SPC = pool.tile([P, 85], mybir.dt.float32, name="SPC")
        OC = pool.tile([P, 85], mybir.dt.float32, name="OC")
        SPD = pool.tile([P, 85], mybir.dt.float32, name="SPD")
        OUTS = pool.tile([P, 4], mybir.dt.float32, name="OUTS")

        # ---- input loads ----
        for im in range(2):
            for a in range(3):
                nc.sync.dma_start(
                    out=A(RAW, im * 480 + a * 32, [[960, P], [96, 5], [1, 32]]),
                    in_=A(rawsh, (im * 255 + a * 85) * NCELL,
                          [[32, P], [NCELL, 5], [1, 32]]),
                )
        nc.sync.dma_start(out=CONSTS[:], in_=consts[:])

        # Single DVE-local copy of the combined const block: walrus allows
        # only one sync-wait per compute instruction, so downstream ops must
        # not mix DMA-lane waits with engine waits.
        nc.vector.tensor_copy(out=CONSTB[:], in_=CONSTS[:])

        def cview(off, width, rows=P):
            return A(CONSTB, off, [[CW, rows], [1, width]])

        GTPB = cview(off_gtp, 192)
        GTMB = cview(off_gtm, 192)
        TGTB = cview(off_tgt, 85)
        OHB = cview(off_oh, 85)
        VNB = cview(off_vn, 85)
        if DTYPE_BF16:
            G5B = pool.tile([P, 5 * S * D4], dt, name="G5B")
            XYCB = pool.tile([P, 192], dt, name="XYCB")
            nc.vector.tensor_copy(out=G5B[:], in_=cview(off_g5, 5 * S * D4))
            nc.vector.tensor_copy(out=XYCB[:], in_=cview(off_xyc, 192))
            g5_base, g5_pitch = 0, 5 * S * D4
            xyc_base, xyc_pitch = 0, 192
            G5H, XYCH = G5B, XYCB
        else:
            g5_base, g5_pitch = off_g5, CW
            xyc_base, xyc_pitch = off_xyc, CW
            G5H, XYCH = CONSTB, CONSTB

        # ---- pred prep ----
        # E = exp(raw + ln(aw_norm)) per (img, anchor): six ops so each
        # waits on exactly one plane-load DMA (ISA sync-wait slot limit),
        # with the anchor scale folded into the exp bias.
        exp_insts = []
        for im in range(2):
            for a in range(3):
                ei = nc.scalar.activation(
                    out=A(E, im * 384 + a * 32, [[768, P], [96, 4], [1, 32]]),
                    in_=A(RAW, im * 480 + a * 32, [[960, P], [96, 4], [1, 32]]),
                    func=AF.Exp,
                    bias=lnaw_vals[a],
                )
                exp_insts.append(ei.ins)
        xyc_b = A(XYCH, xyc_base, [[xyc_pitch, P], [96, 2], [0, 2], [1, 96]])
        e_lt = A(E, 0, [[768, P], [96, 2], [384, 2], [1, 96]])
        e_rb = A(E, 192, [[768, P], [96, 2], [384, 2], [1, 96]])
        quad = [[384, P], [192, 2], [96, 2], [1, 96]]
        nc.vector.tensor_tensor(out=A(TL, 0, quad), in0=xyc_b, in1=e_lt,
                                op=OP.subtract)
        nc.vector.tensor_tensor(out=A(BR, 0, quad), in0=xyc_b, in1=e_rb,
                                op=OP.add)
        nc.vector.tensor_tensor(out=A(WH, 0, quad), in0=e_lt, in1=e_rb, op=OP.add)
        nc.vector.tensor_tensor(out=AREA[:], in0=WH[:, 0:192], in1=WH[:, 192:384],
                                op=OP.mult)
        nc.vector.tensor_scalar(out=ATH[:], in0=AREA[:], scalar1=cthre,
                                scalar2=None, op0=OP.mult)

        # ---- iou ignore-mask scan ----
        IX = pool.tile([P, W], dt, name="IX")
        IY = pool.tile([P, W], dt, name="IY")
        AX = pool.tile([P, W], dt, name="AX")
        AY = pool.tile([P, W], dt, name="AY")
        IWH = pool.tile([P, 2 * W], dt, name="IWH")
        IWHR = pool.tile([P, 2 * W], dt, name="IWHR")
        INTER = pool.tile([P, W], dt, name="INTER")
        M0 = pool.tile([P, W], dt, name="M0")
        M1 = pool.tile([P, W], dt, name="M1")

        batched = [[W, P], [192, 2 * KB], [96, 2], [1, 96]]

        def gt_ap(comp, s0):
            dims = [[g5_pitch, P], [2 * DUP, 2 * KB], [DUP, 2]]
            if DUP == 1:
                dims.append([0, 96])
            else:
                dims += [[0, 96 // DUP], [1, DUP]]
            return A(G5H, g5_base + comp * S * D4 + s0 * D4, dims)

        def pred_ap(t, off):
            return A(t, off, [[384, P], [0, 2 * KB], [96, 2], [1, 96]])

        ath_b = A(ATH, 0, [[192, P], [0, 2 * KB], [96, 2], [1, 96]])

        for bi in range(nb):
            s0 = bi * KB
            nc.vector.tensor_tensor(out=A(IX, 0, batched), in0=pred_ap(TL, 0),
                                    in1=gt_ap(0, s0), op=OP.max)
            nc.vector.tensor_tensor(out=A(IY, 0, batched), in0=pred_ap(TL, 192),
                                    in1=gt_ap(1, s0), op=OP.max)
            nc.vector.tensor_tensor(out=A(AX, 0, batched), in0=pred_ap(BR, 0),
                                    in1=gt_ap(2, s0), op=OP.min)
            nc.vector.tensor_tensor(out=A(AY, 0, batched), in0=pred_ap(BR, 192),
                                    in1=gt_ap(3, s0), op=OP.min)
            nc.vector.tensor_tensor(out=IWH[:, 0:W], in0=AX[:], in1=IX[:],
                                    op=OP.subtract)
            nc.vector.tensor_tensor(out=IWH[:, W:2 * W], in0=AY[:], in1=IY[:],
                                    op=OP.subtract)
            nc.vector.tensor_scalar(out=IWHR[:], in0=IWH[:], scalar1=0.0,
                                    scalar2=None, op0=OP.max)
            nc.vector.tensor_tensor(out=INTER[:], in0=IWHR[:, 0:W],
                                    in1=IWHR[:, W:2 * W], op=OP.mult)
            nc.vector.tensor_tensor(out=A(M0, 0, batched),
                                    in0=A(INTER, 0, batched), in1=ath_b,
                                    op=OP.subtract)
            nc.vector.tensor_tensor(out=A(M0, 0, batched), in0=A(M0, 0, batched),
                                    in1=gt_ap(4, s0), op=OP.subtract)
            # tree max over the 2*KB streams
            width, src, flip = W, M0, 0
            while width > 192:
                h = width // 2
                dst = (M1, M0)[flip % 2]
                nc.vector.tensor_tensor(out=dst[:, 0:h], in0=src[:, 0:h],
                                        in1=src[:, h:2 * h], op=OP.max)
                src, width, flip = dst, h, flip + 1
            if bi == 0:
                nc.vector.tensor_copy(out=ACC[:], in_=src[:, 0:192])
            else:
                nc.vector.tensor_tensor(out=ACC[:], in0=ACC[:], in1=src[:, 0:192],
                                        op=OP.max)

        # ---- dense conf loss ----
        nc.vector.tensor_scalar(out=MASK[:], in0=ACC[:], scalar1=0.0,
                                scalar2=None, op0=OP.is_le)
        nc.vector.tensor_tensor(out=MASKF[:], in0=MASK[:], in1=GTPB, op=OP.max)
        nc.vector.tensor_tensor(out=MEXCL[:], in0=MASKF[:], in1=GTMB,
                                op=OP.subtract)
        conf_view = A(RAW, 384, [[960, P], [480, 2], [1, 96]])
        sp_flat = [[192, P], [96, 2], [1, 96]]
        abs_i = nc.scalar.activation(out=A(SPA, 0, sp_flat), in_=conf_view,
                                     func=AF.Abs)
        for ei in exp_insts:
            add_dep_helper(abs_i.ins, ei, sync=False,
                           reason="order conf abs after exps (wait-slot limit)")
        nc.scalar.activation(out=SPB[:], in_=SPA[:], func=AF.Exp, scale=-1.0)
        nc.scalar.activation(out=SPA[:], in_=SPB[:], func=AF.Ln, bias=1.0)
        relu_i = nc.scalar.activation(out=A(SPB, 0, sp_flat), in_=conf_view,
                                      func=AF.Relu)
        add_dep_helper(relu_i.ins, abs_i.ins, sync=False,
                       reason="order conf relu after abs")
        nc.vector.tensor_tensor(out=SP[:], in0=SPA[:], in1=SPB[:], op=OP.add)
        nc.vector.tensor_tensor(out=SPM[:], in0=SP[:], in1=MEXCL[:], op=OP.mult)
        nc.vector.memset(OUTS[:], 0.0)
        import concourse.mybir as _mb
        nc.vector.reduce_sum(out=OUTS[:, 0:1], in_=SPM[:], axis=_mb.AxisListType.X)

        # ---- sparse gt-cell terms ----
        nc.gpsimd.indirect_dma_start(
            out=GT85[0:NGmax, 0:85],
            out_offset=None,
            in_=A(rawsh, 0, [[1, (2 * 255 - 85 + 1) * NCELL], [NCELL, 85], [1, 1]]),
            in_offset=bass.IndirectOffsetOnAxis(
                ap=A(CONSTB, off_gidx, [[CW, 1], [1, NGmax]]).bitcast(
                    mybir.dt.int32),
                axis=0),
        )
        # softplus(z) = ln(1 + exp(-|z|)) + relu(z) over cols 4..85
        nc.scalar.activation(out=SPC[0:NGmax, 4:85], in_=GT85[0:NGmax, 4:85], func=AF.Abs)
        nc.scalar.activation(out=SPD[0:NGmax, 4:85], in_=SPC[0:NGmax, 4:85],
                             func=AF.Exp, scale=-1.0)
        nc.scalar.activation(out=SPC[0:NGmax, 4:85], in_=SPD[0:NGmax, 4:85],
                             func=AF.Ln, bias=1.0)
        nc.scalar.activation(out=SPD[0:NGmax, 4:85], in_=GT85[0:NGmax, 4:85],
                             func=AF.Relu)
        nc.vector.tensor_tensor(out=OC[0:NGmax, 4:85], in0=GT85[0:NGmax, 4:85],
                                in1=A(CONSTB, off_oh + 4, [[CW, NGmax], [1, 81]]), op=OP.mult)
        nc.vector.tensor_tensor(out=U[0:NGmax, 4:85], in0=SPC[0:NGmax, 4:85],
                                in1=SPD[0:NGmax, 4:85], op=OP.add)
        nc.vector.tensor_tensor(out=U[0:NGmax, 4:85], in0=U[0:NGmax, 4:85],
                                in1=OC[0:NGmax, 4:85], op=OP.subtract)
        # bbox: (ltrb_raw - tgt)^2 in cols 0..4
        nc.vector.tensor_tensor(out=OC[0:NGmax, 0:4], in0=GT85[0:NGmax, 0:4],
                                in1=A(CONSTB, off_tgt, [[CW, NGmax], [1, 4]]), op=OP.subtract)
        nc.scalar.activation(out=U[0:NGmax, 0:4], in_=OC[0:NGmax, 0:4], func=AF.Square)
        nc.vector.tensor_tensor(out=U[0:NGmax, :], in0=U[0:NGmax, :],
                                in1=A(CONSTB, off_vn, [[CW, NGmax], [1, 85]]), op=OP.mult)
        nc.vector.reduce_sum(out=OUTS[0:NGmax, 2:3], in_=U[0:NGmax, :],
                             axis=_mb.AxisListType.X)

        nc.sync.dma_start(out=out[:], in_=OUTS[:])

    return nc


_CACHE = {}
TRACE = False
LAST_RESULTS = None


def _split_multiwait(nc):
    """Walrus codegen on this toolchain supports only one sync-wait command
    per instruction; split multi-wait instructions (the kernel-tail drain)
    into single-wait NOPs on the same engine."""
    import concourse.mybir as mybir

    if getattr(nc, "_fcos_wait_split", False):
        return
    nc._fcos_wait_split = True
    for bb in nc.m.functions[0].blocks:
        insts = bb.instructions
        for ins in list(insts):
            si = ins.sync_info
            if si is not None and len(si.on_wait) > 1:
                waits = list(si.on_wait)
                idx = insts.index(ins)
                nops = []
                for j, w in enumerate(waits[:-1]):
                    nop = mybir.InstNoOp(name=f"{ins.name}-wsplit{j}", ins=[],
                                         outs=[])
                    nop.engine = ins.engine
                    nop.sync_info = mybir.SyncInfo(on_wait=[w], on_update=[])
                    nops.append(nop)
                ins.sync_info = mybir.SyncInfo(on_wait=[waits[-1]],
                                               on_update=list(si.on_update))
                for nop in reversed(nops):
                    insts.insert(idx, nop)


def _plan(labels, anchors_all, img_size):
    per_img = _host_precompute(labels, anchors_all, img_size)
    Smax = max(max((info["K"] + 1) // 2 for info in per_img), 1)
    S = ((Smax + KB - 1) // KB) * KB
    NGmax = max(
        max(len(per_img[2 * c]["cellmap"]) + len(per_img[2 * c + 1]["cellmap"])
            for c in range(N_CORES)), 1)
    NGmax = min(((NGmax + 7) // 8) * 8, P)
    return per_img, S, NGmax


def kernel(raw, labels, anchors_all, img_size):
    from concourse.bass_utils import run_bass_kernel_spmd

    raw = np.asarray(raw, f32)
    labels_np = np.asarray(labels, f32)
    anchors_np = np.asarray(anchors_all, f32)
    isize = int(img_size)

    per_img, S, NGmax = _plan(labels_np, anchors_np, isize)
    key = (S, NGmax, KB, DUP, DTYPE_BF16, anchors_np.tobytes(), isize)
    if key not in _CACHE:
        _CACHE[key] = _build_program(S, NGmax, anchors_np.tolist(), isize)
    nc = _CACHE[key]
    _split_multiwait(nc)

    in_maps = [
        _pack_core_inputs(c, per_img, raw, anchors_np, isize, S, NGmax)
        for c in range(N_CORES)
    ]
    global LAST_RESULTS
    res = run_bass_kernel_spmd(nc, in_maps, list(range(N_CORES)), trace=TRACE)
    LAST_RESULTS = res
    total = np.float64(0.0)
    for c in range(N_CORES):
        o = res.results[c]["out"]
        total += np.sum(o[:, 0], dtype=np.float64)
        total += np.sum(o[:, 2], dtype=np.float64)
    return f32(total)


if __name__ == "__main__":
    import importlib.util

    spec = importlib.util.spec_from_file_location("reference",
                                                  "/root/problem/reference.py")
    ref = importlib.util.module_from_spec(spec)
    spec.loader.exec_module(ref)
    inputs = ref.setup_inputs()
    np_inputs = {k: np.asarray(v) for k, v in inputs.items()}
    got = kernel(**np_inputs)
    print("kernel:", got)



# revision 2
# speedup vs baseline: 1.6089x; 1.6089x over previous
"""Trainium2 Bass kernel for the YOLO/FCOS-layer loss (nn_FCOSLayer_22840636080477).

Sharding: data-parallel over batch, 2 images per NeuronCore x 8 cores, one
SPMD program. Host does label-side preprocessing (anchor matching, scatter
dedup, constant packing); device does everything that touches `raw`:

  loss = sum_cells softplus(conf) * (conf_mask & ~gt)          (dense, DVE+ACT)
       + sum_gtcells [ softplus(conf)-conf                      (sparse, gather)
                      + sum_c (softplus(cls_c) - onehot_c*cls_c)
                      + sum_4 (ltrb_raw - tgt)^2 ]

The ignore mask (`conf_mask`) needs a max-IoU scan of all 12288 pred boxes
against each image's gt boxes - that is the compute-heavy part, done with
broadcast-AP batched vector ops.
"""
import sys
import numpy as np

sys.path.insert(0, "/opt/trn_rl_repo")

N_CLS = 80
nA = 3
STRIDE = 8
IGNORE_THRE = 0.6
EPS = 1e-16
B = 16
K = 50
nG = 64
N_CORES = 8
P = 128
NCELL = nG * nG
f32 = np.float32

# tunables
KB = 4              # gt slots per batched op group
DTYPE_BF16 = True   # compute iou scan in bf16
DUP = 2             # duplicate gt scalars pairwise (bf16 2x packing aid)


# ---------------------------------------------------------------------------
# host-side label math (replicates reference.py semantics in f32 numpy)
# ---------------------------------------------------------------------------

def _host_precompute(labels, anchors_all, img_size):
    labels = np.asarray(labels, f32)
    anchors_all = np.asarray(anchors_all, f32)
    img_size = f32(img_size)
    anchors = anchors_all[:nA]
    norm_anch = anchors_all / img_size
    anch_w_n = anchors[:, 0] / img_size

    per_img = []
    for bb in range(B):
        lab = labels[bb]
        valid_row = lab.sum(-1) > 0
        tw, th = lab[:, 3], lab[:, 4]
        inter = np.minimum(tw[:, None], norm_anch[:, 0]) * np.minimum(
            th[:, None], norm_anch[:, 1]
        )
        union = tw[:, None] * th[:, None] + norm_anch[:, 0] * norm_anch[:, 1] - inter
        an_iou = inter / (union + f32(EPS))
        best_n_all = np.argmax(an_iou, axis=-1)
        best_n = best_n_all % nA
        valid = valid_row & (best_n_all < nA)

        ks = np.where(valid_row)[0]
        gcx, gcy, gw, gh = lab[ks, 1], lab[ks, 2], lab[ks, 3], lab[ks, 4]
        gt = dict(
            tlx=(gcx - gw / 2).astype(f32),
            tly=(gcy - gh / 2).astype(f32),
            brx=(gcx + gw / 2).astype(f32),
            bry=(gcy + gh / 2).astype(f32),
            area=(gw * gh).astype(f32),
        )

        tx = lab[:, 1] * nG
        ty = lab[:, 2] * nG
        ti = tx.astype(np.int32)
        tj = ty.astype(np.int32)
        tcls = lab[:, 0].astype(np.int32)
        lw, lh = lab[:, 3] * nG, lab[:, 4] * nG
        xc = np.floor(tx) + f32(0.5)
        yc = np.floor(ty) + f32(0.5)
        lab_ltrb = (
            np.maximum(
                np.stack(
                    [xc - (tx - lw / 2), yc - (ty - lh / 2),
                     (tx + lw / 2) - xc, (ty + lh / 2) - yc], -1),
                0.0,
            ) / f32(nG)
        ).astype(f32)
        cellmap = {}
        for k in range(K):
            if not valid[k]:
                continue
            key = (int(best_n[k]), int(tj[k]), int(ti[k]))
            tgt = np.log(lab_ltrb[k] / anch_w_n[best_n[k]] + f32(EPS)).astype(f32)
            if key not in cellmap:
                cellmap[key] = dict(tgt=tgt, cls=set([int(tcls[k])]))
            else:
                cellmap[key]["tgt"] = tgt  # scatter last-wins
                cellmap[key]["cls"].add(int(tcls[k]))
        per_img.append(dict(K=len(ks), gt=gt, cellmap=cellmap,
                            has_valid=bool(valid.any())))
    return per_img


def _pack_core_inputs(core, per_img, raw, anchors_all, img_size, S, NGmax):
    img_size = f32(img_size)
    thre = f32(IGNORE_THRE)
    cthre = (thre / (f32(1.0) + thre)).astype(f32)
    b0 = core * 2
    D4 = 4 * DUP

    rawsh = np.ascontiguousarray(raw[b0:b0 + 2]).reshape(2, 255, NCELL)

    # G5 [128, 5*S*D4]: comp-major {tlx,tly,brx,bry,athc}; within comp:
    # col = slot*4*DUP + strm*2*DUP + img*DUP + dup
    g5 = np.zeros((5, S, 2, 2, DUP), f32)
    g5[4] = cthre * f32(EPS)
    for im in range(2):
        info = per_img[b0 + im]
        gt = info["gt"]
        for k in range(info["K"]):
            strm, slot = k % 2, k // 2
            g5[0, slot, strm, im, :] = gt["tlx"][k]
            g5[1, slot, strm, im, :] = gt["tly"][k]
            g5[2, slot, strm, im, :] = gt["brx"][k]
            g5[3, slot, strm, im, :] = gt["bry"][k]
            g5[4, slot, strm, im, :] = cthre * (gt["area"][k] + f32(EPS))
    g5 = np.ascontiguousarray(
        np.broadcast_to(g5.reshape(1, 5 * S * D4), (P, 5 * S * D4))
    )

    # XYC [128, 192]: col = comp*96 + aq, cell q = 32p + (aq % 32)
    pidx = np.arange(P)[:, None]
    aqidx = np.arange(96)[None, :]
    q = 32 * pidx + (aqidx % 32)
    gx = (q % nG).astype(f32)
    gy = (q // nG).astype(f32)
    xyc = np.concatenate([(gx + f32(0.5)) / f32(nG), (gy + f32(0.5)) / f32(nG)],
                         axis=1).astype(f32)

    gtmask = np.zeros((P, 192), f32)
    gtplane = np.zeros((P, 192), f32)
    cells = []
    for im in range(2):
        info = per_img[b0 + im]
        for (a, j, i), d in info["cellmap"].items():
            cq = j * nG + i
            gtmask[cq // 32, im * 96 + a * 32 + cq % 32] = 1.0
            cells.append((im, a, cq, d["tgt"], d["cls"]))
        if info["has_valid"]:
            gtplane[:, im * 96:(im + 1) * 96] = gtmask[:, im * 96:(im + 1) * 96]
        else:
            gtplane[:, im * 96:(im + 1) * 96] = 1.0

    tgt85 = np.zeros((P, 85), f32)
    onehot = np.zeros((P, 85), f32)
    validng = np.zeros((P, 85), f32)
    gidx = np.zeros((P, NGmax), np.int32)
    for g, (im, a, cq, tgt, clsset) in enumerate(cells):
        tgt85[g, 0:4] = tgt
        onehot[g, 4] = 1.0
        for c in clsset:
            onehot[g, 5 + c] = 1.0
        validng[g, :] = 1.0
        gidx[:, g] = (im * 255 + a * 85) * NCELL + cq
    consts = np.concatenate(
        [g5, xyc, gtplane, gtmask, tgt85, onehot, validng,
         gidx.view(f32)], axis=1).astype(f32)
    return dict(rawsh=rawsh, consts=consts)


# ---------------------------------------------------------------------------
# device program
# ---------------------------------------------------------------------------

def _build_program(S, NGmax, anchors_all, img_size):
    import concourse.bass as bass
    import concourse.mybir as mybir
    from concourse.tile import TileContext
    from concourse.tile_rust import add_dep_helper

    dt = mybir.dt.bfloat16 if DTYPE_BF16 else mybir.dt.float32
    D4 = 4 * DUP
    AF = mybir.ActivationFunctionType
    OP = mybir.AluOpType
    cthre = float(IGNORE_THRE / (1.0 + IGNORE_THRE))
    nb = S // KB
    W = KB * 384  # flat width of one slot batch

    nc = bass.Bass()

    def _reg_const(value):
        key = (mybir.dt.float32, value)
        if key not in nc.const_aps.aps:
            t = nc.alloc_sbuf_tensor(f"const-f32-{value}", [P, 1],
                                     mybir.dt.float32)
            nc.gpsimd.memset(t.ap(), value)
            nc.const_aps.aps[key] = t.ap()

    import math
    lnaw_vals = [float(math.log(anchors_all[a][0] / img_size)) for a in range(nA)]
    for v in lnaw_vals:
        _reg_const(v)
    nc.all_engine_barrier()

    rawsh = nc.declare_dram_parameter("rawsh", [2, 255, NCELL], mybir.dt.float32, False)
    CW = 5 * S * D4 + 192 * 3 + 85 * 3 + NGmax
    off_g5 = 0
    off_xyc = 5 * S * D4
    off_gtp = off_xyc + 192
    off_gtm = off_gtp + 192
    off_tgt = off_gtm + 192
    off_oh = off_tgt + 85
    off_vn = off_oh + 85
    off_gidx = off_vn + 85
    consts = nc.declare_dram_parameter("consts", [P, CW], mybir.dt.float32, False)
    out = nc.declare_dram_parameter("out", [P, 4], mybir.dt.float32, True)

    def A(t, offset, dims):
        h = t.tensor if hasattr(t, "tensor") else t
        return bass.AP(h, offset, dims)

    with TileContext(nc) as tc, \
            tc.tile_pool(name="main", bufs=1) as pool, \
            tc.tile_pool(name="psum", bufs=1, space="PSUM") as ppool:
        RAW = pool.tile([P, 960], mybir.dt.float32, name="RAW")
        E = pool.tile([P, 768], dt, name="E")
        TL = pool.tile([P, 384], dt, name="TL")
        BR = pool.tile([P, 384], dt, name="BR")
        WH = pool.tile([P, 384], dt, name="WH")
        AREA = pool.tile([P, 192], dt, name="AREA")
        ATH = pool.tile([P, 192], dt, name="ATH")
        CONSTS = pool.tile([P, CW], mybir.dt.float32, name="CONSTS")
        CONSTB = pool.tile([P, CW], mybir.dt.float32, name="CONSTB")
        ACC = pool.tile([P, 192], dt, name="ACC")
        MASK = pool.tile([P, 192], mybir.dt.float32, name="MASK")
        MASKF = pool.tile([P, 192], mybir.dt.float32, name="MASKF")
        MEXCL = pool.tile([P, 192], mybir.dt.float32, name="MEXCL")
        SP = pool.tile([P, 192], mybir.dt.float32, name="SP")
        SPA = pool.tile([P, 192], mybir.dt.float32, name="SPA")
        SPB = pool.tile([P, 192], mybir.dt.float32, name="SPB")
        SPM = pool.tile([P, 192], mybir.dt.float32, name="SPM")
        GT85 = pool.tile([P, 85], mybir.dt.float32, name="GT85")
        U = pool.tile([P, 85], mybir.dt.float32, name="U")
        SPC = pool.tile([P, 85], mybir.dt.float32, name="SPC")
        OC = pool.tile([P, 85], mybir.dt.float32, name="OC")
        SPD = pool.tile([P, 85], mybir.dt.float32, name="SPD")
        OUTS = pool.tile([P, 4], mybir.dt.float32, name="OUTS")

        # ---- input loads ----
        for im in range(2):
            for a in range(3):
                nc.sync.dma_start(
                    out=A(RAW, im * 480 + a * 32, [[960, P], [96, 5], [1, 32]]),
                    in_=A(rawsh, (im * 255 + a * 85) * NCELL,
                          [[32, P], [NCELL, 5], [1, 32]]),
                )
        nc.sync.dma_start(out=CONSTS[:], in_=consts[:])

        # Single DVE-local copy of the combined const block: walrus allows
        # only one sync-wait per compute instruction, so downstream ops must
        # not mix DMA-lane waits with engine waits.
        nc.vector.tensor_copy(out=CONSTB[:], in_=CONSTS[:])

        def cview(off, width, rows=P):
            return A(CONSTB, off, [[CW, rows], [1, width]])

        GTPB = cview(off_gtp, 192)
        GTMB = cview(off_gtm, 192)
        TGTB = cview(off_tgt, 85)
        OHB = cview(off_oh, 85)
        VNB = cview(off_vn, 85)
        if DTYPE_BF16:
            G5B = pool.tile([P, 5 * S * D4], dt, name="G5B")
            XYCB = pool.tile([P, 192], dt, name="XYCB")
            nc.vector.tensor_copy(out=G5B[:], in_=cview(off_g5, 5 * S * D4))
            nc.vector.tensor_copy(out=XYCB[:], in_=cview(off_xyc, 192))
            g5_base, g5_pitch = 0, 5 * S * D4
            xyc_base, xyc_pitch = 0, 192
            G5H, XYCH = G5B, XYCB
        else:
            g5_base, g5_pitch = off_g5, CW
            xyc_base, xyc_pitch = off_xyc, CW
            G5H, XYCH = CONSTB, CONSTB

        # ---- pred prep ----
        # E = exp(raw + ln(aw_norm)) per (img, anchor): six ops so each
        # waits on exactly one plane-load DMA (ISA sync-wait slot limit),
        # with the anchor scale folded into the exp bias.
        exp_insts = []
        for im in range(2):
            for a in range(3):
                ei = nc.scalar.activation(
                    out=A(E, im * 384 + a * 32, [[768, P], [96, 4], [1, 32]]),
                    in_=A(RAW, im * 480 + a * 32, [[960, P], [96, 4], [1, 32]]),
                    func=AF.Exp,
                    bias=lnaw_vals[a],
                )
                exp_insts.append(ei.ins)
        xyc_b = A(XYCH, xyc_base, [[xyc_pitch, P], [96, 2], [0, 2], [1, 96]])
        e_lt = A(E, 0, [[768, P], [96, 2], [384, 2], [1, 96]])
        e_rb = A(E, 192, [[768, P], [96, 2], [384, 2], [1, 96]])
        quad = [[384, P], [192, 2], [96, 2], [1, 96]]
        nc.vector.tensor_tensor(out=A(TL, 0, quad), in0=xyc_b, in1=e_lt,
                                op=OP.subtract)
        nc.vector.tensor_tensor(out=A(BR, 0, quad), in0=xyc_b, in1=e_rb,
                                op=OP.add)
        nc.vector.tensor_tensor(out=A(WH, 0, quad), in0=e_lt, in1=e_rb, op=OP.add)
        nc.vector.tensor_tensor(out=AREA[:], in0=WH[:, 0:192], in1=WH[:, 192:384],
                                op=OP.mult)
        nc.vector.tensor_scalar(out=ATH[:], in0=AREA[:], scalar1=cthre,
                                scalar2=None, op0=OP.mult)

        # ---- iou ignore-mask scan ----
        IX = pool.tile([P, W], dt, name="IX")
        IY = pool.tile([P, W], dt, name="IY")
        AX = pool.tile([P, W], dt, name="AX")
        AY = pool.tile([P, W], dt, name="AY")
        IWH = pool.tile([P, 2 * W], dt, name="IWH")
        IWHR = pool.tile([P, 2 * W], dt, name="IWHR")
        INTER = pool.tile([P, W], dt, name="INTER")
        M0 = pool.tile([P, W], dt, name="M0")
        M1 = pool.tile([P, W], dt, name="M1")

        batched = [[W, P], [192, 2 * KB], [96, 2], [1, 96]]

        def gt_ap(comp, s0):
            dims = [[g5_pitch, P], [2 * DUP, 2 * KB], [DUP, 2]]
            if DUP == 1:
                dims.append([0, 96])
            else:
                dims += [[0, 96 // DUP], [1, DUP]]
            return A(G5H, g5_base + comp * S * D4 + s0 * D4, dims)

        def pred_ap(t, off):
            return A(t, off, [[384, P], [0, 2 * KB], [96, 2], [1, 96]])

        ath_b = A(ATH, 0, [[192, P], [0, 2 * KB], [96, 2], [1, 96]])

        for bi in range(nb):
            s0 = bi * KB
            nc.vector.tensor_tensor(out=A(IX, 0, batched), in0=pred_ap(TL, 0),
                                    in1=gt_ap(0, s0), op=OP.max)
            nc.vector.tensor_tensor(out=A(IY, 0, batched), in0=pred_ap(TL, 192),
                                    in1=gt_ap(1, s0), op=OP.max)
            nc.vector.tensor_tensor(out=A(AX, 0, batched), in0=pred_ap(BR, 0),
                                    in1=gt_ap(2, s0), op=OP.min)
            nc.vector.tensor_tensor(out=A(AY, 0, batched), in0=pred_ap(BR, 192),
                                    in1=gt_ap(3, s0), op=OP.min)
            nc.vector.tensor_tensor(out=IWH[:, 0:W], in0=AX[:], in1=IX[:],
                                    op=OP.subtract)
            nc.vector.tensor_tensor(out=IWH[:, W:2 * W], in0=AY[:], in1=IY[:],
                                    op=OP.subtract)
            nc.vector.tensor_scalar(out=IWHR[:], in0=IWH[:], scalar1=0.0,
                                    scalar2=None, op0=OP.max)
            nc.vector.tensor_tensor(out=INTER[:], in0=IWHR[:, 0:W],
                                    in1=IWHR[:, W:2 * W], op=OP.mult)
            nc.vector.tensor_tensor(out=A(M0, 0, batched),
                                    in0=A(INTER, 0, batched), in1=ath_b,
                                    op=OP.subtract)
            nc.vector.tensor_tensor(out=A(M0, 0, batched), in0=A(M0, 0, batched),
                                    in1=gt_ap(4, s0), op=OP.subtract)
            # tree max over the 2*KB streams
            width, src, flip = W, M0, 0
            while width > 192:
                h = width // 2
                dst = (M1, M0)[flip % 2]
                nc.vector.tensor_tensor(out=dst[:, 0:h], in0=src[:, 0:h],
                                        in1=src[:, h:2 * h], op=OP.max)
                src, width, flip = dst, h, flip + 1
            if bi == 0:
                nc.vector.tensor_copy(out=ACC[:], in_=src[:, 0:192])
            else:
                nc.vector.tensor_tensor(out=ACC[:], in0=ACC[:], in1=src[:, 0:192],
                                        op=OP.max)

        # ---- dense conf loss ----
        nc.vector.tensor_scalar(out=MASK[:], in0=ACC[:], scalar1=0.0,
                                scalar2=None, op0=OP.is_le)
        nc.vector.tensor_tensor(out=MASKF[:], in0=MASK[:], in1=GTPB, op=OP.max)
        nc.vector.tensor_tensor(out=MEXCL[:], in0=MASKF[:], in1=GTMB,
                                op=OP.subtract)
        conf_view = A(RAW, 384, [[960, P], [480, 2], [1, 96]])
        sp_flat = [[192, P], [96, 2], [1, 96]]
        abs_i = nc.scalar.activation(out=A(SPA, 0, sp_flat), in_=conf_view,
                                     func=AF.Abs)
        for ei in exp_insts:
            add_dep_helper(abs_i.ins, ei, sync=False,
                           reason="order conf abs after exps (wait-slot limit)")
        nc.scalar.activation(out=SPB[:], in_=SPA[:], func=AF.Exp, scale=-1.0)
        nc.scalar.activation(out=SPA[:], in_=SPB[:], func=AF.Ln, bias=1.0)
        relu_i = nc.scalar.activation(out=A(SPB, 0, sp_flat), in_=conf_view,
                                      func=AF.Relu)
        add_dep_helper(relu_i.ins, abs_i.ins, sync=False,
                       reason="order conf relu after abs")
        nc.vector.tensor_tensor(out=SP[:], in0=SPA[:], in1=SPB[:], op=OP.add)
        nc.vector.tensor_tensor(out=SPM[:], in0=SP[:], in1=MEXCL[:], op=OP.mult)
        nc.vector.memset(OUTS[:], 0.0)
        import concourse.mybir as _mb
        nc.vector.reduce_sum(out=OUTS[:, 0:1], in_=SPM[:], axis=_mb.AxisListType.X)

        # ---- sparse gt-cell terms ----
        nc.gpsimd.indirect_dma_start(
            out=GT85[0:NGmax, 0:85],
            out_offset=None,
            in_=A(rawsh, 0, [[1, (2 * 255 - 85 + 1) * NCELL], [NCELL, 85], [1, 1]]),
            in_offset=bass.IndirectOffsetOnAxis(
                ap=A(CONSTB, off_gidx, [[CW, 1], [1, NGmax]]).bitcast(
                    mybir.dt.int32),
                axis=0),
        )
        # softplus(z) = ln(1 + exp(-|z|)) + relu(z) over cols 4..85
        nc.scalar.activation(out=SPC[0:NGmax, 4:85], in_=GT85[0:NGmax, 4:85], func=AF.Abs)
        nc.scalar.activation(out=SPD[0:NGmax, 4:85], in_=SPC[0:NGmax, 4:85],
                             func=AF.Exp, scale=-1.0)
        nc.scalar.activation(out=SPC[0:NGmax, 4:85], in_=SPD[0:NGmax, 4:85],
                             func=AF.Ln, bias=1.0)
        nc.scalar.activation(out=SPD[0:NGmax, 4:85], in_=GT85[0:NGmax, 4:85],
                             func=AF.Relu)
        nc.vector.tensor_tensor(out=OC[0:NGmax, 4:85], in0=GT85[0:NGmax, 4:85],
                                in1=A(CONSTB, off_oh + 4, [[CW, NGmax], [1, 81]]), op=OP.mult)
        nc.vector.tensor_tensor(out=U[0:NGmax, 4:85], in0=SPC[0:NGmax, 4:85],
                                in1=SPD[0:NGmax, 4:85], op=OP.add)
        nc.vector.tensor_tensor(out=U[0:NGmax, 4:85], in0=U[0:NGmax, 4:85],
                                in1=OC[0:NGmax, 4:85], op=OP.subtract)
        # bbox: (ltrb_raw - tgt)^2 in cols 0..4
        nc.vector.tensor_tensor(out=OC[0:NGmax, 0:4], in0=GT85[0:NGmax, 0:4],
                                in1=A(CONSTB, off_tgt, [[CW, NGmax], [1, 4]]), op=OP.subtract)
        nc.scalar.activation(out=U[0:NGmax, 0:4], in_=OC[0:NGmax, 0:4], func=AF.Square)
        nc.vector.tensor_tensor(out=U[0:NGmax, :], in0=U[0:NGmax, :],
                                in1=A(CONSTB, off_vn, [[CW, NGmax], [1, 85]]), op=OP.mult)
        nc.vector.reduce_sum(out=OUTS[0:NGmax, 2:3], in_=U[0:NGmax, :],
                             axis=_mb.AxisListType.X)

        nc.sync.dma_start(out=out[:], in_=OUTS[:])

    return nc


_CACHE = {}
TRACE = False
LAST_RESULTS = None


def _split_multiwait(nc):
    """Walrus codegen on this toolchain supports only one sync-wait command
    per instruction; split multi-wait instructions (the kernel-tail drain)
    into single-wait NOPs on the same engine."""
    import concourse.mybir as mybir

    if getattr(nc, "_fcos_wait_split", False):
        return
    nc._fcos_wait_split = True
    for bb in nc.m.functions[0].blocks:
        insts = bb.instructions
        for ins in list(insts):
            si = ins.sync_info
            if si is not None and len(si.on_wait) > 1:
                waits = list(si.on_wait)
                idx = insts.index(ins)
                nops = []
                for j, w in enumerate(waits[:-1]):
                    nop = mybir.InstNoOp(name=f"{ins.name}-wsplit{j}", ins=[],
                                         outs=[])
                    nop.engine = ins.engine
                    nop.sync_info = mybir.SyncInfo(on_wait=[w], on_update=[])
                    nops.append(nop)
                ins.sync_info = mybir.SyncInfo(on_wait=[waits[-1]],
                                               on_update=list(si.on_update))
                for nop in reversed(nops):
                    insts.insert(idx, nop)


def _plan(labels, anchors_all, img_size):
    per_img = _host_precompute(labels, anchors_all, img_size)
    Smax = max(max((info["K"] + 1) // 2 for info in per_img), 1)
    S = ((Smax + KB - 1) // KB) * KB
    NGmax = max(
        max(len(per_img[2 * c]["cellmap"]) + len(per_img[2 * c + 1]["cellmap"])
            for c in range(N_CORES)), 1)
    NGmax = min(((NGmax + 7) // 8) * 8, P)
    return per_img, S, NGmax


def kernel(raw, labels, anchors_all, img_size):
    from concourse.bass_utils import run_bass_kernel_spmd

    raw = np.asarray(raw, f32)
    labels_np = np.asarray(labels, f32)
    anchors_np = np.asarray(anchors_all, f32)
    isize = int(img_size)

    per_img, S, NGmax = _plan(labels_np, anchors_np, isize)
    key = (S, NGmax, KB, DUP, DTYPE_BF16, anchors_np.tobytes(), isize)
    if key not in _CACHE:
        _CACHE[key] = _build_program(S, NGmax, anchors_np.tolist(), isize)
    nc = _CACHE[key]
    _split_multiwait(nc)

    in_maps = [
        _pack_core_inputs(c, per_img, raw, anchors_np, isize, S, NGmax)
        for c in range(N_CORES)
    ]
    global LAST_RESULTS
    res = run_bass_kernel_spmd(nc, in_maps, list(range(N_CORES)), trace=TRACE)
    LAST_RESULTS = res
    total = np.float64(0.0)
    for c in range(N_CORES):
        o = res.results[c]["out"]
        total += np.sum(o[:, 0], dtype=np.float64)
        total += np.sum(o[:, 2], dtype=np.float64)
    return f32(total)


if __name__ == "__main__":
    import importlib.util

    spec = importlib.util.spec_from_file_location("reference",
                                                  "/root/problem/reference.py")
    ref = importlib.util.module_from_spec(spec)
    spec.loader.exec_module(ref)
    inputs = ref.setup_inputs()
    np_inputs = {k: np.asarray(v) for k, v in inputs.items()}
    got = kernel(**np_inputs)
    print("kernel:", got)



# revision 3
# speedup vs baseline: 2.3600x; 1.4669x over previous
"""Trainium2 Bass kernel for the YOLO/FCOS-layer loss (nn_FCOSLayer_22840636080477).

Sharding: data-parallel over batch, 2 images per NeuronCore x 8 cores, one
SPMD program. Host does label-side preprocessing (anchor matching, scatter
dedup, row-band gt->partition scheduling, constant packing); device does
everything that touches `raw`:

  loss = sum_cells softplus(conf) * (conf_mask & ~gt)          (dense)
       + sum_gtcells [ softplus(conf)-conf                      (sparse, gather)
                      + sum_c (softplus(cls_c) - onehot_c*cls_c)
                      + sum_4 (ltrb_raw - tgt)^2 ]

The ignore mask needs a max-IoU scan of 12288 pred boxes against each
image's gt boxes.  Three structural tricks make it cheap:

1. bf16 + packed last-axis APs -> DVE 2x_1p mode (0.52 ns/elem).
2. Row banding: IoU > 0.6 forces the pred's cell center inside the gt
   box dilated by (2/3)*(wg,hg) (pred-independent bound), so each gt
   only concerns a contiguous band of partitions (partition p holds
   row p//2).  Interval-coloring packs gts into "rounds"; each round
   instruction gives every partition its own gt via per-partition
   consts.  Rounds ~ max-load instead of K.
3. Margin m = min(iw*ih - athg, iw) has the exact sign of the ignore
   condition (valid when gt heights <= 1), killing both clamps; the
   per-pred threshold athp = cthre*areap is compared once at the end:
   ignore <=> max_g m_g > athp.
"""
import sys
import math
import numpy as np

sys.path.insert(0, "/opt/trn_rl_repo")

import ml_dtypes

bf16 = ml_dtypes.bfloat16

N_CLS = 80
nA = 3
STRIDE = 8
IGNORE_THRE = 0.6
EPS = 1e-16
B = 16
K = 50
nG = 64
N_CORES = 8
P = 128
NCELL = nG * nG
f32 = np.float32

# tunables
G_R = 8          # rounds per batched scan op (power of 2)
DUP = 2          # duplicate gt scalars pairwise (bf16 2x packing aid)
DIL = 0.70       # band dilation factor (theory: 2/3; slack for bf16)
ROWPAD = 1       # extra band rows for fp safety
GPSIMD_TREE = False  # offload min/tree/acc to GpSimd


# ---------------------------------------------------------------------------
# host-side label math (replicates reference.py semantics in f32 numpy)
# ---------------------------------------------------------------------------

def _host_precompute(labels, anchors_all, img_size):
    labels = np.asarray(labels, f32)
    anchors_all = np.asarray(anchors_all, f32)
    img_size = f32(img_size)
    anchors = anchors_all[:nA]
    norm_anch = anchors_all / img_size
    anch_w_n = anchors[:, 0] / img_size

    per_img = []
    for bb in range(B):
        lab = labels[bb]
        valid_row = lab.sum(-1) > 0
        tw, th = lab[:, 3], lab[:, 4]
        inter = np.minimum(tw[:, None], norm_anch[:, 0]) * np.minimum(
            th[:, None], norm_anch[:, 1]
        )
        union = tw[:, None] * th[:, None] + norm_anch[:, 0] * norm_anch[:, 1] - inter
        an_iou = inter / (union + f32(EPS))
        best_n_all = np.argmax(an_iou, axis=-1)
        best_n = best_n_all % nA
        valid = valid_row & (best_n_all < nA)

        ks = np.where(valid_row)[0]
        gcx, gcy, gw, gh = lab[ks, 1], lab[ks, 2], lab[ks, 3], lab[ks, 4]
        gt = dict(
            tlx=(gcx - gw / 2).astype(f32),
            tly=(gcy - gh / 2).astype(f32),
            brx=(gcx + gw / 2).astype(f32),
            bry=(gcy + gh / 2).astype(f32),
            area=(gw * gh).astype(f32),
            gh=gh.astype(f32),
        )

        tx = lab[:, 1] * nG
        ty = lab[:, 2] * nG
        ti = tx.astype(np.int32)
        tj = ty.astype(np.int32)
        tcls = lab[:, 0].astype(np.int32)
        lw, lh = lab[:, 3] * nG, lab[:, 4] * nG
        xc = np.floor(tx) + f32(0.5)
        yc = np.floor(ty) + f32(0.5)
        lab_ltrb = (
            np.maximum(
                np.stack(
                    [xc - (tx - lw / 2), yc - (ty - lh / 2),
                     (tx + lw / 2) - xc, (ty + lh / 2) - yc], -1),
                0.0,
            ) / f32(nG)
        ).astype(f32)
        cellmap = {}
        for k in range(K):
            if not valid[k]:
                continue
            key = (int(best_n[k]), int(tj[k]), int(ti[k]))
            tgt = np.log(lab_ltrb[k] / anch_w_n[best_n[k]] + f32(EPS)).astype(f32)
            if key not in cellmap:
                cellmap[key] = dict(tgt=tgt, cls=set([int(tcls[k])]))
            else:
                cellmap[key]["tgt"] = tgt  # scatter last-wins
                cellmap[key]["cls"].add(int(tcls[k]))

        # row-band partition intervals for the iou scan
        gt_n = len(ks)
        ylo = gt["tly"].astype(np.float64) - DIL * gt["gh"].astype(np.float64)
        yhi = gt["bry"].astype(np.float64) + DIL * gt["gh"].astype(np.float64)
        j0 = np.clip(np.floor(ylo * nG - 0.5).astype(int) - ROWPAD, 0, nG - 1)
        j1 = np.clip(np.ceil(yhi * nG - 0.5).astype(int) + ROWPAD, 0, nG - 1)
        iv = list(zip(2 * j0, 2 * j1 + 2)) if gt_n else []
        # interval partitioning into rounds (optimal: R = max point load)
        order = sorted(range(gt_n), key=lambda k2: iv[k2][0])
        rounds = []  # [last_end, [gt indices]]
        for k2 in order:
            p0, p1 = iv[k2]
            for r in rounds:
                if r[0] <= p0:
                    r[0] = p1
                    r[1].append(k2)
                    break
            else:
                rounds.append([p1, [k2]])
        per_img.append(dict(K=gt_n, gt=gt, cellmap=cellmap,
                            has_valid=bool(valid.any()),
                            iv=iv, rounds=[r[1] for r in rounds]))
    return per_img


def _plan(labels, anchors_all, img_size):
    per_img = _host_precompute(labels, anchors_all, img_size)
    Rs = [len(info["rounds"]) for info in per_img]
    order = sorted(range(B), key=lambda i: -Rs[i])
    A_imgs = order[:N_CORES]
    B_imgs = order[N_CORES:][::-1]  # pair big-A with small-B
    RA = max(max((Rs[i] for i in A_imgs), default=0), 1)
    RB = max(max((Rs[i] for i in B_imgs), default=0), 1)
    NA = -(-RA // G_R)
    NB = -(-RB // G_R)
    NGmax = 1
    for c in range(N_CORES):
        n = (len(per_img[A_imgs[c]]["cellmap"])
             + len(per_img[B_imgs[c]]["cellmap"]))
        NGmax = max(NGmax, n)
    NGmax = min(-(-NGmax // 8) * 8, P)
    return per_img, A_imgs, B_imgs, NA, NB, NGmax


# ---------------------------------------------------------------------------
# per-core input packing
# ---------------------------------------------------------------------------

def _pack_core_inputs(core, per_img, A_imgs, B_imgs, raw, img_size,
                      NA, NB, NGmax):
    img_size = f32(img_size)
    thre = f32(IGNORE_THRE)
    cthre = (thre / (f32(1.0) + thre)).astype(f32)
    imgs = [A_imgs[core], B_imgs[core]]
    SA, SB = NA * G_R, NB * G_R

    # full-channel raw (gather source) + pre-packed 5-channel bf16 block
    rawsh = np.ascontiguousarray(raw[imgs]).reshape(2, 255, NCELL)
    # raw5 [P, 960]: col = im*480 + ch*96 + a*32 + c ; cell q = 32p + c
    r6 = rawsh.reshape(2, nA, 85, P, 32)[:, :, 0:5]       # im,a,ch,p,c
    raw5 = np.ascontiguousarray(
        r6.transpose(3, 0, 2, 1, 4).reshape(P, 960)).astype(bf16)

    # scan consts (bf16): per-image g5 blocks + xyc
    # g5 block for image im: col = comp*(S*DUP) + slot*DUP + dup,
    # per-partition values from the round schedule.
    blocks = []
    for sl, im in zip((SA, SB), imgs):
        info = per_img[im]
        g5 = np.zeros((5, sl, P, DUP), f32)
        g5[4] = 1.0  # pad: margin = min(prod - 1, iw) <= 0 since prod < 1
        gt = info["gt"]
        for r, ks in enumerate(info["rounds"]):
            for k in ks:
                p0, p1 = info["iv"][k]
                g5[0, r, p0:p1] = gt["tlx"][k]
                g5[1, r, p0:p1] = gt["tly"][k]
                g5[2, r, p0:p1] = gt["brx"][k]
                g5[3, r, p0:p1] = gt["bry"][k]
                g5[4, r, p0:p1] = cthre * (gt["area"][k] + f32(EPS))
        blocks.append(g5.transpose(2, 0, 1, 3).reshape(P, 5 * sl * DUP))

    # xyc [P,192]: col = comp*96 + aq, cell q = 32p + (aq % 32)
    pidx = np.arange(P)[:, None]
    aqidx = np.arange(96)[None, :]
    q = 32 * pidx + (aqidx % 32)
    gx = (q % nG).astype(f32)
    gy = (q // nG).astype(f32)
    xyc = np.concatenate([(gx + f32(0.5)) / f32(nG), (gy + f32(0.5)) / f32(nG)],
                         axis=1).astype(f32)
    consts_bf = np.ascontiguousarray(
        np.concatenate(blocks + [xyc], axis=1)).astype(bf16)

    # tail consts (f32): gtplane, gtmask, tgt85, onehot, validng, gidx
    gtmask = np.zeros((P, 192), f32)
    gtplane = np.zeros((P, 192), f32)
    cells = []
    for iml, im in enumerate(imgs):
        info = per_img[im]
        for (a, j, i), d in info["cellmap"].items():
            cq = j * nG + i
            gtmask[cq // 32, iml * 96 + a * 32 + cq % 32] = 1.0
            cells.append((iml, a, cq, d["tgt"], d["cls"]))
        if info["has_valid"]:
            gtplane[:, iml * 96:(iml + 1) * 96] = \
                gtmask[:, iml * 96:(iml + 1) * 96]
        else:
            gtplane[:, iml * 96:(iml + 1) * 96] = 1.0

    tgt85 = np.zeros((P, 85), f32)
    onehot = np.zeros((P, 85), f32)
    validng = np.zeros((P, 85), f32)
    gidx = np.zeros((P, NGmax), np.int32)
    for g, (iml, a, cq, tgt, clsset) in enumerate(cells):
        tgt85[g, 0:4] = tgt
        onehot[g, 4] = 1.0
        for c in clsset:
            onehot[g, 5 + c] = 1.0
        validng[g, :] = 1.0
        gidx[:, g] = (iml * 255 + a * 85) * NCELL + cq
    consts_f = np.ascontiguousarray(np.concatenate(
        [gtplane, gtmask, tgt85, onehot, validng, gidx.view(f32)],
        axis=1)).astype(f32)
    return dict(rawsh=rawsh, raw5=raw5, cbf=consts_bf, cf=consts_f)


# ---------------------------------------------------------------------------
# device program
# ---------------------------------------------------------------------------

def _build_program(NA, NB, NGmax, anchors_all, img_size):
    import concourse.bass as bass
    import concourse.mybir as mybir
    from concourse.tile import TileContext

    dtb = mybir.dt.bfloat16
    dtf = mybir.dt.float32
    AF = mybir.ActivationFunctionType
    OP = mybir.AluOpType
    AX = mybir.AxisListType
    cthre = float(IGNORE_THRE / (1.0 + IGNORE_THRE))
    SA, SB = NA * G_R, NB * G_R
    W = G_R * 96

    nc = bass.Bass()

    def _reg_const(value):
        key = (mybir.dt.float32, value)
        if key not in nc.const_aps.aps:
            t = nc.alloc_sbuf_tensor(f"const-f32-{value}", [P, 1],
                                     mybir.dt.float32)
            nc.gpsimd.memset(t.ap(), value)
            nc.const_aps.aps[key] = t.ap()

    lnaw_vals = [float(math.log(anchors_all[a][0] / img_size)) for a in range(nA)]
    for v in lnaw_vals:
        _reg_const(v)
    nc.all_engine_barrier()

    rawsh = nc.declare_dram_parameter("rawsh", [2, 255, NCELL], dtf, False)
    raw5d = nc.declare_dram_parameter("raw5", [P, 960], dtb, False)
    CWS = 5 * (SA + SB) * DUP + 192
    off_g5 = [0, 5 * SA * DUP]
    off_xyc = 5 * (SA + SB) * DUP
    cbf = nc.declare_dram_parameter("cbf", [P, CWS], dtb, False)
    CWF = 192 * 2 + 85 * 3 + NGmax
    off_gtp, off_gtm = 0, 192
    off_tgt, off_oh, off_vn = 384, 384 + 85, 384 + 170
    off_gidx = 384 + 255
    cf = nc.declare_dram_parameter("cf", [P, CWF], dtf, False)
    out = nc.declare_dram_parameter("out", [P, 4], dtf, True)

    def A(t, offset, dims):
        h = t.tensor if hasattr(t, "tensor") else t
        return bass.AP(h, offset, dims)

    with TileContext(nc) as tc, \
            tc.tile_pool(name="main", bufs=1) as pool:
        RAW = pool.tile([P, 960], dtb, name="RAW")
        CS = pool.tile([P, CWS], dtb, name="CS")
        CF = pool.tile([P, CWF], dtf, name="CF")
        E = pool.tile([P, 768], dtb, name="E")
        TL = pool.tile([P, 384], dtb, name="TL")
        BR = pool.tile([P, 384], dtb, name="BR")
        WH = pool.tile([P, 384], dtb, name="WH")
        AREA = pool.tile([P, 192], dtb, name="AREA")
        ATH = pool.tile([P, 192], dtb, name="ATH")
        IX = pool.tile([P, W], dtb, name="IX")
        AXT = pool.tile([P, W], dtb, name="AXT")
        IY = pool.tile([P, W], dtb, name="IY")
        AY = pool.tile([P, W], dtb, name="AY")
        IW = [pool.tile([P, W], dtb, name=f"IW{i}") for i in range(2)]
        IH = pool.tile([P, W], dtb, name="IH")
        PROD = pool.tile([P, W], dtb, name="PROD")
        T = [pool.tile([P, W], dtb, name=f"T{i}") for i in range(2)]
        M = pool.tile([P, W], dtb, name="M")
        M2 = pool.tile([P, W // 2], dtb, name="M2")
        ACC = pool.tile([P, 192], dtb, name="ACC")
        MASK = pool.tile([P, 192], dtf, name="MASK")
        MASKF = pool.tile([P, 192], dtf, name="MASKF")
        MEXCL = pool.tile([P, 192], dtf, name="MEXCL")
        SP = pool.tile([P, 192], dtf, name="SP")
        SPA = pool.tile([P, 192], dtf, name="SPA")
        SPB = pool.tile([P, 192], dtf, name="SPB")
        SPM = pool.tile([P, 192], dtf, name="SPM")
        GT85 = pool.tile([P, 85], dtf, name="GT85")
        U = pool.tile([P, 85], dtf, name="U")
        SPC = pool.tile([P, 85], dtf, name="SPC")
        OC = pool.tile([P, 85], dtf, name="OC")
        SPD = pool.tile([P, 85], dtf, name="SPD")
        OUTS = pool.tile([P, 4], dtf, name="OUTS")

        # ---- input loads (SP queue: raw5+cbf critical, cf for the tail) ----
        nc.sync.dma_start(out=RAW[:], in_=raw5d[:])
        nc.sync.dma_start(out=CS[:], in_=cbf[:])
        nc.sync.dma_start(out=CF[:], in_=cf[:])

        # ---- sparse gt-cell gather (independent of the scan; runs early) ----
        nc.vector.memset(OUTS[:], 0.0)
        nc.gpsimd.indirect_dma_start(
            out=GT85[0:NGmax, 0:85],
            out_offset=None,
            in_=A(rawsh, 0, [[1, (2 * 255 - 85 + 1) * NCELL], [NCELL, 85], [1, 1]]),
            in_offset=bass.IndirectOffsetOnAxis(
                ap=A(CF, off_gidx, [[CWF, 1], [1, NGmax]]).bitcast(
                    mybir.dt.int32),
                axis=0),
        )
        # softplus(z) = ln(1 + exp(-|z|)) + relu(z) over cols 4..85
        nc.scalar.activation(out=SPC[0:NGmax, 4:85], in_=GT85[0:NGmax, 4:85],
                             func=AF.Abs)
        nc.scalar.activation(out=SPD[0:NGmax, 4:85], in_=SPC[0:NGmax, 4:85],
                             func=AF.Exp, scale=-1.0)
        nc.scalar.activation(out=SPC[0:NGmax, 4:85], in_=SPD[0:NGmax, 4:85],
                             func=AF.Ln, bias=1.0)
        nc.scalar.activation(out=SPD[0:NGmax, 4:85], in_=GT85[0:NGmax, 4:85],
                             func=AF.Relu)
        nc.vector.tensor_tensor(out=OC[0:NGmax, 4:85], in0=GT85[0:NGmax, 4:85],
                                in1=A(CF, off_oh + 4, [[CWF, NGmax], [1, 81]]),
                                op=OP.mult)
        nc.vector.tensor_tensor(out=U[0:NGmax, 4:85], in0=SPC[0:NGmax, 4:85],
                                in1=SPD[0:NGmax, 4:85], op=OP.add)
        nc.vector.tensor_tensor(out=U[0:NGmax, 4:85], in0=U[0:NGmax, 4:85],
                                in1=OC[0:NGmax, 4:85], op=OP.subtract)
        # bbox: (ltrb_raw - tgt)^2 in cols 0..4
        nc.vector.tensor_tensor(out=OC[0:NGmax, 0:4], in0=GT85[0:NGmax, 0:4],
                                in1=A(CF, off_tgt, [[CWF, NGmax], [1, 4]]),
                                op=OP.subtract)
        nc.scalar.activation(out=U[0:NGmax, 0:4], in_=OC[0:NGmax, 0:4],
                             func=AF.Square)
        nc.vector.tensor_tensor(out=U[0:NGmax, :], in0=U[0:NGmax, :],
                                in1=A(CF, off_vn, [[CWF, NGmax], [1, 85]]),
                                op=OP.mult)
        nc.vector.reduce_sum(out=OUTS[0:NGmax, 2:3], in_=U[0:NGmax, :],
                             axis=AX.X)

        # ---- pred prep ----
        # E = exp(raw + ln(aw_norm)) per (img, anchor)
        for im in range(2):
            for a in range(3):
                nc.scalar.activation(
                    out=A(E, im * 384 + a * 32, [[768, P], [96, 4], [1, 32]]),
                    in_=A(RAW, im * 480 + a * 32, [[960, P], [96, 4], [1, 32]]),
                    func=AF.Exp,
                    bias=lnaw_vals[a],
                )
        xyc_b = A(CS, off_xyc, [[CWS, P], [96, 2], [0, 2], [1, 96]])
        e_lt = A(E, 0, [[768, P], [96, 2], [384, 2], [1, 96]])
        e_rb = A(E, 192, [[768, P], [96, 2], [384, 2], [1, 96]])
        quad = [[384, P], [192, 2], [96, 2], [1, 96]]
        nc.vector.tensor_tensor(out=A(TL, 0, quad), in0=xyc_b, in1=e_lt,
                                op=OP.subtract)
        nc.vector.tensor_tensor(out=A(BR, 0, quad), in0=xyc_b, in1=e_rb,
                                op=OP.add)
        nc.vector.tensor_tensor(out=A(WH, 0, quad), in0=e_lt, in1=e_rb,
                                op=OP.add)
        nc.vector.tensor_tensor(out=AREA[:], in0=WH[:, 0:192],
                                in1=WH[:, 192:384], op=OP.mult)
        nc.vector.tensor_scalar(out=ATH[:], in0=AREA[:], scalar1=cthre,
                                scalar2=None, op0=OP.mult)
        nc.vector.memset(ACC[:], -100.0)

        # ---- banded iou scan ----
        treng = nc.gpsimd if GPSIMD_TREE else nc.vector

        def gt_ap(im, comp, s0, n=G_R):
            base = off_g5[im] + comp * ((SA, SB)[im] * DUP) + s0 * DUP
            return A(CS, base, [[CWS, P], [DUP, n], [0, 96 // DUP], [1, DUP]])

        def pred_ap(t, comp, im, n=G_R):
            return A(t, comp * 192 + im * 96, [[384, P], [0, n], [1, 96]])

        for im, nbat in ((0, NA), (1, NB)):
            for bi in range(nbat):
                s0 = bi * G_R
                iw = IW[bi % 2]
                tt = T[bi % 2]
                nc.vector.tensor_tensor(out=IX[:], in0=pred_ap(TL, 0, im),
                                        in1=gt_ap(im, 0, s0), op=OP.max)
                nc.vector.tensor_tensor(out=AXT[:], in0=pred_ap(BR, 0, im),
                                        in1=gt_ap(im, 2, s0), op=OP.min)
                nc.vector.tensor_tensor(out=iw[:], in0=AXT[:], in1=IX[:],
                                        op=OP.subtract)
                nc.vector.tensor_tensor(out=IY[:], in0=pred_ap(TL, 1, im),
                                        in1=gt_ap(im, 1, s0), op=OP.max)
                nc.vector.tensor_tensor(out=AY[:], in0=pred_ap(BR, 1, im),
                                        in1=gt_ap(im, 3, s0), op=OP.min)
                nc.vector.tensor_tensor(out=IH[:], in0=AY[:], in1=IY[:],
                                        op=OP.subtract)
                nc.vector.tensor_tensor(out=PROD[:], in0=iw[:], in1=IH[:],
                                        op=OP.mult)
                nc.vector.tensor_tensor(out=tt[:], in0=PROD[:],
                                        in1=gt_ap(im, 4, s0), op=OP.subtract)
                # margin m = min(prod - athg, iw); tree-max over rounds
                treng.tensor_tensor(out=M[:], in0=tt[:], in1=iw[:], op=OP.min)
                width, src, flip = W, M, 0
                while width > 96:
                    h = width // 2
                    dst = (M2, M)[flip % 2]
                    treng.tensor_tensor(out=dst[:, 0:h], in0=src[:, 0:h],
                                        in1=src[:, h:2 * h], op=OP.max)
                    src, width, flip = dst, h, flip + 1
                acc_sl = ACC[:, im * 96:(im + 1) * 96]
                treng.tensor_tensor(out=acc_sl, in0=acc_sl, in1=src[:, 0:96],
                                    op=OP.max)

        # ---- dense conf loss ----
        # notign = (max margin <= athp), per image
        for im in range(2):
            nc.vector.tensor_tensor(out=MASK[:, im * 96:(im + 1) * 96],
                                    in0=ACC[:, im * 96:(im + 1) * 96],
                                    in1=ATH[:, im * 96:(im + 1) * 96],
                                    op=OP.is_le)
        nc.vector.tensor_tensor(out=MASKF[:], in0=MASK[:],
                                in1=A(CF, off_gtp, [[CWF, P], [1, 192]]),
                                op=OP.max)
        nc.vector.tensor_tensor(out=MEXCL[:], in0=MASKF[:],
                                in1=A(CF, off_gtm, [[CWF, P], [1, 192]]),
                                op=OP.subtract)
        conf_view = A(RAW, 384, [[960, P], [480, 2], [1, 96]])
        sp_flat = [[192, P], [96, 2], [1, 96]]
        nc.scalar.activation(out=A(SPA, 0, sp_flat), in_=conf_view, func=AF.Abs)
        nc.scalar.activation(out=SPB[:], in_=SPA[:], func=AF.Exp, scale=-1.0)
        nc.scalar.activation(out=SPA[:], in_=SPB[:], func=AF.Ln, bias=1.0)
        nc.scalar.activation(out=A(SPB, 0, sp_flat), in_=conf_view, func=AF.Relu)
        nc.vector.tensor_tensor(out=SP[:], in0=SPA[:], in1=SPB[:], op=OP.add)
        nc.vector.tensor_tensor(out=SPM[:], in0=SP[:], in1=MEXCL[:], op=OP.mult)
        nc.vector.reduce_sum(out=OUTS[:, 0:1], in_=SPM[:], axis=AX.X)

        nc.sync.dma_start(out=out[:], in_=OUTS[:])

    return nc


_CACHE = {}
TRACE = False
LAST_RESULTS = None


def _split_multiwait(nc):
    """Walrus codegen on this toolchain supports only one sync-wait command
    per instruction; split multi-wait instructions into single-wait NOPs on
    the same engine."""
    import concourse.mybir as mybir

    if getattr(nc, "_fcos_wait_split", False):
        return
    nc._fcos_wait_split = True
    for bb in nc.m.functions[0].blocks:
        insts = bb.instructions
        for ins in list(insts):
            si = ins.sync_info
            if si is not None and len(si.on_wait) > 1:
                waits = list(si.on_wait)
                idx = insts.index(ins)
                nops = []
                for j, w in enumerate(waits[:-1]):
                    nop = mybir.InstNoOp(name=f"{ins.name}-wsplit{j}", ins=[],
                                         outs=[])
                    nop.engine = ins.engine
                    nop.sync_info = mybir.SyncInfo(on_wait=[w], on_update=[])
                    nops.append(nop)
                ins.sync_info = mybir.SyncInfo(on_wait=[waits[-1]],
                                               on_update=list(si.on_update))
                for nop in reversed(nops):
                    insts.insert(idx, nop)


def kernel(raw, labels, anchors_all, img_size):
    from concourse.bass_utils import run_bass_kernel_spmd

    raw = np.asarray(raw, f32)
    labels_np = np.asarray(labels, f32)
    anchors_np = np.asarray(anchors_all, f32)
    isize = int(img_size)

    per_img, A_imgs, B_imgs, NA, NB, NGmax = _plan(labels_np, anchors_np, isize)
    key = (NA, NB, NGmax, G_R, DUP, GPSIMD_TREE, anchors_np.tobytes(), isize)
    if key not in _CACHE:
        _CACHE[key] = _build_program(NA, NB, NGmax, anchors_np.tolist(), isize)
    nc = _CACHE[key]
    _split_multiwait(nc)

    in_maps = [
        _pack_core_inputs(c, per_img, A_imgs, B_imgs, raw, isize, NA, NB, NGmax)
        for c in range(N_CORES)
    ]
    global LAST_RESULTS
    res = run_bass_kernel_spmd(nc, in_maps, list(range(N_CORES)), trace=TRACE)
    LAST_RESULTS = res
    total = np.float64(0.0)
    for c in range(N_CORES):
        o = res.results[c]["out"]
        total += np.sum(o[:, 0], dtype=np.float64)
        total += np.sum(o[:, 2], dtype=np.float64)
    return f32(total)


if __name__ == "__main__":
    import importlib.util

    spec = importlib.util.spec_from_file_location("reference",
                                                  "/root/problem/reference.py")
    ref = importlib.util.module_from_spec(spec)
    spec.loader.exec_module(ref)
    inputs = ref.setup_inputs()
    np_inputs = {k: np.asarray(v) for k, v in inputs.items()}
    got = kernel(**np_inputs)
    print("kernel:", got)


# revision 14
# speedup vs baseline: 2.4393x; 1.0336x over previous
"""Trainium2 Bass kernel for the YOLO/FCOS-layer loss (nn_FCOSLayer_22840636080477).

Sharding: data-parallel over batch, 2 images per NeuronCore x 8 cores, one
SPMD program. Host does label-side preprocessing (anchor matching, scatter
dedup, row-band gt->partition scheduling, constant packing); device does
everything that touches `raw`:

  loss = sum_cells softplus(conf) * (conf_mask & ~gt)          (dense)
       + sum_gtcells [ softplus(conf)-conf                      (sparse, gather)
                      + sum_c (softplus(cls_c) - onehot_c*cls_c)
                      + sum_4 (ltrb_raw - tgt)^2 ]

The ignore mask needs a max-IoU scan of 12288 pred boxes against each
image's gt boxes.  Three structural tricks make it cheap:

1. bf16 + packed last-axis APs -> DVE 2x_1p mode (0.52 ns/elem).
2. Row banding: IoU > 0.6 forces the pred's cell center inside the gt
   box dilated by (2/3)*(wg,hg) (pred-independent bound), so each gt
   only concerns a contiguous band of partitions (partition p holds
   row p//2).  Interval-coloring packs gts into "rounds"; each round
   instruction gives every partition its own gt via per-partition
   consts.  Rounds ~ max-load instead of K.
3. Margin m = min(iw*ih - athg, iw) has the exact sign of the ignore
   condition (valid when gt heights <= 1), killing both clamps; the
   per-pred threshold athp = cthre*areap is compared once at the end:
   ignore <=> max_g m_g > athp.
"""
import sys
import math
import numpy as np

sys.path.insert(0, "/opt/trn_rl_repo")

import ml_dtypes

bf16 = ml_dtypes.bfloat16

N_CLS = 80
nA = 3
STRIDE = 8
IGNORE_THRE = 0.6
EPS = 1e-16
B = 16
K = 50
nG = 64
N_CORES = 8
P = 128
NCELL = nG * nG
f32 = np.float32

# tunables
G_R = 8          # rounds per batched scan op (power of 2)
DUP = 2          # duplicate gt scalars pairwise (bf16 2x packing aid)
DIL = 0.70       # band dilation factor (theory: 2/3; slack for bf16)
ROWPAD = 1       # extra band rows for fp safety
GPSIMD_TREE = False  # Pool engine rejects TENSOR_TENSOR (ISA check)


# ---------------------------------------------------------------------------
# host-side label math (replicates reference.py semantics in f32 numpy)
# ---------------------------------------------------------------------------

def _host_precompute(labels, anchors_all, img_size):
    labels = np.asarray(labels, f32)
    anchors_all = np.asarray(anchors_all, f32)
    img_size = f32(img_size)
    anchors = anchors_all[:nA]
    norm_anch = anchors_all / img_size
    anch_w_n = anchors[:, 0] / img_size

    per_img = []
    for bb in range(B):
        lab = labels[bb]
        valid_row = lab.sum(-1) > 0
        tw, th = lab[:, 3], lab[:, 4]
        inter = np.minimum(tw[:, None], norm_anch[:, 0]) * np.minimum(
            th[:, None], norm_anch[:, 1]
        )
        union = tw[:, None] * th[:, None] + norm_anch[:, 0] * norm_anch[:, 1] - inter
        an_iou = inter / (union + f32(EPS))
        best_n_all = np.argmax(an_iou, axis=-1)
        best_n = best_n_all % nA
        valid = valid_row & (best_n_all < nA)

        ks = np.where(valid_row)[0]
        gcx, gcy, gw, gh = lab[ks, 1], lab[ks, 2], lab[ks, 3], lab[ks, 4]
        gt = dict(
            tlx=(gcx - gw / 2).astype(f32),
            tly=(gcy - gh / 2).astype(f32),
            brx=(gcx + gw / 2).astype(f32),
            bry=(gcy + gh / 2).astype(f32),
            area=(gw * gh).astype(f32),
            gh=gh.astype(f32),
        )

        tx = lab[:, 1] * nG
        ty = lab[:, 2] * nG
        ti = tx.astype(np.int32)
        tj = ty.astype(np.int32)
        tcls = lab[:, 0].astype(np.int32)
        lw, lh = lab[:, 3] * nG, lab[:, 4] * nG
        xc = np.floor(tx) + f32(0.5)
        yc = np.floor(ty) + f32(0.5)
        lab_ltrb = (
            np.maximum(
                np.stack(
                    [xc - (tx - lw / 2), yc - (ty - lh / 2),
                     (tx + lw / 2) - xc, (ty + lh / 2) - yc], -1),
                0.0,
            ) / f32(nG)
        ).astype(f32)
        cellmap = {}
        for k in range(K):
            if not valid[k]:
                continue
            key = (int(best_n[k]), int(tj[k]), int(ti[k]))
            tgt = np.log(lab_ltrb[k] / anch_w_n[best_n[k]] + f32(EPS)).astype(f32)
            if key not in cellmap:
                cellmap[key] = dict(tgt=tgt, cls=set([int(tcls[k])]))
            else:
                cellmap[key]["tgt"] = tgt  # scatter last-wins
                cellmap[key]["cls"].add(int(tcls[k]))

        # row-banded partition sets for the iou scan.  IoU > 0.6 forces the
        # cell center inside the gt box dilated by (2/3)*(wg,hg); partition
        # p holds row p//2, half p%2 (i<32 on even p, i>=32 on odd p).
        gt_n = len(ks)
        gw64 = (gt["brx"] - gt["tlx"]).astype(np.float64)
        gh64 = gt["gh"].astype(np.float64)
        ylo = gt["tly"].astype(np.float64) - DIL * gh64
        yhi = gt["bry"].astype(np.float64) + DIL * gh64
        xlo = gt["tlx"].astype(np.float64) - DIL * gw64
        xhi = gt["brx"].astype(np.float64) + DIL * gw64
        j0 = np.clip(np.floor(ylo * nG - 0.5).astype(int) - ROWPAD, 0, nG - 1)
        j1 = np.clip(np.ceil(yhi * nG - 0.5).astype(int) + ROWPAD, 0, nG - 1)
        i0 = np.clip(np.floor(xlo * nG - 0.5).astype(int) - ROWPAD, 0, nG - 1)
        i1 = np.clip(np.ceil(xhi * nG - 0.5).astype(int) + ROWPAD, 0, nG - 1)
        # greedy set-packing into rounds (128-bit occupancy masks)
        sched = []  # (k, round, p0, p1, step)
        occ = []
        for k2 in sorted(range(gt_n),
                         key=lambda q: (j0[q] - j1[q], q)):
            lo, hi = 2 * int(j0[k2]), 2 * int(j1[k2]) + 2
            if i1[k2] < 32:
                lo, step = lo, 2          # even partitions only
            elif i0[k2] >= 32:
                lo, step = lo + 1, 2      # odd partitions only
            else:
                step = 1
            mask = 0
            for p in range(lo, hi, step):
                mask |= 1 << p
            for r, o in enumerate(occ):
                if not (o & mask):
                    occ[r] |= mask
                    sched.append((k2, r, lo, hi, step))
                    break
            else:
                occ.append(mask)
                sched.append((k2, len(occ) - 1, lo, hi, step))
        per_img.append(dict(K=gt_n, gt=gt, cellmap=cellmap,
                            has_valid=bool(valid.any()),
                            sched=sched, R=len(occ)))
    return per_img


def _plan(labels, anchors_all, img_size):
    per_img = _host_precompute(labels, anchors_all, img_size)
    Rs = [len(info["rounds"]) for info in per_img]
    order = sorted(range(B), key=lambda i: -Rs[i])
    A_imgs = order[:N_CORES]
    B_imgs = order[N_CORES:][::-1]  # pair big-A with small-B
    RA = max(max((Rs[i] for i in A_imgs), default=0), 1)
    RB = max(max((Rs[i] for i in B_imgs), default=0), 1)
    NA = -(-RA // G_R)
    NB = -(-RB // G_R)
    NGmax = 1
    for c in range(N_CORES):
        n = (len(per_img[A_imgs[c]]["cellmap"])
             + len(per_img[B_imgs[c]]["cellmap"]))
        NGmax = max(NGmax, n)
    NGmax = min(-(-NGmax // 8) * 8, P)
    return per_img, A_imgs, B_imgs, NA, NB, NGmax


# ---------------------------------------------------------------------------
# per-core input packing
# ---------------------------------------------------------------------------

def _pack_core_inputs(core, per_img, A_imgs, B_imgs, raw, img_size,
                      NA, NB, NGmax):
    img_size = f32(img_size)
    thre = f32(IGNORE_THRE)
    cthre = (thre / (f32(1.0) + thre)).astype(f32)
    imgs = [A_imgs[core], B_imgs[core]]
    SA, SB = NA * G_R, NB * G_R

    # full-channel raw (gather source) + pre-packed 5-channel bf16 block
    rawsh = np.ascontiguousarray(raw[imgs]).reshape(2, 255, NCELL)
    # raw5 [P, 960]: col = im*480 + ch*96 + a*32 + c ; cell q = 32p + c
    r6 = rawsh.reshape(2, nA, 85, P, 32)[:, :, 0:5]       # im,a,ch,p,c
    raw5 = np.ascontiguousarray(
        r6.transpose(3, 0, 2, 1, 4).reshape(P, 960)).astype(bf16)

    # scan consts (bf16): per-image g5 blocks + xyc
    # g5 block for image im: col = comp*(S*DUP) + slot*DUP + dup,
    # per-partition values from the round schedule.
    blocks = []
    for sl, im in zip((SA, SB), imgs):
        info = per_img[im]
        g5 = np.zeros((5, sl, P, DUP), f32)
        g5[4] = 1.0  # pad: margin = min(prod - 1, iw) <= 0 since prod < 1
        gt = info["gt"]
        for r, ks in enumerate(info["rounds"]):
            for k in ks:
                p0, p1 = info["iv"][k]
                g5[0, r, p0:p1] = gt["tlx"][k]
                g5[1, r, p0:p1] = gt["tly"][k]
                g5[2, r, p0:p1] = gt["brx"][k]
                g5[3, r, p0:p1] = gt["bry"][k]
                g5[4, r, p0:p1] = cthre * (gt["area"][k] + f32(EPS))
        blocks.append(g5.transpose(2, 0, 1, 3).reshape(P, 5 * sl * DUP))

    # xyc [P,192]: col = comp*96 + aq, cell q = 32p + (aq % 32)
    pidx = np.arange(P)[:, None]
    aqidx = np.arange(96)[None, :]
    q = 32 * pidx + (aqidx % 32)
    gx = (q % nG).astype(f32)
    gy = (q // nG).astype(f32)
    xyc = np.concatenate([(gx + f32(0.5)) / f32(nG), (gy + f32(0.5)) / f32(nG)],
                         axis=1).astype(f32)
    consts_bf = np.ascontiguousarray(
        np.concatenate(blocks + [xyc], axis=1)).astype(bf16)

    # tail consts (f32): gtplane, gtmask, tgt85, onehot, validng, gidx
    gtmask = np.zeros((P, 192), f32)
    gtplane = np.zeros((P, 192), f32)
    cells = []
    for iml, im in enumerate(imgs):
        info = per_img[im]
        for (a, j, i), d in info["cellmap"].items():
            cq = j * nG + i
            gtmask[cq // 32, iml * 96 + a * 32 + cq % 32] = 1.0
            cells.append((iml, a, cq, d["tgt"], d["cls"]))
        if info["has_valid"]:
            gtplane[:, iml * 96:(iml + 1) * 96] = \
                gtmask[:, iml * 96:(iml + 1) * 96]
        else:
            gtplane[:, iml * 96:(iml + 1) * 96] = 1.0

    tgt85 = np.zeros((P, 85), f32)
    onehot = np.zeros((P, 85), f32)
    validng = np.zeros((P, 85), f32)
    gidx = np.zeros((P, NGmax), np.int32)
    for g, (iml, a, cq, tgt, clsset) in enumerate(cells):
        tgt85[g, 0:4] = tgt
        onehot[g, 4] = 1.0
        for c in clsset:
            onehot[g, 5 + c] = 1.0
        validng[g, :] = 1.0
        gidx[:, g] = (iml * 255 + a * 85) * NCELL + cq
    consts_f = np.ascontiguousarray(np.concatenate(
        [gtplane, gtmask, tgt85, onehot, validng, gidx.view(f32)],
        axis=1)).astype(f32)
    return dict(rawsh=rawsh, raw5=raw5, cbf=consts_bf, cf=consts_f)


# ---------------------------------------------------------------------------
# device program
# ---------------------------------------------------------------------------

def _build_program(NA, NB, NGmax, anchors_all, img_size):
    import concourse.bass as bass
    import concourse.mybir as mybir
    from concourse.tile import TileContext

    dtb = mybir.dt.bfloat16
    dtf = mybir.dt.float32
    AF = mybir.ActivationFunctionType
    OP = mybir.AluOpType
    AX = mybir.AxisListType
    cthre = float(IGNORE_THRE / (1.0 + IGNORE_THRE))
    SA, SB = NA * G_R, NB * G_R
    W = G_R * 96

    nc = bass.Bass()

    lnaw_vals = [float(math.log(anchors_all[a][0] / img_size)) for a in range(nA)]

    rawsh = nc.declare_dram_parameter("rawsh", [2, 255, NCELL], dtf, False)
    raw5d = nc.declare_dram_parameter("raw5", [P, 960], dtb, False)
    CWS = 5 * (SA + SB) * DUP + 192
    off_g5 = [0, 5 * SA * DUP]
    off_xyc = 5 * (SA + SB) * DUP
    cbf = nc.declare_dram_parameter("cbf", [P, CWS], dtb, False)
    CWF = 192 * 2 + 85 * 3 + NGmax
    off_gtp, off_gtm = 0, 192
    off_tgt, off_oh, off_vn = 384, 384 + 85, 384 + 170
    off_gidx = 384 + 255
    cf = nc.declare_dram_parameter("cf", [P, CWF], dtf, False)
    out = nc.declare_dram_parameter("out", [P, 4], dtf, True)

    def A(t, offset, dims):
        h = t.tensor if hasattr(t, "tensor") else t
        return bass.AP(h, offset, dims)

    with TileContext(nc) as tc, \
            tc.tile_pool(name="main", bufs=1) as pool:
        RAW = pool.tile([P, 960], dtb, name="RAW")
        CS = pool.tile([P, CWS], dtb, name="CS")
        CF = pool.tile([P, CWF], dtf, name="CF")
        E = pool.tile([P, 768], dtb, name="E")
        TL = pool.tile([P, 384], dtb, name="TL")
        BR = pool.tile([P, 384], dtb, name="BR")
        WH = pool.tile([P, 384], dtb, name="WH")
        AREA = pool.tile([P, 192], dtb, name="AREA")
        ATH = pool.tile([P, 192], dtb, name="ATH")
        LNAW = pool.tile([P, 4], dtf, name="LNAW")
        IX = pool.tile([P, W], dtb, name="IX")
        AXT = pool.tile([P, W], dtb, name="AXT")
        IY = pool.tile([P, W], dtb, name="IY")
        AY = pool.tile([P, W], dtb, name="AY")
        IW = pool.tile([P, W], dtb, name="IW")
        IH = pool.tile([P, W], dtb, name="IH")
        PROD = pool.tile([P, W], dtb, name="PROD")
        T = [pool.tile([P, W], dtb, name=f"T{i}") for i in range(2)]
        TR1 = pool.tile([P, W // 2], dtb, name="TR1")
        TR2 = pool.tile([P, W // 4], dtb, name="TR2")
        ACC = pool.tile([P, 192], dtb, name="ACC")
        MASK = pool.tile([P, 192], dtf, name="MASK")
        MASKF = pool.tile([P, 192], dtf, name="MASKF")
        MEXCL = pool.tile([P, 192], dtf, name="MEXCL")
        SP = pool.tile([P, 192], dtf, name="SP")
        SPA = pool.tile([P, 192], dtf, name="SPA")
        SPB = pool.tile([P, 192], dtf, name="SPB")
        SPM = pool.tile([P, 192], dtf, name="SPM")
        GT85 = pool.tile([P, 85], dtf, name="GT85")
        U = pool.tile([P, 85], dtf, name="U")
        SPC = pool.tile([P, 85], dtf, name="SPC")
        OC = pool.tile([P, 85], dtf, name="OC")
        SPD = pool.tile([P, 85], dtf, name="SPD")
        OUTS = pool.tile([P, 4], dtf, name="OUTS")

        # ---- input loads, spread across three DGE queues ----
        nc.sync.dma_start(out=RAW[:], in_=raw5d[:])
        nc.scalar.dma_start(out=CS[:], in_=cbf[:])
        nc.gpsimd.dma_start(out=CF[:], in_=cf[:])
        for a in range(nA):
            nc.gpsimd.memset(LNAW[:, a:a + 1], lnaw_vals[a])

        # ---- sparse gt-cell gather (gpsimd queue, before batch trees; its
        # consumers are emitted after the scan so they don't stall DVE/ACT) ----
        nc.vector.memset(OUTS[:], 0.0)
        nc.gpsimd.indirect_dma_start(
            out=GT85[0:NGmax, 0:85],
            out_offset=None,
            in_=A(rawsh, 0, [[1, (2 * 255 - 85 + 1) * NCELL], [NCELL, 85], [1, 1]]),
            in_offset=bass.IndirectOffsetOnAxis(
                ap=A(CF, off_gidx, [[CWF, 1], [1, NGmax]]).bitcast(
                    mybir.dt.int32),
                axis=0),
        )

        # ---- pred prep ----
        # E = exp(raw + ln(aw_norm)) per (img, anchor); bias from LNAW tile
        for im in range(2):
            for a in range(3):
                nc.scalar.activation(
                    out=A(E, im * 384 + a * 32, [[768, P], [96, 4], [1, 32]]),
                    in_=A(RAW, im * 480 + a * 32, [[960, P], [96, 4], [1, 32]]),
                    func=AF.Exp,
                    bias=A(LNAW, a, [[4, P], [1, 1]]),
                )
        xyc_b = A(CS, off_xyc, [[CWS, P], [96, 2], [0, 2], [1, 96]])
        e_lt = A(E, 0, [[768, P], [96, 2], [384, 2], [1, 96]])
        e_rb = A(E, 192, [[768, P], [96, 2], [384, 2], [1, 96]])
        quad = [[384, P], [192, 2], [96, 2], [1, 96]]
        nc.vector.tensor_tensor(out=A(TL, 0, quad), in0=xyc_b, in1=e_lt,
                                op=OP.subtract)
        nc.vector.tensor_tensor(out=A(BR, 0, quad), in0=xyc_b, in1=e_rb,
                                op=OP.add)
        nc.vector.tensor_tensor(out=A(WH, 0, quad), in0=e_lt, in1=e_rb,
                                op=OP.add)
        nc.vector.tensor_tensor(out=AREA[:], in0=WH[:, 0:192],
                                in1=WH[:, 192:384], op=OP.mult)
        nc.vector.tensor_scalar(out=ATH[:], in0=AREA[:], scalar1=cthre,
                                scalar2=None, op0=OP.mult)
        # conf softplus on ACT while DVE scans
        conf_view = A(RAW, 384, [[960, P], [480, 2], [1, 96]])
        sp_flat = [[192, P], [96, 2], [1, 96]]
        nc.scalar.activation(out=A(SPA, 0, sp_flat), in_=conf_view, func=AF.Abs)
        nc.scalar.activation(out=SPB[:], in_=SPA[:], func=AF.Exp, scale=-1.0)
        nc.scalar.activation(out=SPA[:], in_=SPB[:], func=AF.Ln, bias=1.0)
        nc.scalar.activation(out=A(SPB, 0, sp_flat), in_=conf_view, func=AF.Relu)
        treng = nc.gpsimd if GPSIMD_TREE else nc.vector
        treng.memset(ACC[:], -100.0)

        # ---- banded iou scan ----
        def gt_ap(im, comp, s0, n=G_R):
            base = off_g5[im] + comp * ((SA, SB)[im] * DUP) + s0 * DUP
            return A(CS, base, [[CWS, P], [DUP, n], [0, 96 // DUP], [1, DUP]])

        def pred_ap(t, comp, im, n=G_R):
            return A(t, comp * 192 + im * 96, [[384, P], [0, n], [1, 96]])

        for im, nbat in ((0, NA), (1, NB)):
            for bi in range(nbat):
                s0 = bi * G_R
                tt = T[bi % 2]
                nc.vector.tensor_tensor(out=IX[:], in0=pred_ap(TL, 0, im),
                                        in1=gt_ap(im, 0, s0), op=OP.max)
                nc.vector.tensor_tensor(out=AXT[:], in0=pred_ap(BR, 0, im),
                                        in1=gt_ap(im, 2, s0), op=OP.min)
                nc.vector.tensor_tensor(out=IW[:], in0=AXT[:], in1=IX[:],
                                        op=OP.subtract)
                nc.vector.tensor_tensor(out=IY[:], in0=pred_ap(TL, 1, im),
                                        in1=gt_ap(im, 1, s0), op=OP.max)
                nc.vector.tensor_tensor(out=AY[:], in0=pred_ap(BR, 1, im),
                                        in1=gt_ap(im, 3, s0), op=OP.min)
                nc.vector.tensor_tensor(out=IH[:], in0=AY[:], in1=IY[:],
                                        op=OP.subtract)
                # clamp ih only: iw<0 or ih<0 both give prod <= 0 < ath
                nc.vector.tensor_scalar(out=IH[:], in0=IH[:], scalar1=0.0,
                                        scalar2=None, op0=OP.max)
                nc.vector.tensor_tensor(out=PROD[:], in0=IW[:], in1=IH[:],
                                        op=OP.mult)
                nc.vector.tensor_tensor(out=tt[:], in0=PROD[:],
                                        in1=gt_ap(im, 4, s0), op=OP.subtract)
                # tree-max margins over rounds -> ACC (GpSimd)
                treng.tensor_tensor(out=TR1[:], in0=tt[:, 0:W // 2],
                                    in1=tt[:, W // 2:W], op=OP.max)
                treng.tensor_tensor(out=TR2[:], in0=TR1[:, 0:W // 4],
                                    in1=TR1[:, W // 4:W // 2], op=OP.max)
                acc_sl = ACC[:, im * 96:(im + 1) * 96]
                treng.tensor_tensor(out=TR2[:, 0:96], in0=TR2[:, 0:96],
                                    in1=TR2[:, 96:192], op=OP.max)
                treng.tensor_tensor(out=acc_sl, in0=acc_sl, in1=TR2[:, 0:96],
                                    op=OP.max)
            # notign = (max margin <= athp) for this image
            nc.vector.tensor_tensor(out=MASK[:, im * 96:(im + 1) * 96],
                                    in0=ACC[:, im * 96:(im + 1) * 96],
                                    in1=ATH[:, im * 96:(im + 1) * 96],
                                    op=OP.is_le)

        # ---- sparse gt-cell terms (consumers of the early gather) ----
        # softplus(z) = ln(1 + exp(-|z|)) + relu(z) over cols 4..85
        nc.scalar.activation(out=SPC[0:NGmax, 4:85], in_=GT85[0:NGmax, 4:85],
                             func=AF.Abs)
        nc.scalar.activation(out=SPD[0:NGmax, 4:85], in_=SPC[0:NGmax, 4:85],
                             func=AF.Exp, scale=-1.0)
        nc.scalar.activation(out=SPC[0:NGmax, 4:85], in_=SPD[0:NGmax, 4:85],
                             func=AF.Ln, bias=1.0)
        nc.scalar.activation(out=SPD[0:NGmax, 4:85], in_=GT85[0:NGmax, 4:85],
                             func=AF.Relu)
        nc.vector.tensor_tensor(out=OC[0:NGmax, 4:85], in0=GT85[0:NGmax, 4:85],
                                in1=A(CF, off_oh + 4, [[CWF, NGmax], [1, 81]]),
                                op=OP.mult)
        nc.vector.tensor_tensor(out=U[0:NGmax, 4:85], in0=SPC[0:NGmax, 4:85],
                                in1=SPD[0:NGmax, 4:85], op=OP.add)
        nc.vector.tensor_tensor(out=U[0:NGmax, 4:85], in0=U[0:NGmax, 4:85],
                                in1=OC[0:NGmax, 4:85], op=OP.subtract)
        # bbox: (ltrb_raw - tgt)^2 in cols 0..4
        nc.vector.tensor_tensor(out=OC[0:NGmax, 0:4], in0=GT85[0:NGmax, 0:4],
                                in1=A(CF, off_tgt, [[CWF, NGmax], [1, 4]]),
                                op=OP.subtract)
        nc.scalar.activation(out=U[0:NGmax, 0:4], in_=OC[0:NGmax, 0:4],
                             func=AF.Square)
        nc.vector.tensor_tensor(out=U[0:NGmax, :], in0=U[0:NGmax, :],
                                in1=A(CF, off_vn, [[CWF, NGmax], [1, 85]]),
                                op=OP.mult)
        nc.vector.reduce_sum(out=OUTS[0:NGmax, 2:3], in_=U[0:NGmax, :],
                             axis=AX.X)

        # ---- dense conf loss ----
        nc.vector.tensor_tensor(out=MASKF[:], in0=MASK[:],
                                in1=A(CF, off_gtp, [[CWF, P], [1, 192]]),
                                op=OP.max)
        nc.vector.tensor_tensor(out=MEXCL[:], in0=MASKF[:],
                                in1=A(CF, off_gtm, [[CWF, P], [1, 192]]),
                                op=OP.subtract)
        nc.vector.tensor_tensor(out=SP[:], in0=SPA[:], in1=SPB[:], op=OP.add)
        nc.vector.tensor_tensor(out=SPM[:], in0=SP[:], in1=MEXCL[:], op=OP.mult)
        nc.vector.reduce_sum(out=OUTS[:, 0:1], in_=SPM[:], axis=AX.X)

        nc.sync.dma_start(out=out[:], in_=OUTS[:])

    return nc


_CACHE = {}
TRACE = False
LAST_RESULTS = None


def _split_multiwait(nc):
    """Walrus codegen on this toolchain supports only one sync-wait command
    per instruction; split multi-wait instructions into single-wait NOPs on
    the same engine."""
    import concourse.mybir as mybir

    if getattr(nc, "_fcos_wait_split", False):
        return
    nc._fcos_wait_split = True
    for bb in nc.m.functions[0].blocks:
        insts = bb.instructions
        for ins in list(insts):
            si = ins.sync_info
            if si is not None and len(si.on_wait) > 1:
                waits = list(si.on_wait)
                idx = insts.index(ins)
                nops = []
                for j, w in enumerate(waits[:-1]):
                    nop = mybir.InstNoOp(name=f"{ins.name}-wsplit{j}", ins=[],
                                         outs=[])
                    nop.engine = ins.engine
                    nop.sync_info = mybir.SyncInfo(on_wait=[w], on_update=[])
                    nops.append(nop)
                ins.sync_info = mybir.SyncInfo(on_wait=[waits[-1]],
                                               on_update=list(si.on_update))
                for nop in reversed(nops):
                    insts.insert(idx, nop)


def kernel(raw, labels, anchors_all, img_size):
    from concourse.bass_utils import run_bass_kernel_spmd

    raw = np.asarray(raw, f32)
    labels_np = np.asarray(labels, f32)
    anchors_np = np.asarray(anchors_all, f32)
    isize = int(img_size)

    per_img, A_imgs, B_imgs, NA, NB, NGmax = _plan(labels_np, anchors_np, isize)
    key = (NA, NB, NGmax, G_R, DUP, GPSIMD_TREE, anchors_np.tobytes(), isize)
    if key not in _CACHE:
        _CACHE[key] = _build_program(NA, NB, NGmax, anchors_np.tolist(), isize)
    nc = _CACHE[key]
    _split_multiwait(nc)

    in_maps = [
        _pack_core_inputs(c, per_img, A_imgs, B_imgs, raw, isize, NA, NB, NGmax)
        for c in range(N_CORES)
    ]
    global LAST_RESULTS
    res = run_bass_kernel_spmd(nc, in_maps, list(range(N_CORES)), trace=TRACE)
    LAST_RESULTS = res
    total = np.float64(0.0)
    for c in range(N_CORES):
        o = res.results[c]["out"]
        total += np.sum(o[:, 0], dtype=np.float64)
        total += np.sum(o[:, 2], dtype=np.float64)
    return f32(total)


if __name__ == "__main__":
    import importlib.util

    spec = importlib.util.spec_from_file_location("reference",
                                                  "/root/problem/reference.py")
    ref = importlib.util.module_from_spec(spec)
    spec.loader.exec_module(ref)
    inputs = ref.setup_inputs()
    np_inputs = {k: np.asarray(v) for k, v in inputs.items()}
    got = kernel(**np_inputs)
    print("kernel:", got)


# revision 16
# speedup vs baseline: 2.6524x; 1.0874x over previous
"""Trainium2 Bass kernel for the YOLO/FCOS-layer loss (nn_FCOSLayer_22840636080477).

Sharding: data-parallel over batch, 2 images per NeuronCore x 8 cores, one
SPMD program. Host does label-side preprocessing (anchor matching, scatter
dedup, row-band gt->partition scheduling, constant packing); device does
everything that touches `raw`:

  loss = sum_cells softplus(conf) * (conf_mask & ~gt)          (dense)
       + sum_gtcells [ softplus(conf)-conf                      (sparse, gather)
                      + sum_c (softplus(cls_c) - onehot_c*cls_c)
                      + sum_4 (ltrb_raw - tgt)^2 ]

The ignore mask needs a max-IoU scan of 12288 pred boxes against each
image's gt boxes.  Three structural tricks make it cheap:

1. bf16 + packed last-axis APs -> DVE 2x_1p mode (0.52 ns/elem).
2. Row banding: IoU > 0.6 forces the pred's cell center inside the gt
   box dilated by (2/3)*(wg,hg) (pred-independent bound), so each gt
   only concerns a contiguous band of partitions (partition p holds
   row p//2).  Interval-coloring packs gts into "rounds"; each round
   instruction gives every partition its own gt via per-partition
   consts.  Rounds ~ max-load instead of K.
3. Margin m = min(iw*ih - athg, iw) has the exact sign of the ignore
   condition (valid when gt heights <= 1), killing both clamps; the
   per-pred threshold athp = cthre*areap is compared once at the end:
   ignore <=> max_g m_g > athp.
"""
import sys
import math
import numpy as np

sys.path.insert(0, "/opt/trn_rl_repo")

import ml_dtypes

bf16 = ml_dtypes.bfloat16

N_CLS = 80
nA = 3
STRIDE = 8
IGNORE_THRE = 0.6
EPS = 1e-16
B = 16
K = 50
nG = 64
N_CORES = 8
P = 128
NCELL = nG * nG
f32 = np.float32

# tunables
G_R = 8          # rounds per batched scan op (power of 2)
DUP = 2          # duplicate gt scalars pairwise (bf16 2x packing aid)
DIL = 0.70       # band dilation factor (theory: 2/3; slack for bf16)
ROWPAD = 1       # extra band rows for fp safety
GPSIMD_TREE = False  # Pool engine rejects TENSOR_TENSOR (ISA check)


# ---------------------------------------------------------------------------
# host-side label math (replicates reference.py semantics in f32 numpy)
# ---------------------------------------------------------------------------

def _host_precompute(labels, anchors_all, img_size):
    labels = np.asarray(labels, f32)
    anchors_all = np.asarray(anchors_all, f32)
    img_size = f32(img_size)
    anchors = anchors_all[:nA]
    norm_anch = anchors_all / img_size
    anch_w_n = anchors[:, 0] / img_size

    per_img = []
    for bb in range(B):
        lab = labels[bb]
        valid_row = lab.sum(-1) > 0
        tw, th = lab[:, 3], lab[:, 4]
        inter = np.minimum(tw[:, None], norm_anch[:, 0]) * np.minimum(
            th[:, None], norm_anch[:, 1]
        )
        union = tw[:, None] * th[:, None] + norm_anch[:, 0] * norm_anch[:, 1] - inter
        an_iou = inter / (union + f32(EPS))
        best_n_all = np.argmax(an_iou, axis=-1)
        best_n = best_n_all % nA
        valid = valid_row & (best_n_all < nA)

        ks = np.where(valid_row)[0]
        gcx, gcy, gw, gh = lab[ks, 1], lab[ks, 2], lab[ks, 3], lab[ks, 4]
        gt = dict(
            tlx=(gcx - gw / 2).astype(f32),
            tly=(gcy - gh / 2).astype(f32),
            brx=(gcx + gw / 2).astype(f32),
            bry=(gcy + gh / 2).astype(f32),
            area=(gw * gh).astype(f32),
            gh=gh.astype(f32),
        )

        tx = lab[:, 1] * nG
        ty = lab[:, 2] * nG
        ti = tx.astype(np.int32)
        tj = ty.astype(np.int32)
        tcls = lab[:, 0].astype(np.int32)
        lw, lh = lab[:, 3] * nG, lab[:, 4] * nG
        xc = np.floor(tx) + f32(0.5)
        yc = np.floor(ty) + f32(0.5)
        lab_ltrb = (
            np.maximum(
                np.stack(
                    [xc - (tx - lw / 2), yc - (ty - lh / 2),
                     (tx + lw / 2) - xc, (ty + lh / 2) - yc], -1),
                0.0,
            ) / f32(nG)
        ).astype(f32)
        cellmap = {}
        for k in range(K):
            if not valid[k]:
                continue
            key = (int(best_n[k]), int(tj[k]), int(ti[k]))
            tgt = np.log(lab_ltrb[k] / anch_w_n[best_n[k]] + f32(EPS)).astype(f32)
            if key not in cellmap:
                cellmap[key] = dict(tgt=tgt, cls=set([int(tcls[k])]))
            else:
                cellmap[key]["tgt"] = tgt  # scatter last-wins
                cellmap[key]["cls"].add(int(tcls[k]))

        # row-banded partition sets for the iou scan.  IoU > 0.6 forces the
        # cell center inside the gt box dilated by (2/3)*(wg,hg); partition
        # p holds row p//2, half p%2 (i<32 on even p, i>=32 on odd p).
        gt_n = len(ks)
        gw64 = (gt["brx"] - gt["tlx"]).astype(np.float64)
        gh64 = gt["gh"].astype(np.float64)
        ylo = gt["tly"].astype(np.float64) - DIL * gh64
        yhi = gt["bry"].astype(np.float64) + DIL * gh64
        xlo = gt["tlx"].astype(np.float64) - DIL * gw64
        xhi = gt["brx"].astype(np.float64) + DIL * gw64
        j0 = np.clip(np.floor(ylo * nG - 0.5).astype(int) - ROWPAD, 0, nG - 1)
        j1 = np.clip(np.ceil(yhi * nG - 0.5).astype(int) + ROWPAD, 0, nG - 1)
        i0 = np.clip(np.floor(xlo * nG - 0.5).astype(int) - ROWPAD, 0, nG - 1)
        i1 = np.clip(np.ceil(xhi * nG - 0.5).astype(int) + ROWPAD, 0, nG - 1)
        # greedy set-packing into rounds (128-bit occupancy masks)
        sched = []  # (k, round, p0, p1, step)
        occ = []
        for k2 in sorted(range(gt_n),
                         key=lambda q: (j0[q] - j1[q], q)):
            lo, hi = 2 * int(j0[k2]), 2 * int(j1[k2]) + 2
            if i1[k2] < 32:
                lo, step = lo, 2          # even partitions only
            elif i0[k2] >= 32:
                lo, step = lo + 1, 2      # odd partitions only
            else:
                step = 1
            mask = 0
            for p in range(lo, hi, step):
                mask |= 1 << p
            for r, o in enumerate(occ):
                if not (o & mask):
                    occ[r] |= mask
                    sched.append((k2, r, lo, hi, step))
                    break
            else:
                occ.append(mask)
                sched.append((k2, len(occ) - 1, lo, hi, step))
        per_img.append(dict(K=gt_n, gt=gt, cellmap=cellmap,
                            has_valid=bool(valid.any()),
                            sched=sched, R=len(occ)))
    return per_img


def _plan(labels, anchors_all, img_size):
    per_img = _host_precompute(labels, anchors_all, img_size)
    Rs = [info["R"] for info in per_img]
    order = sorted(range(B), key=lambda i: -Rs[i])
    A_imgs = order[:N_CORES]
    B_imgs = order[N_CORES:][::-1]  # pair big-A with small-B
    RA = max(max((Rs[i] for i in A_imgs), default=0), 1)
    RB = max(max((Rs[i] for i in B_imgs), default=0), 1)
    NA = -(-RA // G_R)
    NB = -(-RB // G_R)
    NGmax = 1
    for c in range(N_CORES):
        n = (len(per_img[A_imgs[c]]["cellmap"])
             + len(per_img[B_imgs[c]]["cellmap"]))
        NGmax = max(NGmax, n)
    NGmax = min(-(-NGmax // 8) * 8, P)
    return per_img, A_imgs, B_imgs, NA, NB, NGmax


# ---------------------------------------------------------------------------
# per-core input packing
# ---------------------------------------------------------------------------

def _pack_core_inputs(core, per_img, A_imgs, B_imgs, raw, img_size,
                      NA, NB, NGmax):
    img_size = f32(img_size)
    thre = f32(IGNORE_THRE)
    cthre = (thre / (f32(1.0) + thre)).astype(f32)
    imgs = [A_imgs[core], B_imgs[core]]
    SA, SB = NA * G_R, NB * G_R

    # full-channel raw (gather source) + pre-packed 5-channel bf16 block
    rawsh = np.ascontiguousarray(raw[imgs]).reshape(2, 255, NCELL)
    # raw5 [P, 960]: col = im*480 + ch*96 + a*32 + c ; cell q = 32p + c
    r6 = rawsh.reshape(2, nA, 85, P, 32)[:, :, 0:5]       # im,a,ch,p,c
    raw5 = np.ascontiguousarray(
        r6.transpose(3, 0, 2, 1, 4).reshape(P, 960)).astype(bf16)

    # scan consts (bf16): per-image g5 blocks + xyc
    # g5 block for image im: col = comp*(S*DUP) + slot*DUP + dup,
    # per-partition values from the round schedule.
    blocks = []
    for sl, im in zip((SA, SB), imgs):
        info = per_img[im]
        g5 = np.zeros((5, sl, P, DUP), f32)
        g5[4] = 1.0  # pad: prod - 1 <= 0 always (boxes within [0,1])
        gt = info["gt"]
        for k, r, p0, p1, step in info["sched"]:
            sl_ = slice(p0, p1, step)
            g5[0, r, sl_] = gt["tlx"][k]
            g5[1, r, sl_] = gt["tly"][k]
            g5[2, r, sl_] = gt["brx"][k]
            g5[3, r, sl_] = gt["bry"][k]
            g5[4, r, sl_] = cthre * (gt["area"][k] + f32(EPS))
        blocks.append(g5.transpose(2, 0, 1, 3).reshape(P, 5 * sl * DUP))

    # xyc [P,192]: col = comp*96 + aq, cell q = 32p + (aq % 32)
    pidx = np.arange(P)[:, None]
    aqidx = np.arange(96)[None, :]
    q = 32 * pidx + (aqidx % 32)
    gx = (q % nG).astype(f32)
    gy = (q // nG).astype(f32)
    xyc = np.concatenate([(gx + f32(0.5)) / f32(nG), (gy + f32(0.5)) / f32(nG)],
                         axis=1).astype(f32)
    consts_bf = np.ascontiguousarray(
        np.concatenate(blocks + [xyc], axis=1)).astype(bf16)

    # tail consts (f32): gtplane, gtmask, tgt85, onehot, validng, gidx
    gtmask = np.zeros((P, 192), f32)
    gtplane = np.zeros((P, 192), f32)
    cells = []
    for iml, im in enumerate(imgs):
        info = per_img[im]
        for (a, j, i), d in info["cellmap"].items():
            cq = j * nG + i
            gtmask[cq // 32, iml * 96 + a * 32 + cq % 32] = 1.0
            cells.append((iml, a, cq, d["tgt"], d["cls"]))
        if info["has_valid"]:
            gtplane[:, iml * 96:(iml + 1) * 96] = \
                gtmask[:, iml * 96:(iml + 1) * 96]
        else:
            gtplane[:, iml * 96:(iml + 1) * 96] = 1.0

    tgt85 = np.zeros((P, 85), f32)
    onehot = np.zeros((P, 85), f32)
    validng = np.zeros((P, 85), f32)
    gidx = np.zeros((P, NGmax), np.int32)
    for g, (iml, a, cq, tgt, clsset) in enumerate(cells):
        tgt85[g, 0:4] = tgt
        onehot[g, 4] = 1.0
        for c in clsset:
            onehot[g, 5 + c] = 1.0
        validng[g, :] = 1.0
        gidx[:, g] = (iml * 255 + a * 85) * NCELL + cq
    consts_f = np.ascontiguousarray(np.concatenate(
        [gtplane, gtmask, tgt85, onehot, validng, gidx.view(f32)],
        axis=1)).astype(f32)
    return dict(rawsh=rawsh, raw5=raw5, cbf=consts_bf, cf=consts_f)


# ---------------------------------------------------------------------------
# device program
# ---------------------------------------------------------------------------

def _build_program(NA, NB, NGmax, anchors_all, img_size):
    import concourse.bass as bass
    import concourse.mybir as mybir
    from concourse.tile import TileContext

    dtb = mybir.dt.bfloat16
    dtf = mybir.dt.float32
    AF = mybir.ActivationFunctionType
    OP = mybir.AluOpType
    AX = mybir.AxisListType
    cthre = float(IGNORE_THRE / (1.0 + IGNORE_THRE))
    SA, SB = NA * G_R, NB * G_R
    W = G_R * 96

    nc = bass.Bass()

    lnaw_vals = [float(math.log(anchors_all[a][0] / img_size)) for a in range(nA)]

    rawsh = nc.declare_dram_parameter("rawsh", [2, 255, NCELL], dtf, False)
    raw5d = nc.declare_dram_parameter("raw5", [P, 960], dtb, False)
    CWS = 5 * (SA + SB) * DUP + 192
    off_g5 = [0, 5 * SA * DUP]
    off_xyc = 5 * (SA + SB) * DUP
    cbf = nc.declare_dram_parameter("cbf", [P, CWS], dtb, False)
    CWF = 192 * 2 + 85 * 3 + NGmax
    off_gtp, off_gtm = 0, 192
    off_tgt, off_oh, off_vn = 384, 384 + 85, 384 + 170
    off_gidx = 384 + 255
    cf = nc.declare_dram_parameter("cf", [P, CWF], dtf, False)
    out = nc.declare_dram_parameter("out", [P, 4], dtf, True)

    def A(t, offset, dims):
        h = t.tensor if hasattr(t, "tensor") else t
        return bass.AP(h, offset, dims)

    with TileContext(nc) as tc, \
            tc.tile_pool(name="main", bufs=1) as pool:
        RAW = pool.tile([P, 960], dtb, name="RAW")
        CS = pool.tile([P, CWS], dtb, name="CS")
        CF = pool.tile([P, CWF], dtf, name="CF")
        E = pool.tile([P, 768], dtb, name="E")
        TL = pool.tile([P, 384], dtb, name="TL")
        BR = pool.tile([P, 384], dtb, name="BR")
        WH = pool.tile([P, 384], dtb, name="WH")
        AREA = pool.tile([P, 192], dtb, name="AREA")
        ATH = pool.tile([P, 192], dtb, name="ATH")
        LNAW = pool.tile([P, 4], dtf, name="LNAW")
        IX = pool.tile([P, W], dtb, name="IX")
        AXT = pool.tile([P, W], dtb, name="AXT")
        IY = pool.tile([P, W], dtb, name="IY")
        AY = pool.tile([P, W], dtb, name="AY")
        IW = pool.tile([P, W], dtb, name="IW")
        IH = pool.tile([P, W], dtb, name="IH")
        PROD = pool.tile([P, W], dtb, name="PROD")
        T = [pool.tile([P, W], dtb, name=f"T{i}") for i in range(2)]
        TR1 = pool.tile([P, W // 2], dtb, name="TR1")
        TR2 = pool.tile([P, W // 4], dtb, name="TR2")
        ACC = pool.tile([P, 192], dtb, name="ACC")
        MASK = pool.tile([P, 192], dtf, name="MASK")
        MASKF = pool.tile([P, 192], dtf, name="MASKF")
        MEXCL = pool.tile([P, 192], dtf, name="MEXCL")
        SP = pool.tile([P, 192], dtf, name="SP")
        SPA = pool.tile([P, 192], dtf, name="SPA")
        SPB = pool.tile([P, 192], dtf, name="SPB")
        SPM = pool.tile([P, 192], dtf, name="SPM")
        GT85 = pool.tile([P, 85], dtf, name="GT85")
        U = pool.tile([P, 85], dtf, name="U")
        SPC = pool.tile([P, 85], dtf, name="SPC")
        OC = pool.tile([P, 85], dtf, name="OC")
        SPD = pool.tile([P, 85], dtf, name="SPD")
        OUTS = pool.tile([P, 4], dtf, name="OUTS")

        # ---- input loads, spread across three DGE queues ----
        nc.sync.dma_start(out=RAW[:], in_=raw5d[:])
        nc.scalar.dma_start(out=CS[:], in_=cbf[:])
        nc.gpsimd.dma_start(out=CF[:], in_=cf[:])
        for a in range(nA):
            nc.gpsimd.memset(LNAW[:, a:a + 1], lnaw_vals[a])

        # ---- sparse gt-cell gather (gpsimd queue, before batch trees; its
        # consumers are emitted after the scan so they don't stall DVE/ACT) ----
        nc.vector.memset(OUTS[:], 0.0)
        nc.gpsimd.indirect_dma_start(
            out=GT85[0:NGmax, 0:85],
            out_offset=None,
            in_=A(rawsh, 0, [[1, (2 * 255 - 85 + 1) * NCELL], [NCELL, 85], [1, 1]]),
            in_offset=bass.IndirectOffsetOnAxis(
                ap=A(CF, off_gidx, [[CWF, 1], [1, NGmax]]).bitcast(
                    mybir.dt.int32),
                axis=0),
        )

        # ---- pred prep ----
        # E = exp(raw + ln(aw_norm)) per (img, anchor); bias from LNAW tile
        for im in range(2):
            for a in range(3):
                nc.scalar.activation(
                    out=A(E, im * 384 + a * 32, [[768, P], [96, 4], [1, 32]]),
                    in_=A(RAW, im * 480 + a * 32, [[960, P], [96, 4], [1, 32]]),
                    func=AF.Exp,
                    bias=A(LNAW, a, [[4, P], [1, 1]]),
                )
        xyc_b = A(CS, off_xyc, [[CWS, P], [96, 2], [0, 2], [1, 96]])
        e_lt = A(E, 0, [[768, P], [96, 2], [384, 2], [1, 96]])
        e_rb = A(E, 192, [[768, P], [96, 2], [384, 2], [1, 96]])
        quad = [[384, P], [192, 2], [96, 2], [1, 96]]
        nc.vector.tensor_tensor(out=A(TL, 0, quad), in0=xyc_b, in1=e_lt,
                                op=OP.subtract)
        nc.vector.tensor_tensor(out=A(BR, 0, quad), in0=xyc_b, in1=e_rb,
                                op=OP.add)
        nc.vector.tensor_tensor(out=A(WH, 0, quad), in0=e_lt, in1=e_rb,
                                op=OP.add)
        nc.vector.tensor_tensor(out=AREA[:], in0=WH[:, 0:192],
                                in1=WH[:, 192:384], op=OP.mult)
        nc.vector.tensor_scalar(out=ATH[:], in0=AREA[:], scalar1=cthre,
                                scalar2=None, op0=OP.mult)
        # conf softplus on ACT while DVE scans
        conf_view = A(RAW, 384, [[960, P], [480, 2], [1, 96]])
        sp_flat = [[192, P], [96, 2], [1, 96]]
        nc.scalar.activation(out=A(SPA, 0, sp_flat), in_=conf_view, func=AF.Abs)
        nc.scalar.activation(out=SPB[:], in_=SPA[:], func=AF.Exp, scale=-1.0)
        nc.scalar.activation(out=SPA[:], in_=SPB[:], func=AF.Ln, bias=1.0)
        nc.scalar.activation(out=A(SPB, 0, sp_flat), in_=conf_view, func=AF.Relu)
        treng = nc.gpsimd if GPSIMD_TREE else nc.vector
        treng.memset(ACC[:], -100.0)

        # ---- banded iou scan ----
        def gt_ap(im, comp, s0, n=G_R):
            base = off_g5[im] + comp * ((SA, SB)[im] * DUP) + s0 * DUP
            return A(CS, base, [[CWS, P], [DUP, n], [0, 96 // DUP], [1, DUP]])

        def pred_ap(t, comp, im, n=G_R):
            return A(t, comp * 192 + im * 96, [[384, P], [0, n], [1, 96]])

        for im, nbat in ((0, NA), (1, NB)):
            for bi in range(nbat):
                s0 = bi * G_R
                tt = T[bi % 2]
                nc.vector.tensor_tensor(out=IX[:], in0=pred_ap(TL, 0, im),
                                        in1=gt_ap(im, 0, s0), op=OP.max)
                nc.vector.tensor_tensor(out=AXT[:], in0=pred_ap(BR, 0, im),
                                        in1=gt_ap(im, 2, s0), op=OP.min)
                nc.vector.tensor_tensor(out=IW[:], in0=AXT[:], in1=IX[:],
                                        op=OP.subtract)
                nc.vector.tensor_tensor(out=IY[:], in0=pred_ap(TL, 1, im),
                                        in1=gt_ap(im, 1, s0), op=OP.max)
                nc.vector.tensor_tensor(out=AY[:], in0=pred_ap(BR, 1, im),
                                        in1=gt_ap(im, 3, s0), op=OP.min)
                nc.vector.tensor_tensor(out=IH[:], in0=AY[:], in1=IY[:],
                                        op=OP.subtract)
                # clamp ih only: iw<0 or ih<0 both give prod <= 0 < ath
                nc.vector.tensor_scalar(out=IH[:], in0=IH[:], scalar1=0.0,
                                        scalar2=None, op0=OP.max)
                nc.vector.tensor_tensor(out=PROD[:], in0=IW[:], in1=IH[:],
                                        op=OP.mult)
                nc.vector.tensor_tensor(out=tt[:], in0=PROD[:],
                                        in1=gt_ap(im, 4, s0), op=OP.subtract)
                # tree-max margins over rounds -> ACC (GpSimd)
                treng.tensor_tensor(out=TR1[:], in0=tt[:, 0:W // 2],
                                    in1=tt[:, W // 2:W], op=OP.max)
                treng.tensor_tensor(out=TR2[:], in0=TR1[:, 0:W // 4],
                                    in1=TR1[:, W // 4:W // 2], op=OP.max)
                acc_sl = ACC[:, im * 96:(im + 1) * 96]
                treng.tensor_tensor(out=TR2[:, 0:96], in0=TR2[:, 0:96],
                                    in1=TR2[:, 96:192], op=OP.max)
                treng.tensor_tensor(out=acc_sl, in0=acc_sl, in1=TR2[:, 0:96],
                                    op=OP.max)
            # notign = (max margin <= athp) for this image
            nc.vector.tensor_tensor(out=MASK[:, im * 96:(im + 1) * 96],
                                    in0=ACC[:, im * 96:(im + 1) * 96],
                                    in1=ATH[:, im * 96:(im + 1) * 96],
                                    op=OP.is_le)

        # ---- sparse gt-cell terms (consumers of the early gather) ----
        # softplus(z) = ln(1 + exp(-|z|)) + relu(z) over cols 4..85
        nc.scalar.activation(out=SPC[0:NGmax, 4:85], in_=GT85[0:NGmax, 4:85],
                             func=AF.Abs)
        nc.scalar.activation(out=SPD[0:NGmax, 4:85], in_=SPC[0:NGmax, 4:85],
                             func=AF.Exp, scale=-1.0)
        nc.scalar.activation(out=SPC[0:NGmax, 4:85], in_=SPD[0:NGmax, 4:85],
                             func=AF.Ln, bias=1.0)
        nc.scalar.activation(out=SPD[0:NGmax, 4:85], in_=GT85[0:NGmax, 4:85],
                             func=AF.Relu)
        nc.vector.tensor_tensor(out=OC[0:NGmax, 4:85], in0=GT85[0:NGmax, 4:85],
                                in1=A(CF, off_oh + 4, [[CWF, NGmax], [1, 81]]),
                                op=OP.mult)
        nc.vector.tensor_tensor(out=U[0:NGmax, 4:85], in0=SPC[0:NGmax, 4:85],
                                in1=SPD[0:NGmax, 4:85], op=OP.add)
        nc.vector.tensor_tensor(out=U[0:NGmax, 4:85], in0=U[0:NGmax, 4:85],
                                in1=OC[0:NGmax, 4:85], op=OP.subtract)
        # bbox: (ltrb_raw - tgt)^2 in cols 0..4
        nc.vector.tensor_tensor(out=OC[0:NGmax, 0:4], in0=GT85[0:NGmax, 0:4],
                                in1=A(CF, off_tgt, [[CWF, NGmax], [1, 4]]),
                                op=OP.subtract)
        nc.scalar.activation(out=U[0:NGmax, 0:4], in_=OC[0:NGmax, 0:4],
                             func=AF.Square)
        nc.vector.tensor_tensor(out=U[0:NGmax, :], in0=U[0:NGmax, :],
                                in1=A(CF, off_vn, [[CWF, NGmax], [1, 85]]),
                                op=OP.mult)
        nc.vector.reduce_sum(out=OUTS[0:NGmax, 2:3], in_=U[0:NGmax, :],
                             axis=AX.X)

        # ---- dense conf loss ----
        nc.vector.tensor_tensor(out=MASKF[:], in0=MASK[:],
                                in1=A(CF, off_gtp, [[CWF, P], [1, 192]]),
                                op=OP.max)
        nc.vector.tensor_tensor(out=MEXCL[:], in0=MASKF[:],
                                in1=A(CF, off_gtm, [[CWF, P], [1, 192]]),
                                op=OP.subtract)
        nc.vector.tensor_tensor(out=SP[:], in0=SPA[:], in1=SPB[:], op=OP.add)
        nc.vector.tensor_tensor(out=SPM[:], in0=SP[:], in1=MEXCL[:], op=OP.mult)
        nc.vector.reduce_sum(out=OUTS[:, 0:1], in_=SPM[:], axis=AX.X)

        nc.sync.dma_start(out=out[:], in_=OUTS[:])

    return nc


_CACHE = {}
TRACE = False
LAST_RESULTS = None


def _split_multiwait(nc):
    """Walrus codegen on this toolchain supports only one sync-wait command
    per instruction; split multi-wait instructions into single-wait NOPs on
    the same engine."""
    import concourse.mybir as mybir

    if getattr(nc, "_fcos_wait_split", False):
        return
    nc._fcos_wait_split = True
    for bb in nc.m.functions[0].blocks:
        insts = bb.instructions
        for ins in list(insts):
            si = ins.sync_info
            if si is not None and len(si.on_wait) > 1:
                waits = list(si.on_wait)
                idx = insts.index(ins)
                nops = []
                for j, w in enumerate(waits[:-1]):
                    nop = mybir.InstNoOp(name=f"{ins.name}-wsplit{j}", ins=[],
                                         outs=[])
                    nop.engine = ins.engine
                    nop.sync_info = mybir.SyncInfo(on_wait=[w], on_update=[])
                    nops.append(nop)
                ins.sync_info = mybir.SyncInfo(on_wait=[waits[-1]],
                                               on_update=list(si.on_update))
                for nop in reversed(nops):
                    insts.insert(idx, nop)


def kernel(raw, labels, anchors_all, img_size):
    from concourse.bass_utils import run_bass_kernel_spmd

    raw = np.asarray(raw, f32)
    labels_np = np.asarray(labels, f32)
    anchors_np = np.asarray(anchors_all, f32)
    isize = int(img_size)

    per_img, A_imgs, B_imgs, NA, NB, NGmax = _plan(labels_np, anchors_np, isize)
    key = (NA, NB, NGmax, G_R, DUP, GPSIMD_TREE, anchors_np.tobytes(), isize)
    if key not in _CACHE:
        _CACHE[key] = _build_program(NA, NB, NGmax, anchors_np.tolist(), isize)
    nc = _CACHE[key]
    _split_multiwait(nc)

    in_maps = [
        _pack_core_inputs(c, per_img, A_imgs, B_imgs, raw, isize, NA, NB, NGmax)
        for c in range(N_CORES)
    ]
    global LAST_RESULTS
    res = run_bass_kernel_spmd(nc, in_maps, list(range(N_CORES)), trace=TRACE)
    LAST_RESULTS = res
    total = np.float64(0.0)
    for c in range(N_CORES):
        o = res.results[c]["out"]
        total += np.sum(o[:, 0], dtype=np.float64)
        total += np.sum(o[:, 2], dtype=np.float64)
    return f32(total)


if __name__ == "__main__":
    import importlib.util

    spec = importlib.util.spec_from_file_location("reference",
                                                  "/root/problem/reference.py")
    ref = importlib.util.module_from_spec(spec)
    spec.loader.exec_module(ref)
    inputs = ref.setup_inputs()
    np_inputs = {k: np.asarray(v) for k, v in inputs.items()}
    got = kernel(**np_inputs)
    print("kernel:", got)


# revision 26
# speedup vs baseline: 3.0102x; 1.1349x over previous
"""Trainium2 Bass kernel for the YOLO/FCOS-layer loss (nn_FCOSLayer_22840636080477).

Sharding: data-parallel over batch, 2 images per NeuronCore x 8 cores, one
SPMD program. Host does label-side preprocessing (anchor matching, scatter
dedup, row-band gt->partition scheduling, constant packing); device does
everything that touches `raw`:

  loss = sum_cells softplus(conf) * (conf_mask & ~gt)          (dense)
       + sum_gtcells [ softplus(conf)-conf                      (sparse, gather)
                      + sum_c (softplus(cls_c) - onehot_c*cls_c)
                      + sum_4 (ltrb_raw - tgt)^2 ]

The ignore mask needs a max-IoU scan of 12288 pred boxes against each
image's gt boxes.  Three structural tricks make it cheap:

1. bf16 + packed last-axis APs -> DVE 2x_1p mode (0.52 ns/elem).
2. Row banding: IoU > 0.6 forces the pred's cell center inside the gt
   box dilated by (2/3)*(wg,hg) (pred-independent bound), so each gt
   only concerns a contiguous band of partitions (partition p holds
   row p//2).  Interval-coloring packs gts into "rounds"; each round
   instruction gives every partition its own gt via per-partition
   consts.  Rounds ~ max-load instead of K.
3. Margin m = min(iw*ih - athg, iw) has the exact sign of the ignore
   condition (valid when gt heights <= 1), killing both clamps; the
   per-pred threshold athp = cthre*areap is compared once at the end:
   ignore <=> max_g m_g > athp.
"""
import sys
import math
import numpy as np

sys.path.insert(0, "/opt/trn_rl_repo")

import ml_dtypes

bf16 = ml_dtypes.bfloat16

N_CLS = 80
nA = 3
STRIDE = 8
IGNORE_THRE = 0.6
EPS = 1e-16
B = 16
K = 50
nG = 64
N_CORES = 8
P = 128
NCELL = nG * nG
f32 = np.float32

# tunables
G_R = 8          # rounds per batched scan op (power of 2)
DUP = 2          # duplicate gt scalars pairwise (bf16 2x packing aid)
DIL = 0.70       # band dilation factor (theory: 2/3; slack for bf16)
ROWPAD = 1       # extra band rows for fp safety
GPSIMD_TREE = False  # Pool engine rejects TENSOR_TENSOR (ISA check)


# ---------------------------------------------------------------------------
# host-side label math (replicates reference.py semantics in f32 numpy)
# ---------------------------------------------------------------------------

def _host_precompute(labels, anchors_all, img_size):
    labels = np.asarray(labels, f32)
    anchors_all = np.asarray(anchors_all, f32)
    img_size = f32(img_size)
    anchors = anchors_all[:nA]
    norm_anch = anchors_all / img_size
    anch_w_n = anchors[:, 0] / img_size

    per_img = []
    for bb in range(B):
        lab = labels[bb]
        valid_row = lab.sum(-1) > 0
        tw, th = lab[:, 3], lab[:, 4]
        inter = np.minimum(tw[:, None], norm_anch[:, 0]) * np.minimum(
            th[:, None], norm_anch[:, 1]
        )
        union = tw[:, None] * th[:, None] + norm_anch[:, 0] * norm_anch[:, 1] - inter
        an_iou = inter / (union + f32(EPS))
        best_n_all = np.argmax(an_iou, axis=-1)
        best_n = best_n_all % nA
        valid = valid_row & (best_n_all < nA)

        ks = np.where(valid_row)[0]
        gcx, gcy, gw, gh = lab[ks, 1], lab[ks, 2], lab[ks, 3], lab[ks, 4]
        gt = dict(
            tlx=(gcx - gw / 2).astype(f32),
            tly=(gcy - gh / 2).astype(f32),
            brx=(gcx + gw / 2).astype(f32),
            bry=(gcy + gh / 2).astype(f32),
            area=(gw * gh).astype(f32),
            gh=gh.astype(f32),
        )

        tx = lab[:, 1] * nG
        ty = lab[:, 2] * nG
        ti = tx.astype(np.int32)
        tj = ty.astype(np.int32)
        tcls = lab[:, 0].astype(np.int32)
        lw, lh = lab[:, 3] * nG, lab[:, 4] * nG
        xc = np.floor(tx) + f32(0.5)
        yc = np.floor(ty) + f32(0.5)
        lab_ltrb = (
            np.maximum(
                np.stack(
                    [xc - (tx - lw / 2), yc - (ty - lh / 2),
                     (tx + lw / 2) - xc, (ty + lh / 2) - yc], -1),
                0.0,
            ) / f32(nG)
        ).astype(f32)
        cellmap = {}
        for k in range(K):
            if not valid[k]:
                continue
            key = (int(best_n[k]), int(tj[k]), int(ti[k]))
            tgt = np.log(lab_ltrb[k] / anch_w_n[best_n[k]] + f32(EPS)).astype(f32)
            if key not in cellmap:
                cellmap[key] = dict(tgt=tgt, cls=set([int(tcls[k])]))
            else:
                cellmap[key]["tgt"] = tgt  # scatter last-wins
                cellmap[key]["cls"].add(int(tcls[k]))

        # row-banded partition sets for the iou scan.  IoU > 0.6 forces the
        # cell center inside the gt box dilated by (2/3)*(wg,hg); partition
        # p holds row p//2, half p%2 (i<32 on even p, i>=32 on odd p).
        gt_n = len(ks)
        gw64 = (gt["brx"] - gt["tlx"]).astype(np.float64)
        gh64 = gt["gh"].astype(np.float64)
        ylo = gt["tly"].astype(np.float64) - DIL * gh64
        yhi = gt["bry"].astype(np.float64) + DIL * gh64
        xlo = gt["tlx"].astype(np.float64) - DIL * gw64
        xhi = gt["brx"].astype(np.float64) + DIL * gw64
        j0 = np.clip(np.floor(ylo * nG - 0.5).astype(int) - ROWPAD, 0, nG - 1)
        j1 = np.clip(np.ceil(yhi * nG - 0.5).astype(int) + ROWPAD, 0, nG - 1)
        i0 = np.clip(np.floor(xlo * nG - 0.5).astype(int) - ROWPAD, 0, nG - 1)
        i1 = np.clip(np.ceil(xhi * nG - 0.5).astype(int) + ROWPAD, 0, nG - 1)
        # greedy set-packing into rounds (128-bit occupancy masks).
        # Images with no in-layer gt keep conf_loss_mask all-True in the
        # reference; skip their schedule so ACC stays at -100 -> ~ign = 1.
        sched = []  # (k, round, p0, p1, step)
        occ = []
        for k2 in sorted(range(gt_n if valid.any() else 0),
                         key=lambda q: (j0[q] - j1[q], q)):
            lo, hi = 2 * int(j0[k2]), 2 * int(j1[k2]) + 2
            if i1[k2] < 32:
                lo, step = lo, 2          # even partitions only
            elif i0[k2] >= 32:
                lo, step = lo + 1, 2      # odd partitions only
            else:
                step = 1
            mask = 0
            for p in range(lo, hi, step):
                mask |= 1 << p
            for r, o in enumerate(occ):
                if not (o & mask):
                    occ[r] |= mask
                    sched.append((k2, r, lo, hi, step))
                    break
            else:
                occ.append(mask)
                sched.append((k2, len(occ) - 1, lo, hi, step))
        per_img.append(dict(K=gt_n, gt=gt, cellmap=cellmap,
                            has_valid=bool(valid.any()),
                            sched=sched, R=len(occ)))
    return per_img


def _plan(labels, anchors_all, img_size):
    per_img = _host_precompute(labels, anchors_all, img_size)
    Rs = [info["R"] for info in per_img]
    order = sorted(range(B), key=lambda i: -Rs[i])
    A_imgs = order[:N_CORES]
    B_imgs = order[N_CORES:][::-1]  # pair big-A with small-B
    RA = max(max((Rs[i] for i in A_imgs), default=0), 1)
    RB = max(max((Rs[i] for i in B_imgs), default=0), 1)
    NA = -(-RA // G_R)
    NB = -(-RB // G_R)
    NGmax = 1
    for c in range(N_CORES):
        n = (len(per_img[A_imgs[c]]["cellmap"])
             + len(per_img[B_imgs[c]]["cellmap"]))
        NGmax = max(NGmax, n)
    NGmax = min(-(-NGmax // 8) * 8, P)
    return per_img, A_imgs, B_imgs, NA, NB, NGmax


# ---------------------------------------------------------------------------
# per-core input packing
# ---------------------------------------------------------------------------

def _pack_core_inputs(core, per_img, A_imgs, B_imgs, raw, img_size,
                      NA, NB, NGmax):
    img_size = f32(img_size)
    thre = f32(IGNORE_THRE)
    cthre = (thre / (f32(1.0) + thre)).astype(f32)
    imgs = [A_imgs[core], B_imgs[core]]
    SA, SB = NA * G_R, NB * G_R

    # full-channel raw (gather source) + pre-packed 5-channel bf16 block
    rawsh = np.ascontiguousarray(raw[imgs]).reshape(2, 255, NCELL)
    # raw5 [P, 960]: col = im*480 + ch*96 + a*32 + c ; cell q = 32p + c
    r6 = rawsh.reshape(2, nA, 85, P, 32)[:, :, 0:5]       # im,a,ch,p,c
    raw5 = np.ascontiguousarray(
        r6.transpose(3, 0, 2, 1, 4).reshape(P, 960)).astype(bf16)

    # scan consts (bf16): per-image g5 blocks + xyc
    # g5 block for image im: col = comp*(S*DUP) + slot*DUP + dup,
    # per-partition values from the round schedule.
    blocks = []
    for sl, im in zip((SA, SB), imgs):
        info = per_img[im]
        g5 = np.zeros((5, sl, P, DUP), f32)
        g5[4] = 1.0  # pad: prod - 1 <= 0 always (boxes within [0,1])
        gt = info["gt"]
        for k, r, p0, p1, step in info["sched"]:
            sl_ = slice(p0, p1, step)
            g5[0, r, sl_] = gt["tlx"][k]
            g5[1, r, sl_] = gt["tly"][k]
            g5[2, r, sl_] = gt["brx"][k]
            g5[3, r, sl_] = gt["bry"][k]
            g5[4, r, sl_] = cthre * (gt["area"][k] + f32(EPS))
        blocks.append(g5.transpose(2, 0, 1, 3).reshape(P, 5 * sl * DUP))

    # xyc [P,192]: col = comp*96 + aq, cell q = 32p + (aq % 32)
    pidx = np.arange(P)[:, None]
    aqidx = np.arange(96)[None, :]
    q = 32 * pidx + (aqidx % 32)
    gx = (q % nG).astype(f32)
    gy = (q // nG).astype(f32)
    xyc = np.concatenate([(gx + f32(0.5)) / f32(nG), (gy + f32(0.5)) / f32(nG)],
                         axis=1).astype(f32)
    consts_bf = np.ascontiguousarray(
        np.concatenate(blocks + [xyc], axis=1)).astype(bf16)

    # tail consts: ngm = 1 - gtmask (bf16); f32: tgt85, onehot, validng, gidx
    gtmask = np.zeros((P, 192), f32)
    cells = []
    for iml, im in enumerate(imgs):
        info = per_img[im]
        for (a, j, i), d in info["cellmap"].items():
            cq = j * nG + i
            gtmask[cq // 32, iml * 96 + a * 32 + cq % 32] = 1.0
            cells.append((iml, a, cq, d["tgt"], d["cls"]))
    ngm = np.ascontiguousarray(1.0 - gtmask).astype(bf16)

    tgt85 = np.zeros((P, 85), f32)
    onehot = np.zeros((P, 85), f32)
    validng = np.zeros((P, 85), f32)
    gidx = np.zeros((P, NGmax), np.int32)
    for g, (iml, a, cq, tgt, clsset) in enumerate(cells):
        tgt85[g, 0:4] = tgt
        onehot[g, 4] = 1.0
        for c in clsset:
            onehot[g, 5 + c] = 1.0
        validng[g, :] = 1.0
        gidx[:, g] = (iml * 255 + a * 85) * NCELL + cq
    consts_f = np.ascontiguousarray(np.concatenate(
        [tgt85, onehot, validng, gidx.view(f32)], axis=1)).astype(f32)
    return dict(rawsh=rawsh, raw5=raw5, cbf=consts_bf, cf=consts_f, ngm=ngm)


# ---------------------------------------------------------------------------
# device program
# ---------------------------------------------------------------------------

def _build_program(NA, NB, NGmax, anchors_all, img_size):
    import concourse.bass as bass
    import concourse.mybir as mybir
    from concourse.tile import TileContext

    dtb = mybir.dt.bfloat16
    dtf = mybir.dt.float32
    AF = mybir.ActivationFunctionType
    OP = mybir.AluOpType
    AX = mybir.AxisListType
    cthre = float(IGNORE_THRE / (1.0 + IGNORE_THRE))
    SA, SB = NA * G_R, NB * G_R
    W = G_R * 96

    nc = bass.Bass()

    lnaw_vals = [float(math.log(anchors_all[a][0] / img_size)) for a in range(nA)]

    rawsh = nc.declare_dram_parameter("rawsh", [2, 255, NCELL], dtf, False)
    raw5d = nc.declare_dram_parameter("raw5", [P, 960], dtb, False)
    CWS = 5 * (SA + SB) * DUP + 192
    off_g5 = [0, 5 * SA * DUP]
    off_xyc = 5 * (SA + SB) * DUP
    cbf = nc.declare_dram_parameter("cbf", [P, CWS], dtb, False)
    CWF = 85 * 3 + NGmax
    off_tgt, off_oh, off_vn = 0, 85, 170
    off_gidx = 255
    cf = nc.declare_dram_parameter("cf", [P, CWF], dtf, False)
    ngmd = nc.declare_dram_parameter("ngm", [P, 192], dtb, False)
    out = nc.declare_dram_parameter("out", [P, 4], dtf, True)

    def A(t, offset, dims):
        h = t.tensor if hasattr(t, "tensor") else t
        return bass.AP(h, offset, dims)

    with TileContext(nc) as tc, \
            tc.tile_pool(name="main", bufs=1) as pool:
        RAW = pool.tile([P, 960], dtb, name="RAW")
        CS = pool.tile([P, CWS], dtb, name="CS")
        CF = pool.tile([P, CWF], dtf, name="CF")
        E = pool.tile([P, 768], dtb, name="E")
        TL = pool.tile([P, 384], dtb, name="TL")
        BR = pool.tile([P, 384], dtb, name="BR")
        WH = pool.tile([P, 384], dtb, name="WH")
        AREA = pool.tile([P, 192], dtb, name="AREA")
        ATH = pool.tile([P, 192], dtb, name="ATH")
        LNAW = pool.tile([P, 4], dtf, name="LNAW")
        IX = pool.tile([P, W], dtb, name="IX")
        AXT = pool.tile([P, W], dtb, name="AXT")
        IY = pool.tile([P, W], dtb, name="IY")
        AY = pool.tile([P, W], dtb, name="AY")
        IW = pool.tile([P, W], dtb, name="IW")
        IH = pool.tile([P, W], dtb, name="IH")
        PROD = pool.tile([P, W], dtb, name="PROD")
        T = [pool.tile([P, W], dtb, name=f"T{i}") for i in range(2)]
        TR1 = pool.tile([P, W // 2], dtb, name="TR1")
        TR2 = pool.tile([P, W // 4], dtb, name="TR2")
        ACC = pool.tile([P, 192], dtb, name="ACC")
        NGM = pool.tile([P, 192], dtb, name="NGM")
        MASK = pool.tile([P, 192], dtb, name="MASK")
        MEXCL = pool.tile([P, 192], dtb, name="MEXCL")
        SP = pool.tile([P, 192], dtf, name="SP")
        SPA = pool.tile([P, 192], dtf, name="SPA")
        SPB = pool.tile([P, 192], dtf, name="SPB")
        SPM = pool.tile([P, 192], dtf, name="SPM")
        GT85 = pool.tile([P, 85], dtf, name="GT85")
        U = pool.tile([P, 85], dtf, name="U")
        SPC = pool.tile([P, 85], dtf, name="SPC")
        OC = pool.tile([P, 85], dtf, name="OC")
        SPD = pool.tile([P, 85], dtf, name="SPD")
        OUTS = pool.tile([P, 4], dtf, name="OUTS")

        # ---- input loads.  The sync (SP) DGE queue is by far the fastest;
        # put the critical-path loads there in need-order.  CF/NGM (tail
        # consumers) trickle in on the slower gpsimd queue.  raw5 is split
        # per image so image A's pred prep starts ~2us earlier.
        nc.sync.dma_start(out=CS[:], in_=cbf[:])
        nc.sync.dma_start(out=RAW[:, 0:480], in_=A(raw5d, 0, [[960, P], [1, 480]]))
        nc.sync.dma_start(out=RAW[:, 480:960],
                          in_=A(raw5d, 480, [[960, P], [1, 480]]))
        nc.gpsimd.dma_start(out=CF[:], in_=cf[:])
        nc.gpsimd.dma_start(out=NGM[:], in_=ngmd[:])
        for a in range(nA):
            nc.gpsimd.memset(LNAW[:, a:a + 1], lnaw_vals[a])

        # ---- sparse gt-cell gather (gpsimd queue, before batch trees; its
        # consumers are emitted after the scan so they don't stall DVE/ACT) ----
        nc.vector.memset(OUTS[:], 0.0)
        nc.gpsimd.indirect_dma_start(
            out=GT85[0:NGmax, 0:85],
            out_offset=None,
            in_=A(rawsh, 0, [[1, (2 * 255 - 85 + 1) * NCELL], [NCELL, 85], [1, 1]]),
            in_offset=bass.IndirectOffsetOnAxis(
                ap=A(CF, off_gidx, [[CWF, 1], [1, NGmax]]).bitcast(
                    mybir.dt.int32),
                axis=0),
        )

        # ---- pred prep (per image, so image A's scan starts before image
        # B's raw half lands) + banded iou scan ----
        nc.vector.memset(ACC[:], -100.0)
        treng = nc.gpsimd if GPSIMD_TREE else nc.vector
        xyc_b = A(CS, off_xyc, [[CWS, P], [96, 2], [1, 96]])

        def gt_ap(im, comp, s0, n=G_R):
            base = off_g5[im] + comp * ((SA, SB)[im] * DUP) + s0 * DUP
            return A(CS, base, [[CWS, P], [DUP, n], [0, 96 // DUP], [1, DUP]])

        def pred_ap(t, comp, im, n=G_R):
            return A(t, comp * 192 + im * 96, [[384, P], [0, n], [1, 96]])

        for im, nbat in ((0, NA), (1, NB)):
            for a in range(3):
                nc.scalar.activation(
                    out=A(E, im * 384 + a * 32, [[768, P], [96, 4], [1, 32]]),
                    in_=A(RAW, im * 480 + a * 32, [[960, P], [96, 4], [1, 32]]),
                    func=AF.Exp,
                    bias=A(LNAW, a, [[4, P], [1, 1]]),
                )
            e_lt = A(E, im * 384, [[768, P], [96, 2], [1, 96]])
            e_rb = A(E, im * 384 + 192, [[768, P], [96, 2], [1, 96]])
            quad = [[384, P], [192, 2], [1, 96]]
            nc.vector.tensor_tensor(out=A(TL, im * 96, quad), in0=xyc_b,
                                    in1=e_lt, op=OP.subtract)
            nc.vector.tensor_tensor(out=A(BR, im * 96, quad), in0=xyc_b,
                                    in1=e_rb, op=OP.add)
            nc.vector.tensor_tensor(out=A(WH, im * 96, quad), in0=e_lt,
                                    in1=e_rb, op=OP.add)
            sl96 = slice(im * 96, im * 96 + 96)
            nc.vector.tensor_tensor(out=AREA[:, sl96], in0=WH[:, sl96],
                                    in1=WH[:, 192 + im * 96:192 + im * 96 + 96],
                                    op=OP.mult)
            nc.vector.tensor_scalar(out=ATH[:, sl96], in0=AREA[:, sl96],
                                    scalar1=cthre, scalar2=None, op0=OP.mult)
            for bi in range(nbat):
                s0 = bi * G_R
                tt = T[bi % 2]
                nc.vector.tensor_tensor(out=IX[:], in0=pred_ap(TL, 0, im),
                                        in1=gt_ap(im, 0, s0), op=OP.max)
                nc.vector.tensor_tensor(out=AXT[:], in0=pred_ap(BR, 0, im),
                                        in1=gt_ap(im, 2, s0), op=OP.min)
                nc.vector.tensor_tensor(out=IW[:], in0=AXT[:], in1=IX[:],
                                        op=OP.subtract)
                nc.vector.tensor_tensor(out=IY[:], in0=pred_ap(TL, 1, im),
                                        in1=gt_ap(im, 1, s0), op=OP.max)
                nc.vector.tensor_tensor(out=AY[:], in0=pred_ap(BR, 1, im),
                                        in1=gt_ap(im, 3, s0), op=OP.min)
                nc.vector.tensor_tensor(out=IH[:], in0=AY[:], in1=IY[:],
                                        op=OP.subtract)
                # clamp ih only: iw<0 or ih<0 both give prod <= 0 < ath
                nc.vector.tensor_scalar(out=IH[:], in0=IH[:], scalar1=0.0,
                                        scalar2=None, op0=OP.max)
                nc.vector.tensor_tensor(out=PROD[:], in0=IW[:], in1=IH[:],
                                        op=OP.mult)
                nc.vector.tensor_tensor(out=tt[:], in0=PROD[:],
                                        in1=gt_ap(im, 4, s0), op=OP.subtract)
                # tree-max margins over rounds -> ACC (GpSimd)
                treng.tensor_tensor(out=TR1[:], in0=tt[:, 0:W // 2],
                                    in1=tt[:, W // 2:W], op=OP.max)
                treng.tensor_tensor(out=TR2[:], in0=TR1[:, 0:W // 4],
                                    in1=TR1[:, W // 4:W // 2], op=OP.max)
                acc_sl = ACC[:, im * 96:(im + 1) * 96]
                treng.tensor_tensor(out=TR2[:, 0:96], in0=TR2[:, 0:96],
                                    in1=TR2[:, 96:192], op=OP.max)
                treng.tensor_tensor(out=acc_sl, in0=acc_sl, in1=TR2[:, 0:96],
                                    op=OP.max)
            # notign = (max margin <= athp) for this image
            nc.vector.tensor_tensor(out=MASK[:, im * 96:(im + 1) * 96],
                                    in0=ACC[:, im * 96:(im + 1) * 96],
                                    in1=ATH[:, im * 96:(im + 1) * 96],
                                    op=OP.is_le)

        # conf softplus on ACT while DVE scans
        conf_view = A(RAW, 384, [[960, P], [480, 2], [1, 96]])
        sp_flat = [[192, P], [96, 2], [1, 96]]
        nc.scalar.activation(out=A(SPA, 0, sp_flat), in_=conf_view, func=AF.Abs)
        nc.scalar.activation(out=SPB[:], in_=SPA[:], func=AF.Exp, scale=-1.0)
        nc.scalar.activation(out=SPA[:], in_=SPB[:], func=AF.Ln, bias=1.0)
        nc.scalar.activation(out=A(SPB, 0, sp_flat), in_=conf_view, func=AF.Relu)

        # ---- sparse gt-cell terms (consumers of the early gather) ----
        # softplus(z) = ln(1 + exp(-|z|)) + relu(z) over cols 4..85
        nc.scalar.activation(out=SPC[0:NGmax, 4:85], in_=GT85[0:NGmax, 4:85],
                             func=AF.Abs)
        nc.scalar.activation(out=SPD[0:NGmax, 4:85], in_=SPC[0:NGmax, 4:85],
                             func=AF.Exp, scale=-1.0)
        nc.scalar.activation(out=SPC[0:NGmax, 4:85], in_=SPD[0:NGmax, 4:85],
                             func=AF.Ln, bias=1.0)
        nc.scalar.activation(out=SPD[0:NGmax, 4:85], in_=GT85[0:NGmax, 4:85],
                             func=AF.Relu)
        nc.vector.tensor_tensor(out=OC[0:NGmax, 4:85], in0=GT85[0:NGmax, 4:85],
                                in1=A(CF, off_oh + 4, [[CWF, NGmax], [1, 81]]),
                                op=OP.mult)
        nc.vector.tensor_tensor(out=U[0:NGmax, 4:85], in0=SPC[0:NGmax, 4:85],
                                in1=SPD[0:NGmax, 4:85], op=OP.add)
        nc.vector.tensor_tensor(out=U[0:NGmax, 4:85], in0=U[0:NGmax, 4:85],
                                in1=OC[0:NGmax, 4:85], op=OP.subtract)
        # bbox: (ltrb_raw - tgt)^2 in cols 0..4
        nc.vector.tensor_tensor(out=OC[0:NGmax, 0:4], in0=GT85[0:NGmax, 0:4],
                                in1=A(CF, off_tgt, [[CWF, NGmax], [1, 4]]),
                                op=OP.subtract)
        nc.scalar.activation(out=U[0:NGmax, 0:4], in_=OC[0:NGmax, 0:4],
                             func=AF.Square)
        nc.vector.tensor_tensor(out=U[0:NGmax, :], in0=U[0:NGmax, :],
                                in1=A(CF, off_vn, [[CWF, NGmax], [1, 85]]),
                                op=OP.mult)
        nc.vector.reduce_sum(out=OUTS[0:NGmax, 2:3], in_=U[0:NGmax, :],
                             axis=AX.X)

        # ---- dense conf loss: MEXCL = ~ign * (1 - gtmask) ----
        nc.vector.tensor_tensor(out=MEXCL[:], in0=MASK[:], in1=NGM[:],
                                op=OP.mult)
        nc.vector.tensor_tensor(out=SP[:], in0=SPA[:], in1=SPB[:], op=OP.add)
        nc.vector.tensor_tensor(out=SPM[:], in0=SP[:], in1=MEXCL[:], op=OP.mult)
        nc.vector.reduce_sum(out=OUTS[:, 0:1], in_=SPM[:], axis=AX.X)

        nc.sync.dma_start(out=out[:], in_=OUTS[:])

    return nc


_CACHE = {}
TRACE = False
LAST_RESULTS = None


def _split_multiwait(nc):
    """Walrus codegen on this toolchain supports only one sync-wait command
    per instruction; split multi-wait instructions into single-wait NOPs on
    the same engine."""
    import concourse.mybir as mybir

    if getattr(nc, "_fcos_wait_split", False):
        return
    nc._fcos_wait_split = True
    for bb in nc.m.functions[0].blocks:
        insts = bb.instructions
        for ins in list(insts):
            si = ins.sync_info
            if si is not None and len(si.on_wait) > 1:
                waits = list(si.on_wait)
                idx = insts.index(ins)
                nops = []
                for j, w in enumerate(waits[:-1]):
                    nop = mybir.InstNoOp(name=f"{ins.name}-wsplit{j}", ins=[],
                                         outs=[])
                    nop.engine = ins.engine
                    nop.sync_info = mybir.SyncInfo(on_wait=[w], on_update=[])
                    nops.append(nop)
                ins.sync_info = mybir.SyncInfo(on_wait=[waits[-1]],
                                               on_update=list(si.on_update))
                for nop in reversed(nops):
                    insts.insert(idx, nop)


def kernel(raw, labels, anchors_all, img_size):
    from concourse.bass_utils import run_bass_kernel_spmd

    raw = np.asarray(raw, f32)
    labels_np = np.asarray(labels, f32)
    anchors_np = np.asarray(anchors_all, f32)
    isize = int(img_size)

    per_img, A_imgs, B_imgs, NA, NB, NGmax = _plan(labels_np, anchors_np, isize)
    key = (NA, NB, NGmax, G_R, DUP, GPSIMD_TREE, anchors_np.tobytes(), isize)
    if key not in _CACHE:
        _CACHE[key] = _build_program(NA, NB, NGmax, anchors_np.tolist(), isize)
    nc = _CACHE[key]
    _split_multiwait(nc)

    in_maps = [
        _pack_core_inputs(c, per_img, A_imgs, B_imgs, raw, isize, NA, NB, NGmax)
        for c in range(N_CORES)
    ]
    global LAST_RESULTS
    res = run_bass_kernel_spmd(nc, in_maps, list(range(N_CORES)), trace=TRACE)
    LAST_RESULTS = res
    total = np.float64(0.0)
    for c in range(N_CORES):
        o = res.results[c]["out"]
        total += np.sum(o[:, 0], dtype=np.float64)
        total += np.sum(o[:, 2], dtype=np.float64)
    return f32(total)


if __name__ == "__main__":
    import importlib.util

    spec = importlib.util.spec_from_file_location("reference",
                                                  "/root/problem/reference.py")
    ref = importlib.util.module_from_spec(spec)
    spec.loader.exec_module(ref)
    inputs = ref.setup_inputs()
    np_inputs = {k: np.asarray(v) for k, v in inputs.items()}
    got = kernel(**np_inputs)
    print("kernel:", got)


# revision 35
# speedup vs baseline: 3.6653x; 1.2176x over previous
"""Trainium2 Bass kernel for the YOLO/FCOS-layer loss (nn_FCOSLayer_22840636080477).

Sharding: data-parallel over batch, 2 images per NeuronCore x 8 cores, one
SPMD program. Host does label-side preprocessing (anchor matching, scatter
dedup, row-band gt->partition scheduling, constant packing); device does
everything that touches `raw`:

  loss = sum_cells softplus(conf) * (conf_mask & ~gt)          (dense)
       + sum_gtcells [ softplus(conf)-conf                      (sparse, gather)
                      + sum_c (softplus(cls_c) - onehot_c*cls_c)
                      + sum_4 (ltrb_raw - tgt)^2 ]

The ignore mask needs a max-IoU scan of 12288 pred boxes against each
image's gt boxes.  Three structural tricks make it cheap:

1. bf16 + packed last-axis APs -> DVE 2x_1p mode (0.52 ns/elem).
2. Row banding: IoU > 0.6 forces the pred's cell center inside the gt
   box dilated by (2/3)*(wg,hg) (pred-independent bound), so each gt
   only concerns a contiguous band of partitions (partition p holds
   row p//2).  Interval-coloring packs gts into "rounds"; each round
   instruction gives every partition its own gt via per-partition
   consts.  Rounds ~ max-load instead of K.
3. Margin m = min(iw*ih - athg, iw) has the exact sign of the ignore
   condition (valid when gt heights <= 1), killing both clamps; the
   per-pred threshold athp = cthre*areap is compared once at the end:
   ignore <=> max_g m_g > athp.
"""
import sys
import math
import numpy as np

sys.path.insert(0, "/opt/trn_rl_repo")

import ml_dtypes

bf16 = ml_dtypes.bfloat16

N_CLS = 80
nA = 3
STRIDE = 8
IGNORE_THRE = 0.6
EPS = 1e-16
B = 16
K = 50
nG = 64
N_CORES = 8
P = 128
NCELL = nG * nG
f32 = np.float32

# tunables
DUP = 2          # duplicate gt scalars pairwise (bf16 2x packing aid)
DIL = 0.72       # band dilation factor (theory: (1/tau'-1) ~ 0.692 w/ bf16)
GPSIMD_TREE = False  # Pool engine rejects TENSOR_TENSOR (ISA check)
# batched-op sizes (slots per scan op); per-slot ns cost for the DP
G_COST = {16: 10600, 8: 5480, 4: 4510}


def _decompose(R):
    """Split R rounds into batch widths from G_COST minimizing total cost."""
    if R <= 0:
        return []
    best = {0: (0, ())}
    for s in range(1, R + 16):
        cands = []
        for g, c in G_COST.items():
            if s - g >= 0 and (s - g) in best:
                pc, pl = best[s - g]
                cands.append((pc + c, pl + (g,)))
        if cands:
            best[s] = min(cands)
    return list(min(best[s] for s in best if s >= R)[1])


# ---------------------------------------------------------------------------
# host-side label math (replicates reference.py semantics in f32 numpy)
# ---------------------------------------------------------------------------

def _host_precompute(labels, anchors_all, img_size):
    labels = np.asarray(labels, f32)
    anchors_all = np.asarray(anchors_all, f32)
    img_size = f32(img_size)
    anchors = anchors_all[:nA]
    norm_anch = anchors_all / img_size
    anch_w_n = anchors[:, 0] / img_size

    per_img = []
    for bb in range(B):
        lab = labels[bb]
        valid_row = lab.sum(-1) > 0
        tw, th = lab[:, 3], lab[:, 4]
        inter = np.minimum(tw[:, None], norm_anch[:, 0]) * np.minimum(
            th[:, None], norm_anch[:, 1]
        )
        union = tw[:, None] * th[:, None] + norm_anch[:, 0] * norm_anch[:, 1] - inter
        an_iou = inter / (union + f32(EPS))
        best_n_all = np.argmax(an_iou, axis=-1)
        best_n = best_n_all % nA
        valid = valid_row & (best_n_all < nA)

        ks = np.where(valid_row)[0]
        gcx, gcy, gw, gh = lab[ks, 1], lab[ks, 2], lab[ks, 3], lab[ks, 4]
        gt = dict(
            tlx=(gcx - gw / 2).astype(f32),
            tly=(gcy - gh / 2).astype(f32),
            brx=(gcx + gw / 2).astype(f32),
            bry=(gcy + gh / 2).astype(f32),
            area=(gw * gh).astype(f32),
            gh=gh.astype(f32),
        )

        tx = lab[:, 1] * nG
        ty = lab[:, 2] * nG
        ti = tx.astype(np.int32)
        tj = ty.astype(np.int32)
        tcls = lab[:, 0].astype(np.int32)
        lw, lh = lab[:, 3] * nG, lab[:, 4] * nG
        xc = np.floor(tx) + f32(0.5)
        yc = np.floor(ty) + f32(0.5)
        lab_ltrb = (
            np.maximum(
                np.stack(
                    [xc - (tx - lw / 2), yc - (ty - lh / 2),
                     (tx + lw / 2) - xc, (ty + lh / 2) - yc], -1),
                0.0,
            ) / f32(nG)
        ).astype(f32)
        cellmap = {}
        for k in range(K):
            if not valid[k]:
                continue
            key = (int(best_n[k]), int(tj[k]), int(ti[k]))
            tgt = np.log(lab_ltrb[k] / anch_w_n[best_n[k]] + f32(EPS)).astype(f32)
            if key not in cellmap:
                cellmap[key] = dict(tgt=tgt, cls=set([int(tcls[k])]))
            else:
                cellmap[key]["tgt"] = tgt  # scatter last-wins
                cellmap[key]["cls"].add(int(tcls[k]))

        # row-banded partition sets for the iou scan.  IoU > 0.6 forces the
        # cell center inside the gt box dilated by (2/3)*(wg,hg); partition
        # p holds row p//2, half p%2 (i<32 on even p, i>=32 on odd p).
        gt_n = len(ks)
        gw64 = (gt["brx"] - gt["tlx"]).astype(np.float64)
        gh64 = gt["gh"].astype(np.float64)
        ylo = gt["tly"].astype(np.float64) - DIL * gh64
        yhi = gt["bry"].astype(np.float64) + DIL * gh64
        xlo = gt["tlx"].astype(np.float64) - DIL * gw64
        xhi = gt["brx"].astype(np.float64) + DIL * gw64
        # extra pad row only for small boxes, where the DIL slack over the
        # theoretical 0.692 factor is below the bf16 coordinate noise
        rp_y = (gh64 < 0.1).astype(int)
        rp_x = (gw64 < 0.1).astype(int)
        j0 = np.clip(np.floor(ylo * nG - 0.5).astype(int) - rp_y, 0, nG - 1)
        j1 = np.clip(np.ceil(yhi * nG - 0.5).astype(int) + rp_y, 0, nG - 1)
        i0 = np.clip(np.floor(xlo * nG - 0.5).astype(int) - rp_x, 0, nG - 1)
        i1 = np.clip(np.ceil(xhi * nG - 0.5).astype(int) + rp_x, 0, nG - 1)
        # greedy set-packing into rounds (128-bit occupancy masks).
        # Images with no in-layer gt keep conf_loss_mask all-True in the
        # reference; skip their schedule so ACC stays at -100 -> ~ign = 1.
        sched = []  # (k, round, p0, p1, step)
        occ = []
        for k2 in sorted(range(gt_n if valid.any() else 0),
                         key=lambda q: (j0[q] - j1[q], q)):
            lo, hi = 2 * int(j0[k2]), 2 * int(j1[k2]) + 2
            if i1[k2] < 32:
                lo, step = lo, 2          # even partitions only
            elif i0[k2] >= 32:
                lo, step = lo + 1, 2      # odd partitions only
            else:
                step = 1
            mask = 0
            for p in range(lo, hi, step):
                mask |= 1 << p
            for r, o in enumerate(occ):
                if not (o & mask):
                    occ[r] |= mask
                    sched.append((k2, r, lo, hi, step))
                    break
            else:
                occ.append(mask)
                sched.append((k2, len(occ) - 1, lo, hi, step))
        per_img.append(dict(K=gt_n, gt=gt, cellmap=cellmap,
                            has_valid=bool(valid.any()),
                            sched=sched, R=len(occ)))
    return per_img


def _plan(labels, anchors_all, img_size):
    per_img = _host_precompute(labels, anchors_all, img_size)
    Rs = [info["R"] for info in per_img]
    order = sorted(range(B), key=lambda i: -Rs[i])
    A_imgs = order[:N_CORES]
    B_imgs = order[N_CORES:][::-1]  # pair big-A with small-B
    RA = max((Rs[i] for i in A_imgs), default=0)
    RB = max((Rs[i] for i in B_imgs), default=0)
    GA = tuple(_decompose(max(RA, 1)))
    GB = tuple(_decompose(RB))  # empty when no B image has in-layer gts
    NGmax = 1
    for c in range(N_CORES):
        n = (len(per_img[A_imgs[c]]["cellmap"])
             + len(per_img[B_imgs[c]]["cellmap"]))
        NGmax = max(NGmax, n)
    NGmax = min(-(-NGmax // 8) * 8, P)
    return per_img, A_imgs, B_imgs, GA, GB, NGmax


# ---------------------------------------------------------------------------
# per-core input packing
# ---------------------------------------------------------------------------

def _pack_core_inputs(core, per_img, A_imgs, B_imgs, raw, img_size,
                      GA, GB, NGmax):
    img_size = f32(img_size)
    thre = f32(IGNORE_THRE)
    cthre = (thre / (f32(1.0) + thre)).astype(f32)
    imgs = [A_imgs[core], B_imgs[core]]
    SA, SB = sum(GA), sum(GB)

    # full-channel raw (gather source) + pre-packed 5-channel bf16 block
    rawsh = np.ascontiguousarray(raw[imgs]).reshape(2, 255, NCELL)
    # raw5 [P, 960]: col = im*480 + ch*96 + a*32 + c ; cell q = 32p + c
    r6 = rawsh.reshape(2, nA, 85, P, 32)[:, :, 0:5]       # im,a,ch,p,c
    raw5 = np.ascontiguousarray(
        r6.transpose(3, 0, 2, 1, 4).reshape(P, 960)).astype(bf16)

    # scan consts (bf16): per-image g5 blocks + xyc
    # g5 block for image im: col = comp*(S*DUP) + slot*DUP + dup,
    # per-partition values from the round schedule.
    blocks = []
    for sl, im in zip((SA, SB), imgs):
        info = per_img[im]
        g5 = np.zeros((5, sl, P, DUP), f32)
        g5[4] = 1.0  # pad: prod - 1 <= 0 always (boxes within [0,1])
        gt = info["gt"]
        for k, r, p0, p1, step in info["sched"]:
            sl_ = slice(p0, p1, step)
            g5[0, r, sl_] = gt["tlx"][k]
            g5[1, r, sl_] = gt["tly"][k]
            g5[2, r, sl_] = gt["brx"][k]
            g5[3, r, sl_] = gt["bry"][k]
            g5[4, r, sl_] = cthre * (gt["area"][k] + f32(EPS))
        blocks.append(g5.transpose(2, 0, 1, 3).reshape(P, 5 * sl * DUP))

    # xyc [P,192]: col = comp*96 + aq, cell q = 32p + (aq % 32)
    pidx = np.arange(P)[:, None]
    aqidx = np.arange(96)[None, :]
    q = 32 * pidx + (aqidx % 32)
    gx = (q % nG).astype(f32)
    gy = (q // nG).astype(f32)
    xyc = np.concatenate([(gx + f32(0.5)) / f32(nG), (gy + f32(0.5)) / f32(nG)],
                         axis=1).astype(f32)
    consts_bf = np.ascontiguousarray(
        np.concatenate(blocks + [xyc], axis=1)).astype(bf16)

    # tail consts: ngm = 1 - gtmask (bf16); f32: tgt85, onehot, validng, gidx
    gtmask = np.zeros((P, 192), f32)
    cells = []
    for iml, im in enumerate(imgs):
        info = per_img[im]
        for (a, j, i), d in info["cellmap"].items():
            cq = j * nG + i
            gtmask[cq // 32, iml * 96 + a * 32 + cq % 32] = 1.0
            cells.append((iml, a, cq, d["tgt"], d["cls"]))
    ngm = np.ascontiguousarray(1.0 - gtmask).astype(bf16)

    tgt85 = np.zeros((P, 85), f32)
    onehot = np.zeros((P, 85), f32)
    validng = np.zeros((P, 85), f32)
    gidx = np.zeros((P, NGmax), np.int32)
    for g, (iml, a, cq, tgt, clsset) in enumerate(cells):
        tgt85[g, 0:4] = tgt
        onehot[g, 4] = 1.0
        for c in clsset:
            onehot[g, 5 + c] = 1.0
        validng[g, :] = 1.0
        gidx[:, g] = (iml * 255 + a * 85) * NCELL + cq
    consts_f = np.ascontiguousarray(np.concatenate(
        [tgt85, onehot, validng, gidx.view(f32)], axis=1)).astype(f32)
    return dict(rawsh=rawsh, raw5=raw5, cbf=consts_bf, cf=consts_f, ngm=ngm)


# ---------------------------------------------------------------------------
# device program
# ---------------------------------------------------------------------------

def _build_program(GA, GB, NGmax, anchors_all, img_size):
    import concourse.bass as bass
    import concourse.mybir as mybir
    from concourse.tile import TileContext

    dtb = mybir.dt.bfloat16
    dtf = mybir.dt.float32
    AF = mybir.ActivationFunctionType
    OP = mybir.AluOpType
    AX = mybir.AxisListType
    cthre = float(IGNORE_THRE / (1.0 + IGNORE_THRE))
    SA, SB = sum(GA), sum(GB)
    W = max(GA + GB) * 96  # widest batch; tiles are sized for it

    nc = bass.Bass()

    lnaw_vals = [float(math.log(anchors_all[a][0] / img_size)) for a in range(nA)]

    rawsh = nc.declare_dram_parameter("rawsh", [2, 255, NCELL], dtf, False)
    raw5d = nc.declare_dram_parameter("raw5", [P, 960], dtb, False)
    CWS = 5 * (SA + SB) * DUP + 192
    off_g5 = [0, 5 * SA * DUP]
    off_xyc = 5 * (SA + SB) * DUP
    cbf = nc.declare_dram_parameter("cbf", [P, CWS], dtb, False)
    CWF = 85 * 3 + NGmax
    off_tgt, off_oh, off_vn = 0, 85, 170
    off_gidx = 255
    cf = nc.declare_dram_parameter("cf", [P, CWF], dtf, False)
    ngmd = nc.declare_dram_parameter("ngm", [P, 192], dtb, False)
    out = nc.declare_dram_parameter("out", [P, 4], dtf, True)

    def A(t, offset, dims):
        h = t.tensor if hasattr(t, "tensor") else t
        return bass.AP(h, offset, dims)

    with TileContext(nc) as tc, \
            tc.tile_pool(name="main", bufs=1) as pool:
        RAW = pool.tile([P, 960], dtb, name="RAW")
        CS = pool.tile([P, CWS], dtb, name="CS")
        CF = pool.tile([P, CWF], dtf, name="CF")
        E = pool.tile([P, 768], dtb, name="E")
        TL = pool.tile([P, 384], dtb, name="TL")
        BR = pool.tile([P, 384], dtb, name="BR")
        WH = pool.tile([P, 384], dtb, name="WH")
        AREA = pool.tile([P, 192], dtb, name="AREA")
        ATH = pool.tile([P, 192], dtb, name="ATH")
        LNAW = pool.tile([P, 4], dtf, name="LNAW")
        IX = pool.tile([P, W], dtb, name="IX")
        AXT = pool.tile([P, W], dtb, name="AXT")
        IY = pool.tile([P, W], dtb, name="IY")
        AY = pool.tile([P, W], dtb, name="AY")
        IW = pool.tile([P, W], dtb, name="IW")
        IH = pool.tile([P, W], dtb, name="IH")
        PROD = pool.tile([P, W], dtb, name="PROD")
        T = [pool.tile([P, W], dtb, name=f"T{i}") for i in range(2)]
        TR1 = pool.tile([P, W // 2], dtb, name="TR1")
        TR2 = pool.tile([P, W // 4], dtb, name="TR2")
        ACC = pool.tile([P, 192], dtb, name="ACC")
        NGM = pool.tile([P, 192], dtb, name="NGM")
        MASK = pool.tile([P, 192], dtb, name="MASK")
        MEXCL = pool.tile([P, 192], dtb, name="MEXCL")
        SP = pool.tile([P, 192], dtf, name="SP")
        SPA = pool.tile([P, 192], dtf, name="SPA")
        SPB = pool.tile([P, 192], dtf, name="SPB")
        SPM = pool.tile([P, 192], dtf, name="SPM")
        GT85 = pool.tile([P, 85], dtf, name="GT85")
        U = pool.tile([P, 85], dtf, name="U")
        SPC = pool.tile([P, 85], dtf, name="SPC")
        OC = pool.tile([P, 85], dtf, name="OC")
        SPD = pool.tile([P, 85], dtf, name="SPD")
        OUTS = pool.tile([P, 4], dtf, name="OUTS")

        # ---- input loads.  The sync (SP) DGE queue is by far the fastest;
        # put the critical-path loads there in need-order.  CF/NGM (tail
        # consumers) trickle in on the slower gpsimd queue.  raw5 is split
        # per image so image A's pred prep starts ~2us earlier.
        nc.sync.dma_start(out=RAW[:, 0:480], in_=A(raw5d, 0, [[960, P], [1, 480]]))
        nc.sync.dma_start(out=CS[:], in_=cbf[:])
        if GB:
            nc.sync.dma_start(out=RAW[:, 480:960],
                              in_=A(raw5d, 480, [[960, P], [1, 480]]))
        else:
            # image B contributes only its conf channel (no scan)
            nc.sync.dma_start(out=RAW[:, 864:960],
                              in_=A(raw5d, 864, [[960, P], [1, 96]]))
        nc.gpsimd.dma_start(out=CF[:], in_=cf[:])
        nc.gpsimd.dma_start(out=NGM[:], in_=ngmd[:])
        for a in range(nA):
            nc.gpsimd.memset(LNAW[:, a:a + 1], lnaw_vals[a])

        # ---- sparse gt-cell gather (gpsimd queue, before batch trees; its
        # consumers are emitted after the scan so they don't stall DVE/ACT) ----
        nc.vector.memset(OUTS[:], 0.0)
        nc.gpsimd.indirect_dma_start(
            out=GT85[0:NGmax, 0:85],
            out_offset=None,
            in_=A(rawsh, 0, [[1, (2 * 255 - 85 + 1) * NCELL], [NCELL, 85], [1, 1]]),
            in_offset=bass.IndirectOffsetOnAxis(
                ap=A(CF, off_gidx, [[CWF, 1], [1, NGmax]]).bitcast(
                    mybir.dt.int32),
                axis=0),
        )

        # ---- pred prep (per image, so image A's scan starts before image
        # B's raw half lands) + banded iou scan ----
        nc.vector.memset(ACC[:], -100.0)
        treng = nc.gpsimd if GPSIMD_TREE else nc.vector
        xyc_b = A(CS, off_xyc, [[CWS, P], [96, 2], [1, 96]])

        def gt_ap(im, comp, s0, n):
            base = off_g5[im] + comp * ((SA, SB)[im] * DUP) + s0 * DUP
            return A(CS, base, [[CWS, P], [DUP, n], [0, 96 // DUP], [1, DUP]])

        def pred_ap(t, comp, im, n):
            return A(t, comp * 192 + im * 96, [[384, P], [0, n], [1, 96]])

        for im, glist in ((0, GA), (1, GB)):
            if not glist:
                # no in-layer gts on this slot: mask stays all-pass
                nc.vector.memset(MASK[:, im * 96:(im + 1) * 96], 1.0)
                continue
            for a in range(3):
                nc.scalar.activation(
                    out=A(E, im * 384 + a * 32, [[768, P], [96, 4], [1, 32]]),
                    in_=A(RAW, im * 480 + a * 32, [[960, P], [96, 4], [1, 32]]),
                    func=AF.Exp,
                    bias=A(LNAW, a, [[4, P], [1, 1]]),
                )
            e_lt = A(E, im * 384, [[768, P], [96, 2], [1, 96]])
            e_rb = A(E, im * 384 + 192, [[768, P], [96, 2], [1, 96]])
            quad = [[384, P], [192, 2], [1, 96]]
            nc.vector.tensor_tensor(out=A(TL, im * 96, quad), in0=xyc_b,
                                    in1=e_lt, op=OP.subtract)
            nc.vector.tensor_tensor(out=A(BR, im * 96, quad), in0=xyc_b,
                                    in1=e_rb, op=OP.add)
            nc.vector.tensor_tensor(out=A(WH, im * 96, quad), in0=e_lt,
                                    in1=e_rb, op=OP.add)
            sl96 = slice(im * 96, im * 96 + 96)
            nc.vector.tensor_tensor(out=AREA[:, sl96], in0=WH[:, sl96],
                                    in1=WH[:, 192 + im * 96:192 + im * 96 + 96],
                                    op=OP.mult)
            nc.vector.tensor_scalar(out=ATH[:, sl96], in0=AREA[:, sl96],
                                    scalar1=cthre, scalar2=None, op0=OP.mult)
            s0 = 0
            for bi, g in enumerate(glist):
                wb = g * 96
                tt = T[bi % 2]
                nc.vector.tensor_tensor(out=IX[:, 0:wb],
                                        in0=pred_ap(TL, 0, im, g),
                                        in1=gt_ap(im, 0, s0, g), op=OP.max)
                nc.vector.tensor_tensor(out=AXT[:, 0:wb],
                                        in0=pred_ap(BR, 0, im, g),
                                        in1=gt_ap(im, 2, s0, g), op=OP.min)
                nc.vector.tensor_tensor(out=IW[:, 0:wb], in0=AXT[:, 0:wb],
                                        in1=IX[:, 0:wb], op=OP.subtract)
                nc.vector.tensor_tensor(out=IY[:, 0:wb],
                                        in0=pred_ap(TL, 1, im, g),
                                        in1=gt_ap(im, 1, s0, g), op=OP.max)
                nc.vector.tensor_tensor(out=AY[:, 0:wb],
                                        in0=pred_ap(BR, 1, im, g),
                                        in1=gt_ap(im, 3, s0, g), op=OP.min)
                nc.vector.tensor_tensor(out=IH[:, 0:wb], in0=AY[:, 0:wb],
                                        in1=IY[:, 0:wb], op=OP.subtract)
                # clamp ih only: iw<0 or ih<0 both give prod <= 0 < ath
                nc.vector.tensor_scalar(out=IH[:, 0:wb], in0=IH[:, 0:wb],
                                        scalar1=0.0, scalar2=None, op0=OP.max)
                nc.vector.tensor_tensor(out=PROD[:, 0:wb], in0=IW[:, 0:wb],
                                        in1=IH[:, 0:wb], op=OP.mult)
                nc.vector.tensor_tensor(out=tt[:, 0:wb], in0=PROD[:, 0:wb],
                                        in1=gt_ap(im, 4, s0, g),
                                        op=OP.subtract)
                # tree-max margins over rounds -> ACC
                width, src, flip = wb, tt, 0
                while width > 96:
                    h = width // 2
                    dst = (TR1, TR2)[flip % 2]
                    treng.tensor_tensor(out=dst[:, 0:h], in0=src[:, 0:h],
                                        in1=src[:, h:2 * h], op=OP.max)
                    src, width, flip = dst, h, flip + 1
                acc_sl = ACC[:, im * 96:(im + 1) * 96]
                treng.tensor_tensor(out=acc_sl, in0=acc_sl, in1=src[:, 0:96],
                                    op=OP.max)
                s0 += g
            # notign = (max margin <= athp) for this image
            nc.vector.tensor_tensor(out=MASK[:, im * 96:(im + 1) * 96],
                                    in0=ACC[:, im * 96:(im + 1) * 96],
                                    in1=ATH[:, im * 96:(im + 1) * 96],
                                    op=OP.is_le)

        # conf softplus on ACT while DVE scans
        conf_view = A(RAW, 384, [[960, P], [480, 2], [1, 96]])
        sp_flat = [[192, P], [96, 2], [1, 96]]
        nc.scalar.activation(out=A(SPA, 0, sp_flat), in_=conf_view, func=AF.Abs)
        nc.scalar.activation(out=SPB[:], in_=SPA[:], func=AF.Exp, scale=-1.0)
        nc.scalar.activation(out=SPA[:], in_=SPB[:], func=AF.Ln, bias=1.0)
        nc.scalar.activation(out=A(SPB, 0, sp_flat), in_=conf_view, func=AF.Relu)

        # ---- sparse gt-cell terms (consumers of the early gather) ----
        # softplus(z) = ln(1 + exp(-|z|)) + relu(z) over cols 4..85
        nc.scalar.activation(out=SPC[0:NGmax, 4:85], in_=GT85[0:NGmax, 4:85],
                             func=AF.Abs)
        nc.scalar.activation(out=SPD[0:NGmax, 4:85], in_=SPC[0:NGmax, 4:85],
                             func=AF.Exp, scale=-1.0)
        nc.scalar.activation(out=SPC[0:NGmax, 4:85], in_=SPD[0:NGmax, 4:85],
                             func=AF.Ln, bias=1.0)
        nc.scalar.activation(out=SPD[0:NGmax, 4:85], in_=GT85[0:NGmax, 4:85],
                             func=AF.Relu)
        nc.vector.tensor_tensor(out=OC[0:NGmax, 4:85], in0=GT85[0:NGmax, 4:85],
                                in1=A(CF, off_oh + 4, [[CWF, NGmax], [1, 81]]),
                                op=OP.mult)
        nc.vector.tensor_tensor(out=U[0:NGmax, 4:85], in0=SPC[0:NGmax, 4:85],
                                in1=SPD[0:NGmax, 4:85], op=OP.add)
        nc.vector.tensor_tensor(out=U[0:NGmax, 4:85], in0=U[0:NGmax, 4:85],
                                in1=OC[0:NGmax, 4:85], op=OP.subtract)
        # bbox: (ltrb_raw - tgt)^2 in cols 0..4
        nc.vector.tensor_tensor(out=OC[0:NGmax, 0:4], in0=GT85[0:NGmax, 0:4],
                                in1=A(CF, off_tgt, [[CWF, NGmax], [1, 4]]),
                                op=OP.subtract)
        nc.scalar.activation(out=U[0:NGmax, 0:4], in_=OC[0:NGmax, 0:4],
                             func=AF.Square)
        nc.vector.tensor_tensor(out=U[0:NGmax, :], in0=U[0:NGmax, :],
                                in1=A(CF, off_vn, [[CWF, NGmax], [1, 85]]),
                                op=OP.mult)
        nc.vector.reduce_sum(out=OUTS[0:NGmax, 2:3], in_=U[0:NGmax, :],
                             axis=AX.X)

        # ---- dense conf loss: MEXCL = ~ign * (1 - gtmask) ----
        nc.vector.tensor_tensor(out=MEXCL[:], in0=MASK[:], in1=NGM[:],
                                op=OP.mult)
        nc.vector.tensor_tensor(out=SP[:], in0=SPA[:], in1=SPB[:], op=OP.add)
        nc.vector.tensor_tensor(out=SPM[:], in0=SP[:], in1=MEXCL[:], op=OP.mult)
        nc.vector.reduce_sum(out=OUTS[:, 0:1], in_=SPM[:], axis=AX.X)

        nc.sync.dma_start(out=out[:], in_=OUTS[:])

    return nc


_CACHE = {}
TRACE = False
LAST_RESULTS = None


def _split_multiwait(nc):
    """Walrus codegen on this toolchain supports only one sync-wait command
    per instruction; split multi-wait instructions into single-wait NOPs on
    the same engine."""
    import concourse.mybir as mybir

    if getattr(nc, "_fcos_wait_split", False):
        return
    nc._fcos_wait_split = True
    for bb in nc.m.functions[0].blocks:
        insts = bb.instructions
        for ins in list(insts):
            si = ins.sync_info
            if si is not None and len(si.on_wait) > 1:
                waits = list(si.on_wait)
                idx = insts.index(ins)
                nops = []
                for j, w in enumerate(waits[:-1]):
                    nop = mybir.InstNoOp(name=f"{ins.name}-wsplit{j}", ins=[],
                                         outs=[])
                    nop.engine = ins.engine
                    nop.sync_info = mybir.SyncInfo(on_wait=[w], on_update=[])
                    nops.append(nop)
                ins.sync_info = mybir.SyncInfo(on_wait=[waits[-1]],
                                               on_update=list(si.on_update))
                for nop in reversed(nops):
                    insts.insert(idx, nop)


def kernel(raw, labels, anchors_all, img_size):
    from concourse.bass_utils import run_bass_kernel_spmd

    raw = np.asarray(raw, f32)
    labels_np = np.asarray(labels, f32)
    anchors_np = np.asarray(anchors_all, f32)
    isize = int(img_size)

    per_img, A_imgs, B_imgs, GA, GB, NGmax = _plan(labels_np, anchors_np, isize)
    key = (GA, GB, NGmax, DUP, GPSIMD_TREE, anchors_np.tobytes(), isize)
    if key not in _CACHE:
        _CACHE[key] = _build_program(GA, GB, NGmax, anchors_np.tolist(), isize)
    nc = _CACHE[key]
    _split_multiwait(nc)

    in_maps = [
        _pack_core_inputs(c, per_img, A_imgs, B_imgs, raw, isize, GA, GB, NGmax)
        for c in range(N_CORES)
    ]
    global LAST_RESULTS
    res = run_bass_kernel_spmd(nc, in_maps, list(range(N_CORES)), trace=TRACE)
    LAST_RESULTS = res
    total = np.float64(0.0)
    for c in range(N_CORES):
        o = res.results[c]["out"]
        total += np.sum(o[:, 0], dtype=np.float64)
        total += np.sum(o[:, 2], dtype=np.float64)
    return f32(total)


if __name__ == "__main__":
    import importlib.util

    spec = importlib.util.spec_from_file_location("reference",
                                                  "/root/problem/reference.py")
    ref = importlib.util.module_from_spec(spec)
    spec.loader.exec_module(ref)
    inputs = ref.setup_inputs()
    np_inputs = {k: np.asarray(v) for k, v in inputs.items()}
    got = kernel(**np_inputs)
    print("kernel:", got)


# revision 45
# speedup vs baseline: 3.8107x; 1.0397x over previous
"""Trainium2 Bass kernel for the YOLO/FCOS-layer loss (nn_FCOSLayer_22840636080477).

Sharding: data-parallel over batch, 2 images per NeuronCore x 8 cores, one
SPMD program. Host does label-side preprocessing (anchor matching, scatter
dedup, row-band gt->partition scheduling, constant packing); device does
everything that touches `raw`:

  loss = sum_cells softplus(conf) * (conf_mask & ~gt)          (dense)
       + sum_gtcells [ softplus(conf)-conf                      (sparse, gather)
                      + sum_c (softplus(cls_c) - onehot_c*cls_c)
                      + sum_4 (ltrb_raw - tgt)^2 ]

The ignore mask needs a max-IoU scan of 12288 pred boxes against each
image's gt boxes.  Three structural tricks make it cheap:

1. bf16 + packed last-axis APs -> DVE 2x_1p mode (0.52 ns/elem).
2. Row banding: IoU > 0.6 forces the pred's cell center inside the gt
   box dilated by (2/3)*(wg,hg) (pred-independent bound), so each gt
   only concerns a contiguous band of partitions (partition p holds
   row p//2).  Interval-coloring packs gts into "rounds"; each round
   instruction gives every partition its own gt via per-partition
   consts.  Rounds ~ max-load instead of K.
3. Margin m = min(iw*ih - athg, iw) has the exact sign of the ignore
   condition (valid when gt heights <= 1), killing both clamps; the
   per-pred threshold athp = cthre*areap is compared once at the end:
   ignore <=> max_g m_g > athp.
"""
import sys
import math
import numpy as np

sys.path.insert(0, "/opt/trn_rl_repo")

import ml_dtypes

bf16 = ml_dtypes.bfloat16

N_CLS = 80
nA = 3
STRIDE = 8
IGNORE_THRE = 0.6
EPS = 1e-16
B = 16
K = 50
nG = 64
N_CORES = 8
P = 128
NCELL = nG * nG
f32 = np.float32

# tunables
DUP = 2          # duplicate gt scalars pairwise (bf16 2x packing aid)
DIL = 0.72       # band dilation factor (theory: (1/tau'-1) ~ 0.692 w/ bf16)
GPSIMD_TREE = False  # Pool engine rejects TENSOR_TENSOR (ISA check)
# batched-op sizes (slots per scan op); per-slot ns cost for the DP
G_COST = {16: 10600, 8: 5480, 4: 4510}


def _decompose(R):
    """Split R rounds into batch widths from G_COST minimizing total cost."""
    if R <= 0:
        return []
    best = {0: (0, ())}
    for s in range(1, R + 16):
        cands = []
        for g, c in G_COST.items():
            if s - g >= 0 and (s - g) in best:
                pc, pl = best[s - g]
                cands.append((pc + c, pl + (g,)))
        if cands:
            best[s] = min(cands)
    return list(min(best[s] for s in best if s >= R)[1])


# ---------------------------------------------------------------------------
# host-side label math (replicates reference.py semantics in f32 numpy)
# ---------------------------------------------------------------------------

def _host_precompute(labels, anchors_all, img_size):
    labels = np.asarray(labels, f32)
    anchors_all = np.asarray(anchors_all, f32)
    img_size = f32(img_size)
    anchors = anchors_all[:nA]
    norm_anch = anchors_all / img_size
    anch_w_n = anchors[:, 0] / img_size

    per_img = []
    for bb in range(B):
        lab = labels[bb]
        valid_row = lab.sum(-1) > 0
        tw, th = lab[:, 3], lab[:, 4]
        inter = np.minimum(tw[:, None], norm_anch[:, 0]) * np.minimum(
            th[:, None], norm_anch[:, 1]
        )
        union = tw[:, None] * th[:, None] + norm_anch[:, 0] * norm_anch[:, 1] - inter
        an_iou = inter / (union + f32(EPS))
        best_n_all = np.argmax(an_iou, axis=-1)
        best_n = best_n_all % nA
        valid = valid_row & (best_n_all < nA)

        ks = np.where(valid_row)[0]
        gcx, gcy, gw, gh = lab[ks, 1], lab[ks, 2], lab[ks, 3], lab[ks, 4]
        gt = dict(
            tlx=(gcx - gw / 2).astype(f32),
            tly=(gcy - gh / 2).astype(f32),
            brx=(gcx + gw / 2).astype(f32),
            bry=(gcy + gh / 2).astype(f32),
            area=(gw * gh).astype(f32),
            gh=gh.astype(f32),
        )

        tx = lab[:, 1] * nG
        ty = lab[:, 2] * nG
        ti = tx.astype(np.int32)
        tj = ty.astype(np.int32)
        tcls = lab[:, 0].astype(np.int32)
        lw, lh = lab[:, 3] * nG, lab[:, 4] * nG
        xc = np.floor(tx) + f32(0.5)
        yc = np.floor(ty) + f32(0.5)
        lab_ltrb = (
            np.maximum(
                np.stack(
                    [xc - (tx - lw / 2), yc - (ty - lh / 2),
                     (tx + lw / 2) - xc, (ty + lh / 2) - yc], -1),
                0.0,
            ) / f32(nG)
        ).astype(f32)
        cellmap = {}
        for k in range(K):
            if not valid[k]:
                continue
            key = (int(best_n[k]), int(tj[k]), int(ti[k]))
            tgt = np.log(lab_ltrb[k] / anch_w_n[best_n[k]] + f32(EPS)).astype(f32)
            if key not in cellmap:
                cellmap[key] = dict(tgt=tgt, cls=set([int(tcls[k])]))
            else:
                cellmap[key]["tgt"] = tgt  # scatter last-wins
                cellmap[key]["cls"].add(int(tcls[k]))

        # row-banded partition sets for the iou scan.  IoU > 0.6 forces the
        # cell center inside the gt box dilated by (2/3)*(wg,hg); partition
        # p holds row p//2, half p%2 (i<32 on even p, i>=32 on odd p).
        gt_n = len(ks)
        gw64 = (gt["brx"] - gt["tlx"]).astype(np.float64)
        gh64 = gt["gh"].astype(np.float64)
        ylo = gt["tly"].astype(np.float64) - DIL * gh64
        yhi = gt["bry"].astype(np.float64) + DIL * gh64
        xlo = gt["tlx"].astype(np.float64) - DIL * gw64
        xhi = gt["brx"].astype(np.float64) + DIL * gw64
        # extra pad row only for small boxes, where the DIL slack over the
        # theoretical 0.692 factor is below the bf16 coordinate noise
        rp_y = (gh64 < 0.1).astype(int)
        rp_x = (gw64 < 0.1).astype(int)
        j0 = np.clip(np.floor(ylo * nG - 0.5).astype(int) - rp_y, 0, nG - 1)
        j1 = np.clip(np.ceil(yhi * nG - 0.5).astype(int) + rp_y, 0, nG - 1)
        i0 = np.clip(np.floor(xlo * nG - 0.5).astype(int) - rp_x, 0, nG - 1)
        i1 = np.clip(np.ceil(xhi * nG - 0.5).astype(int) + rp_x, 0, nG - 1)
        # greedy set-packing into rounds (128-bit occupancy masks).
        # Images with no in-layer gt keep conf_loss_mask all-True in the
        # reference; skip their schedule so ACC stays at -100 -> ~ign = 1.
        sched = []  # (k, round, p0, p1, step)
        occ = []
        for k2 in sorted(range(gt_n if valid.any() else 0),
                         key=lambda q: (j0[q] - j1[q], q)):
            lo, hi = 2 * int(j0[k2]), 2 * int(j1[k2]) + 2
            if i1[k2] < 32:
                lo, step = lo, 2          # even partitions only
            elif i0[k2] >= 32:
                lo, step = lo + 1, 2      # odd partitions only
            else:
                step = 1
            mask = 0
            for p in range(lo, hi, step):
                mask |= 1 << p
            for r, o in enumerate(occ):
                if not (o & mask):
                    occ[r] |= mask
                    sched.append((k2, r, lo, hi, step))
                    break
            else:
                occ.append(mask)
                sched.append((k2, len(occ) - 1, lo, hi, step))
        per_img.append(dict(K=gt_n, gt=gt, cellmap=cellmap,
                            has_valid=bool(valid.any()),
                            sched=sched, R=len(occ)))
    return per_img


def _plan(labels, anchors_all, img_size):
    per_img = _host_precompute(labels, anchors_all, img_size)
    Rs = [info["R"] for info in per_img]
    order = sorted(range(B), key=lambda i: -Rs[i])
    A_imgs = order[:N_CORES]
    B_imgs = order[N_CORES:][::-1]  # pair big-A with small-B
    RA = max((Rs[i] for i in A_imgs), default=0)
    RB = max((Rs[i] for i in B_imgs), default=0)
    GA = tuple(_decompose(max(RA, 1)))
    GB = tuple(_decompose(RB))  # empty when no B image has in-layer gts
    NGmax = 1
    for c in range(N_CORES):
        n = (len(per_img[A_imgs[c]]["cellmap"])
             + len(per_img[B_imgs[c]]["cellmap"]))
        NGmax = max(NGmax, n)
    NGmax = min(-(-NGmax // 8) * 8, P)
    return per_img, A_imgs, B_imgs, GA, GB, NGmax


# ---------------------------------------------------------------------------
# per-core input packing
# ---------------------------------------------------------------------------

def _pack_core_inputs(core, per_img, A_imgs, B_imgs, raw, img_size,
                      GA, GB, NGmax):
    img_size = f32(img_size)
    thre = f32(IGNORE_THRE)
    cthre = (thre / (f32(1.0) + thre)).astype(f32)
    imgs = [A_imgs[core], B_imgs[core]]
    SA, SB = sum(GA), sum(GB)

    # full-channel raw (gather source) + pre-packed 5-channel bf16 block
    rawsh = np.ascontiguousarray(raw[imgs]).reshape(2, 255, NCELL)
    # raw5 [P, 960]: col = im*480 + ch*96 + a*32 + c ; cell q = 32p + c
    r6 = rawsh.reshape(2, nA, 85, P, 32)[:, :, 0:5]       # im,a,ch,p,c
    raw5 = np.ascontiguousarray(
        r6.transpose(3, 0, 2, 1, 4).reshape(P, 960)).astype(bf16)

    # scan consts (bf16): per-(image, batch) g5 blocks + xyc
    # block for batch of g slots: col = comp*(g*DUP) + slot*DUP + dup,
    # per-partition values from the round schedule.  Per-batch blocks are
    # contiguous so each batch's first op waits only on its own DMA.
    blocks = []
    for glist, im in zip((GA, GB), imgs):
        info = per_img[im]
        sl = sum(glist)
        g5 = np.zeros((5, sl, P, DUP), f32)
        g5[4] = 1.0  # pad: prod - 1 <= 0 always (boxes within [0,1])
        gt = info["gt"]
        for k, r, p0, p1, step in info["sched"]:
            sl_ = slice(p0, p1, step)
            g5[0, r, sl_] = gt["tlx"][k]
            g5[1, r, sl_] = gt["tly"][k]
            g5[2, r, sl_] = gt["brx"][k]
            g5[3, r, sl_] = gt["bry"][k]
            g5[4, r, sl_] = cthre * (gt["area"][k] + f32(EPS))
        s0 = 0
        for g in glist:
            blk = g5[:, s0:s0 + g]  # (5, g, P, DUP)
            blocks.append(blk.transpose(2, 0, 1, 3).reshape(P, 5 * g * DUP))
            s0 += g

    # xyc [P,192]: col = comp*96 + aq, cell q = 32p + (aq % 32)
    pidx = np.arange(P)[:, None]
    aqidx = np.arange(96)[None, :]
    q = 32 * pidx + (aqidx % 32)
    gx = (q % nG).astype(f32)
    gy = (q // nG).astype(f32)
    xyc = np.concatenate([(gx + f32(0.5)) / f32(nG), (gy + f32(0.5)) / f32(nG)],
                         axis=1).astype(f32)
    consts_bf = np.ascontiguousarray(
        np.concatenate(blocks + [xyc], axis=1)).astype(bf16)

    # tail consts: ngm = 1 - gtmask (bf16); f32: tgt85, onehot, validng, gidx
    gtmask = np.zeros((P, 192), f32)
    cells = []
    for iml, im in enumerate(imgs):
        info = per_img[im]
        for (a, j, i), d in info["cellmap"].items():
            cq = j * nG + i
            gtmask[cq // 32, iml * 96 + a * 32 + cq % 32] = 1.0
            cells.append((iml, a, cq, d["tgt"], d["cls"]))
    ngm = np.ascontiguousarray(1.0 - gtmask).astype(bf16)

    tgt85 = np.zeros((P, 85), f32)
    onehot = np.zeros((P, 85), f32)
    validng = np.zeros((P, 85), f32)
    gidx = np.zeros((P, NGmax), np.int32)
    for g, (iml, a, cq, tgt, clsset) in enumerate(cells):
        tgt85[g, 0:4] = tgt
        onehot[g, 4] = 1.0
        for c in clsset:
            onehot[g, 5 + c] = 1.0
        validng[g, :] = 1.0
        gidx[:, g] = (iml * 255 + a * 85) * NCELL + cq
    consts_f = np.ascontiguousarray(np.concatenate(
        [tgt85, onehot, validng, gidx.view(f32)], axis=1)).astype(f32)
    return dict(rawsh=rawsh, raw5=raw5, cbf=consts_bf, cf=consts_f, ngm=ngm)


# ---------------------------------------------------------------------------
# device program
# ---------------------------------------------------------------------------

def _build_program(GA, GB, NGmax, anchors_all, img_size):
    import concourse.bass as bass
    import concourse.mybir as mybir
    from concourse.tile import TileContext

    dtb = mybir.dt.bfloat16
    dtf = mybir.dt.float32
    AF = mybir.ActivationFunctionType
    OP = mybir.AluOpType
    AX = mybir.AxisListType
    cthre = float(IGNORE_THRE / (1.0 + IGNORE_THRE))
    SA, SB = sum(GA), sum(GB)
    W = max(GA + GB) * 96  # widest batch; tiles are sized for it

    nc = bass.Bass()

    lnaw_vals = [float(math.log(anchors_all[a][0] / img_size)) for a in range(nA)]

    rawsh = nc.declare_dram_parameter("rawsh", [2, 255, NCELL], dtf, False)
    raw5d = nc.declare_dram_parameter("raw5", [P, 960], dtb, False)
    CWS = 5 * (SA + SB) * DUP + 192
    blk_offs = []
    cur = 0
    for g in GA + GB:
        blk_offs.append(cur)
        cur += 5 * g * DUP
    off_xyc = cur
    cbf = nc.declare_dram_parameter("cbf", [P, CWS], dtb, False)
    CWF = 85 * 3 + NGmax
    off_tgt, off_oh, off_vn = 0, 85, 170
    off_gidx = 255
    cf = nc.declare_dram_parameter("cf", [P, CWF], dtf, False)
    ngmd = nc.declare_dram_parameter("ngm", [P, 192], dtb, False)
    out = nc.declare_dram_parameter("out", [P, 4], dtf, True)

    def A(t, offset, dims):
        h = t.tensor if hasattr(t, "tensor") else t
        return bass.AP(h, offset, dims)

    with TileContext(nc) as tc, \
            tc.tile_pool(name="main", bufs=1) as pool:
        RAW = pool.tile([P, 960], dtb, name="RAW")
        CS = pool.tile([P, CWS], dtb, name="CS")
        CF = pool.tile([P, CWF], dtf, name="CF")
        E = pool.tile([P, 768], dtb, name="E")
        TL = pool.tile([P, 384], dtb, name="TL")
        BR = pool.tile([P, 384], dtb, name="BR")
        WH = pool.tile([P, 384], dtb, name="WH")
        AREA = pool.tile([P, 192], dtb, name="AREA")
        ATH = pool.tile([P, 192], dtb, name="ATH")
        LNAW = pool.tile([P, 4], dtf, name="LNAW")
        IX = pool.tile([P, W], dtb, name="IX")
        AXT = pool.tile([P, W], dtb, name="AXT")
        IY = pool.tile([P, W], dtb, name="IY")
        AY = pool.tile([P, W], dtb, name="AY")
        IW = pool.tile([P, W], dtb, name="IW")
        IH = pool.tile([P, W], dtb, name="IH")
        PROD = pool.tile([P, W], dtb, name="PROD")
        T = [pool.tile([P, W], dtb, name=f"T{i}") for i in range(2)]
        TR1 = pool.tile([P, W // 2], dtb, name="TR1")
        TR2 = pool.tile([P, W // 4], dtb, name="TR2")
        ACC = pool.tile([P, 192], dtb, name="ACC")
        NGM = pool.tile([P, 192], dtb, name="NGM")
        MASK = pool.tile([P, 192], dtb, name="MASK")
        MEXCL = pool.tile([P, 192], dtb, name="MEXCL")
        SP = pool.tile([P, 192], dtf, name="SP")
        SPA = pool.tile([P, 192], dtf, name="SPA")
        SPB = pool.tile([P, 192], dtf, name="SPB")
        SPM = pool.tile([P, 192], dtf, name="SPM")
        GT85 = pool.tile([P, 85], dtf, name="GT85")
        U = pool.tile([P, 85], dtf, name="U")
        SPC = pool.tile([P, 85], dtf, name="SPC")
        OC = pool.tile([P, 85], dtf, name="OC")
        SPD = pool.tile([P, 85], dtf, name="SPD")
        OUTS = pool.tile([P, 4], dtf, name="OUTS")

        # ---- input loads.  The sync (SP) DGE queue is by far the fastest;
        # put the critical-path loads there in need-order.  CF/NGM (tail
        # consumers) trickle in on the slower gpsimd queue.  raw5 is split
        # per image so image A's pred prep starts ~2us earlier.
        nc.sync.dma_start(out=RAW[:, 0:480], in_=A(raw5d, 0, [[960, P], [1, 480]]))
        nc.sync.dma_start(out=CS[:, off_xyc:off_xyc + 192],
                          in_=A(cbf, off_xyc, [[CWS, P], [1, 192]]))
        b0_end = blk_offs[1] if len(blk_offs) > 1 else off_xyc
        nc.sync.dma_start(out=CS[:, 0:b0_end],
                          in_=A(cbf, 0, [[CWS, P], [1, b0_end]]))
        if b0_end < off_xyc:
            nc.sync.dma_start(out=CS[:, b0_end:off_xyc],
                              in_=A(cbf, b0_end, [[CWS, P], [1, off_xyc - b0_end]]))
        if GB:
            nc.sync.dma_start(out=RAW[:, 480:960],
                              in_=A(raw5d, 480, [[960, P], [1, 480]]))
        else:
            # image B contributes only its conf channel (no scan)
            nc.sync.dma_start(out=RAW[:, 864:960],
                              in_=A(raw5d, 864, [[960, P], [1, 96]]))
        nc.gpsimd.dma_start(out=CF[:], in_=cf[:])
        nc.gpsimd.dma_start(out=NGM[:], in_=ngmd[:])
        for a in range(nA):
            nc.gpsimd.memset(LNAW[:, a:a + 1], lnaw_vals[a])

        # ---- sparse gt-cell gather (gpsimd queue, before batch trees; its
        # consumers are emitted after the scan so they don't stall DVE/ACT) ----
        nc.vector.memset(OUTS[:], 0.0)
        nc.gpsimd.indirect_dma_start(
            out=GT85[0:NGmax, 0:85],
            out_offset=None,
            in_=A(rawsh, 0, [[1, (2 * 255 - 85 + 1) * NCELL], [NCELL, 85], [1, 1]]),
            in_offset=bass.IndirectOffsetOnAxis(
                ap=A(CF, off_gidx, [[CWF, 1], [1, NGmax]]).bitcast(
                    mybir.dt.int32),
                axis=0),
        )

        # ---- pred prep (per image, so image A's scan starts before image
        # B's raw half lands) + banded iou scan ----
        nc.vector.memset(ACC[:], -100.0)
        treng = nc.gpsimd if GPSIMD_TREE else nc.vector
        xyc_b = A(CS, off_xyc, [[CWS, P], [96, 2], [1, 96]])

        def gt_ap(blk, comp, n):
            base = blk + comp * (n * DUP)
            return A(CS, base, [[CWS, P], [DUP, n], [0, 96 // DUP], [1, DUP]])

        def pred_ap(t, comp, im, n):
            return A(t, comp * 192 + im * 96, [[384, P], [0, n], [1, 96]])

        for im, glist in ((0, GA), (1, GB)):
            if not glist:
                # no in-layer gts on this slot: mask stays all-pass
                nc.vector.memset(MASK[:, im * 96:(im + 1) * 96], 1.0)
                continue
            for a in range(3):
                nc.scalar.activation(
                    out=A(E, im * 384 + a * 32, [[768, P], [96, 4], [1, 32]]),
                    in_=A(RAW, im * 480 + a * 32, [[960, P], [96, 4], [1, 32]]),
                    func=AF.Exp,
                    bias=A(LNAW, a, [[4, P], [1, 1]]),
                )
            e_lt = A(E, im * 384, [[768, P], [96, 2], [1, 96]])
            e_rb = A(E, im * 384 + 192, [[768, P], [96, 2], [1, 96]])
            quad = [[384, P], [192, 2], [1, 96]]
            nc.vector.tensor_tensor(out=A(TL, im * 96, quad), in0=xyc_b,
                                    in1=e_lt, op=OP.subtract)
            nc.vector.tensor_tensor(out=A(BR, im * 96, quad), in0=xyc_b,
                                    in1=e_rb, op=OP.add)
            nc.vector.tensor_tensor(out=A(WH, im * 96, quad), in0=e_lt,
                                    in1=e_rb, op=OP.add)
            sl96 = slice(im * 96, im * 96 + 96)
            nc.vector.tensor_tensor(out=AREA[:, sl96], in0=WH[:, sl96],
                                    in1=WH[:, 192 + im * 96:192 + im * 96 + 96],
                                    op=OP.mult)
            nc.vector.tensor_scalar(out=ATH[:, sl96], in0=AREA[:, sl96],
                                    scalar1=cthre, scalar2=None, op0=OP.mult)
            for bi, g in enumerate(glist):
                wb = g * 96
                blk = blk_offs[(0 if im == 0 else len(GA)) + bi]
                tt = T[bi % 2]
                nc.vector.tensor_tensor(out=IX[:, 0:wb],
                                        in0=pred_ap(TL, 0, im, g),
                                        in1=gt_ap(blk, 0, g), op=OP.max)
                nc.vector.tensor_tensor(out=AXT[:, 0:wb],
                                        in0=pred_ap(BR, 0, im, g),
                                        in1=gt_ap(blk, 2, g), op=OP.min)
                nc.vector.tensor_tensor(out=IW[:, 0:wb], in0=AXT[:, 0:wb],
                                        in1=IX[:, 0:wb], op=OP.subtract)
                nc.vector.tensor_tensor(out=IY[:, 0:wb],
                                        in0=pred_ap(TL, 1, im, g),
                                        in1=gt_ap(blk, 1, g), op=OP.max)
                nc.vector.tensor_tensor(out=AY[:, 0:wb],
                                        in0=pred_ap(BR, 1, im, g),
                                        in1=gt_ap(blk, 3, g), op=OP.min)
                nc.vector.tensor_tensor(out=IH[:, 0:wb], in0=AY[:, 0:wb],
                                        in1=IY[:, 0:wb], op=OP.subtract)
                # clamp ih only: iw<0 or ih<0 both give prod <= 0 < ath
                nc.vector.tensor_scalar(out=IH[:, 0:wb], in0=IH[:, 0:wb],
                                        scalar1=0.0, scalar2=None, op0=OP.max)
                nc.vector.tensor_tensor(out=PROD[:, 0:wb], in0=IW[:, 0:wb],
                                        in1=IH[:, 0:wb], op=OP.mult)
                nc.vector.tensor_tensor(out=tt[:, 0:wb], in0=PROD[:, 0:wb],
                                        in1=gt_ap(blk, 4, g),
                                        op=OP.subtract)
                # tree-max margins over rounds -> ACC
                width, src, flip = wb, tt, 0
                while width > 96:
                    h = width // 2
                    dst = (TR1, TR2)[flip % 2]
                    treng.tensor_tensor(out=dst[:, 0:h], in0=src[:, 0:h],
                                        in1=src[:, h:2 * h], op=OP.max)
                    src, width, flip = dst, h, flip + 1
                acc_sl = ACC[:, im * 96:(im + 1) * 96]
                treng.tensor_tensor(out=acc_sl, in0=acc_sl, in1=src[:, 0:96],
                                    op=OP.max)
            # notign = (max margin <= athp) for this image
            nc.vector.tensor_tensor(out=MASK[:, im * 96:(im + 1) * 96],
                                    in0=ACC[:, im * 96:(im + 1) * 96],
                                    in1=ATH[:, im * 96:(im + 1) * 96],
                                    op=OP.is_le)

        # conf softplus on ACT while DVE scans
        conf_view = A(RAW, 384, [[960, P], [480, 2], [1, 96]])
        sp_flat = [[192, P], [96, 2], [1, 96]]
        nc.scalar.activation(out=A(SPA, 0, sp_flat), in_=conf_view, func=AF.Abs)
        nc.scalar.activation(out=SPB[:], in_=SPA[:], func=AF.Exp, scale=-1.0)
        nc.scalar.activation(out=SPA[:], in_=SPB[:], func=AF.Ln, bias=1.0)
        nc.scalar.activation(out=A(SPB, 0, sp_flat), in_=conf_view, func=AF.Relu)

        # ---- sparse gt-cell terms (consumers of the early gather) ----
        # softplus(z) = ln(1 + exp(-|z|)) + relu(z) over cols 4..85
        nc.scalar.activation(out=SPC[0:NGmax, 4:85], in_=GT85[0:NGmax, 4:85],
                             func=AF.Abs)
        nc.scalar.activation(out=SPD[0:NGmax, 4:85], in_=SPC[0:NGmax, 4:85],
                             func=AF.Exp, scale=-1.0)
        nc.scalar.activation(out=SPC[0:NGmax, 4:85], in_=SPD[0:NGmax, 4:85],
                             func=AF.Ln, bias=1.0)
        nc.scalar.activation(out=SPD[0:NGmax, 4:85], in_=GT85[0:NGmax, 4:85],
                             func=AF.Relu)
        nc.vector.tensor_tensor(out=OC[0:NGmax, 4:85], in0=GT85[0:NGmax, 4:85],
                                in1=A(CF, off_oh + 4, [[CWF, NGmax], [1, 81]]),
                                op=OP.mult)
        nc.vector.tensor_tensor(out=U[0:NGmax, 4:85], in0=SPC[0:NGmax, 4:85],
                                in1=SPD[0:NGmax, 4:85], op=OP.add)
        nc.vector.tensor_tensor(out=U[0:NGmax, 4:85], in0=U[0:NGmax, 4:85],
                                in1=OC[0:NGmax, 4:85], op=OP.subtract)
        # bbox: (ltrb_raw - tgt)^2 in cols 0..4
        nc.vector.tensor_tensor(out=OC[0:NGmax, 0:4], in0=GT85[0:NGmax, 0:4],
                                in1=A(CF, off_tgt, [[CWF, NGmax], [1, 4]]),
                                op=OP.subtract)
        nc.scalar.activation(out=U[0:NGmax, 0:4], in_=OC[0:NGmax, 0:4],
                             func=AF.Square)
        nc.vector.tensor_tensor(out=U[0:NGmax, :], in0=U[0:NGmax, :],
                                in1=A(CF, off_vn, [[CWF, NGmax], [1, 85]]),
                                op=OP.mult)
        nc.vector.reduce_sum(out=OUTS[0:NGmax, 2:3], in_=U[0:NGmax, :],
                             axis=AX.X)

        # ---- dense conf loss: MEXCL = ~ign * (1 - gtmask), per image so
        # image B's half (mask all-pass) finishes during image A's scan ----
        nc.vector.tensor_tensor(out=SP[:], in0=SPA[:], in1=SPB[:], op=OP.add)
        for im in (1, 0):
            sl96 = slice(im * 96, im * 96 + 96)
            nc.vector.tensor_tensor(out=MEXCL[:, sl96], in0=MASK[:, sl96],
                                    in1=NGM[:, sl96], op=OP.mult)
            nc.vector.tensor_tensor(out=SPM[:, sl96], in0=SP[:, sl96],
                                    in1=MEXCL[:, sl96], op=OP.mult)
            nc.vector.reduce_sum(out=OUTS[:, im:im + 1], in_=SPM[:, sl96],
                                 axis=AX.X)

        nc.sync.dma_start(out=out[:], in_=OUTS[:])

    return nc


_CACHE = {}
TRACE = False
LAST_RESULTS = None


def _split_multiwait(nc):
    """Walrus codegen on this toolchain supports only one sync-wait command
    per instruction; split multi-wait instructions into single-wait NOPs on
    the same engine."""
    import concourse.mybir as mybir

    if getattr(nc, "_fcos_wait_split", False):
        return
    nc._fcos_wait_split = True
    for bb in nc.m.functions[0].blocks:
        insts = bb.instructions
        for ins in list(insts):
            si = ins.sync_info
            if si is not None and len(si.on_wait) > 1:
                waits = list(si.on_wait)
                idx = insts.index(ins)
                nops = []
                for j, w in enumerate(waits[:-1]):
                    nop = mybir.InstNoOp(name=f"{ins.name}-wsplit{j}", ins=[],
                                         outs=[])
                    nop.engine = ins.engine
                    nop.sync_info = mybir.SyncInfo(on_wait=[w], on_update=[])
                    nops.append(nop)
                ins.sync_info = mybir.SyncInfo(on_wait=[waits[-1]],
                                               on_update=list(si.on_update))
                for nop in reversed(nops):
                    insts.insert(idx, nop)


def kernel(raw, labels, anchors_all, img_size):
    from concourse.bass_utils import run_bass_kernel_spmd

    raw = np.asarray(raw, f32)
    labels_np = np.asarray(labels, f32)
    anchors_np = np.asarray(anchors_all, f32)
    isize = int(img_size)

    per_img, A_imgs, B_imgs, GA, GB, NGmax = _plan(labels_np, anchors_np, isize)
    key = (GA, GB, NGmax, DUP, GPSIMD_TREE, anchors_np.tobytes(), isize)
    if key not in _CACHE:
        _CACHE[key] = _build_program(GA, GB, NGmax, anchors_np.tolist(), isize)
    nc = _CACHE[key]
    _split_multiwait(nc)

    in_maps = [
        _pack_core_inputs(c, per_img, A_imgs, B_imgs, raw, isize, GA, GB, NGmax)
        for c in range(N_CORES)
    ]
    global LAST_RESULTS
    res = run_bass_kernel_spmd(nc, in_maps, list(range(N_CORES)), trace=TRACE)
    LAST_RESULTS = res
    total = np.float64(0.0)
    for c in range(N_CORES):
        o = res.results[c]["out"]
        total += np.sum(o[:, 0:3], dtype=np.float64)
    return f32(total)


if __name__ == "__main__":
    import importlib.util

    spec = importlib.util.spec_from_file_location("reference",
                                                  "/root/problem/reference.py")
    ref = importlib.util.module_from_spec(spec)
    spec.loader.exec_module(ref)
    inputs = ref.setup_inputs()
    np_inputs = {k: np.asarray(v) for k, v in inputs.items()}
    got = kernel(**np_inputs)
    print("kernel:", got)
